# revision 22
# baseline (speedup 1.0000x reference)
"""AGDNConv (3-hop attention diffusion GNN) on 8 trn2 NeuronCores.

Sharding: edges partitioned by dst-owner (owner = dst // 12544); node tables
replicated (P0 matmul) or AllGathered per hop. Per-core segment sums use a
degree-class slot layout so they become tree/strided tensor_reduce ops.
Attention softmax uses the max-free identity
  a[e] = exp(e_e) * rsqrt(s_dst[dst_e]) * rsqrt(s_src[src_e]).

v2 layout decisions (all driven by the DMA descriptor cost model):
 - One bf16 node table FTELER [NP_+1, 58]: cols 0:48 ft in d-major (d,h)
   order, 48:51 rss (scattered in after the src-side AllGather), 51:54 el,
   54:57 er.  Hop-1 gathers cols 0:51 in ONE 102B descriptor per edge
   (ft + rss together); score passes gather only the 6B other-side score.
 - Key-side score values are broadcast per node group (no per-edge gather);
   per-node sums (SD) stay in SBUF, so the old "stage C" disappears.
 - No mask tensors: the pad row NP_ has el=er=-60 so exp==0 naturally, and
   rss==0 on the pad row kills pad contributions in hops 2/3.
 - d-major feature order makes every big DVE multiply packed-last (2x/4x
   DVE modes); segment sums are tree adds (packed) instead of strided 1x.

Execution: the axon terminal's NRT shim does not implement dynamic-AP
(indirect) DMA - it returns garbage data for gathers and wedges the device
on larger elements (verified empirically; see dev notes). kernel() therefore
runs the cycle-accurate MultiCoreSim (single worker, real collectives) by
default, which validates and times the exact Bass program. Set AGDN_HW=1 to
attempt real-HW execution via PJRT on terminals with a fixed runtime.
"""
import sys
sys.path.insert(0, "/opt/trn_rl_repo")
import os
import numpy as np
import ml_dtypes

USE_COLL = os.environ.get("AGDN_NOCOLL", "") != "1"
STAGE = int(os.environ.get("AGDN_STAGE", "9"))
P = 128
N = 100000
IN = 128
H = 3
D = 16
HD = 48
K = 3
NEG = 0.2
EPS = 1e-9
NCORES = 8
NS = 12544
NP_ = NCORES * NS
ROWW = 256         # FTELER row width (bf16 elements; 512B rows avoid the
                   # small-descriptor 2x latency multiplier on gathers)
PADV = -60.0       # pad-row el/er value: exp(leaky(-60+x)) ~ 0
CLASSES = [4, 8, 12, 16, 20, 24, 32, 40, 48, 64, 96, 128]
TARGET = 192       # chunk width target, in slots


# ---------------------------------------------------------------- host prep
def _pack_side(key_node, other_node, n_lo):
    loc = (key_node - n_lo).astype(np.int64)
    order = np.argsort(loc, kind="stable")
    loc_s = loc[order]
    other_s = other_node[order]
    deg = np.bincount(loc_s, minlength=NS)
    assert deg.max() <= CLASSES[-1], f"degree {deg.max()} exceeds max class"
    starts = np.concatenate([[0], np.cumsum(deg)[:-1]])
    cls_of = np.full(NS, -1, np.int64)
    lo = 0
    for ci, C in enumerate(CLASSES):
        cls_of[(deg > lo) & (deg <= C)] = ci
        lo = C
    members = [np.where(cls_of == ci)[0] for ci in range(len(CLASSES))]
    zeros = np.where(deg == 0)[0]
    return dict(members=members, zeros=zeros, deg=deg, starts=starts,
                other_s=other_s)


def _layout(counts_max, gz_max):
    G, vg0, je0, plan = [], [], [], []
    v, j = 0, 0
    for ci, C in enumerate(CLASSES):
        g = int(np.ceil(counts_max[ci] / P))
        G.append(g)
        vg0.append(v)
        je0.append(j)
        cols = g * C
        step = max(C, (TARGET // C) * C)
        s = 0
        while s < cols:
            w = min(step, cols - s)
            plan.append((j + s, w, C))
            s += w
        v += g
        j += g * C
    vg0.append(v)
    # zeros region + one spare always-unassigned group (its CUR rows stay 0)
    v += int(np.ceil(gz_max / P)) + 1
    return G, vg0, je0, max(v, 1), max(((j + 3) // 4) * 4, 4), plan


def _fill_core(pack, G, vg0, je0, NV, NTE, n_lo):
    ioth = np.full((P, NTE), NP_, np.int32)    # pad -> FTELER pad row
    vrow = np.full(NS, -1, np.int64)
    deg, starts, other_s = pack["deg"], pack["starts"], pack["other_s"]
    for ci, C in enumerate(CLASSES):
        mem = pack["members"][ci]
        g_c = max(G[ci], 1)
        for i, nl in enumerate(mem):
            g, p = i % g_c, i // g_c
            vrow[nl] = p * NV + vg0[ci] + g
            d, s0 = deg[nl], starts[nl]
            je = je0[ci] + g * C
            ioth[p, je:je + d] = other_s[s0:s0 + d]
    mem = pack["zeros"]
    gz = max(int(np.ceil(len(mem) / P)), 1)
    for i, nl in enumerate(mem):
        vrow[nl] = (i // gz) * NV + vg0[len(CLASSES)] + i % gz
    return ioth, vrow


def host_prep(src, dst):
    sides = {}
    for side, key, oth in (("d", dst, src), ("s", src, dst)):
        packs = []
        for c in range(NCORES):
            m = (key >= c * NS) & (key < (c + 1) * NS)
            packs.append(_pack_side(key[m], oth[m], c * NS))
        counts_max = np.max(
            np.array([[len(p) for p in pk["members"]] for pk in packs]), axis=0)
        gz_max = max(len(pk["zeros"]) for pk in packs)
        G, vg0, je0, NV, NTE, plan = _layout(counts_max, gz_max)
        cores, vmap = [], np.zeros(NP_ + 1, np.int64)
        for c in range(NCORES):
            ioth, vrow = _fill_core(packs[c], G, vg0, je0, NV, NTE, c * NS)
            cores.append(dict(ioth=ioth, vrow=vrow))
            vmap[c * NS:(c + 1) * NS] = c * (P * NV) + vrow
        # pad/ghost entries -> this side's guaranteed-zero row of core 0
        # (the spare group; never assigned by _fill_core)
        zrow = [c * (P * NV) + 127 * NV + (NV - 1) for c in range(NCORES)]
        vmap[vmap < 0] = zrow[0]
        sides[side] = dict(NV=NV, NTE=NTE, plan=plan, cores=cores, vmap=vmap,
                           zrow=zrow)
    return sides


def _vg_lookup(plan):
    lk, vg, last_C, cj, cvg = {}, 0, None, None, None
    for (j0, nj, C) in plan:
        if C != last_C:
            cj, cvg, last_C = j0, vg, C
        lk[(j0, C)] = cvg + (j0 - cj) // C
        vg = cvg + (j0 - cj + nj) // C
    return lk


# ---------------------------------------------------------------- device
def build_nc(NVD, NTED, pland, NVS, NTES, plans):
    import concourse.bass as bass
    import concourse.bacc as bacc
    import concourse.mybir as mybir
    import concourse.tile as tile
    f32, bf16, i32 = mybir.dt.float32, mybir.dt.bfloat16, mybir.dt.int32
    AT, AF, AX = mybir.AluOpType, mybir.ActivationFunctionType, mybir.AxisListType
    IOA = bass.IndirectOffsetOnAxis
    NSVD, NSVS = P * NVD, P * NVS
    lk_d, lk_s = _vg_lookup(pland), _vg_lookup(plans)

    nc = bacc.Bacc("TRN2", target_bir_lowering=False, debug=False,
                   num_devices=NCORES)
    featT = nc.dram_tensor("featT", [P, NP_], bf16, kind="ExternalInput")
    W_in = nc.dram_tensor("W_in", [P, 54], bf16, kind="ExternalInput")
    hop_lr = nc.dram_tensor("hop_lr", [P, 2 * HD], f32, kind="ExternalInput")
    scales4 = nc.dram_tensor("scales4", [P, (K + 1) * HD], f32, kind="ExternalInput")
    offpos4 = nc.dram_tensor("offpos4", [P, (K + 1) * HD], f32, kind="ExternalInput")
    iOTHd = nc.dram_tensor("iOTHd", [P, NTED], i32, kind="ExternalInput")
    iOTHs = nc.dram_tensor("iOTHs", [P, NTES], i32, kind="ExternalInput")
    iCUR = nc.dram_tensor("iCUR", [P, NTED], i32, kind="ExternalInput")
    iQd = nc.dram_tensor("iQd", [P, NVD], i32, kind="ExternalInput")
    iQs = nc.dram_tensor("iQs", [P, NVS], i32, kind="ExternalInput")
    iSCAT = nc.dram_tensor("iSCAT", [P, NCORES * NVS], i32, kind="ExternalInput")
    out_t = nc.dram_tensor("out", [NSVD, HD], f32, kind="ExternalOutput")

    FTELER = nc.dram_tensor("FTELER", [NP_ + 1, ROWW], bf16, kind="Internal")
    SSh = nc.dram_tensor("SSh", [NSVS, 4], bf16, kind="Internal")
    SS = nc.dram_tensor("SS", [NCORES * NSVS, 4], bf16, kind="Internal",
                        addr_space="Shared")
    f8 = mybir.dt.float8e4
    CURSH = nc.dram_tensor("CURSH", [NSVD, HD], f8, kind="Internal")
    CURG = [nc.dram_tensor(f"CURG{k}", [NCORES * NSVD, HD], f8,
                           kind="Internal", addr_space="Shared")
            for k in range(K - 1)]
    HSTK = [nc.dram_tensor(f"HSTK{k}", [NSVD, HD], f32, kind="Internal")
            for k in range(K)]
    HQD = nc.dram_tensor("HQD", [NSVD, HD], f32, kind="Internal")
    rg = [list(range(NCORES))]

    with tile.TileContext(nc) as tc:
        with tc.tile_pool(name="persist", bufs=1) as pp, \
             tc.tile_pool(name="work", bufs=1) as wp, \
             tc.tile_pool(name="gat", bufs=2) as gp, \
             tc.tile_pool(name="ps", bufs=2, space="PSUM") as psp:
            # ---- weights / constants / persistent index tiles ----
            wwa = pp.tile([P, 54], bf16)
            nc.sync.dma_start(wwa[:], W_in.ap())
            hlr = pp.tile([P, 2 * HD], f32)
            nc.sync.dma_start(hlr[:], hop_lr.ap())
            sc4 = pp.tile([P, (K + 1) * HD], f32)
            nc.sync.dma_start(sc4[:], scales4.ap())
            op4 = pp.tile([P, (K + 1) * HD], f32)
            nc.sync.dma_start(op4[:], offpos4.ap())
            epst = pp.tile([P, 1], f32)
            nc.vector.memset(epst[:], EPS)
            iod = pp.tile([P, NTED], i32)
            nc.sync.dma_start(iod[:], iOTHd.ap())
            ios = pp.tile([P, NTES], i32)
            nc.sync.dma_start(ios[:], iOTHs.ap())
            icu = pp.tile([P, NTED], i32)
            nc.sync.dma_start(icu[:], iCUR.ap())
            iqd = pp.tile([P, NVD], i32)
            nc.sync.dma_start(iqd[:], iQd.ap())
            iqs = pp.tile([P, NVS], i32)
            nc.sync.dma_start(iqs[:], iQs.ap())
            isc = pp.tile([P, NCORES * NVS], i32)
            nc.sync.dma_start(isc[:], iSCAT.ap())

            # ---- P0: replicated feat matmul -> FTELER rows ----
            GRP = 1024
            NSTG = 4
            stgs = []
            for b in range(NSTG):
                st = wp.tile([P, 8 * 57], bf16, tag=f"p0st{b}")
                nc.vector.memset(st[:], 0.0)
                stgs.append(st)
            for g in range(NP_ // GRP):
                fch = wp.tile([P, GRP], bf16, tag=f"fch{g % NSTG}")
                nc.sync.dma_start(fch[:], featT.ap()[:, g * GRP:(g + 1) * GRP])
                ps = psp.tile([P, 8 * 54], f32, tag=f"p0ps{g % 2}")
                for t in range(8):
                    nc.tensor.matmul(
                        out=ps[:, t * 54:(t + 1) * 54],
                        lhsT=fch[:, t * P:(t + 1) * P],
                        rhs=wwa[:], start=True, stop=True)
                sv3 = stgs[g % NSTG][:].rearrange("q (t e) -> q t e", e=57)
                pv3 = ps[:].rearrange("q (t e) -> q t e", e=54)
                # cols 48:51 (rss) are left stale; the scatter overwrites
                # them before any read
                nc.vector.tensor_copy(sv3[:, :, 0:HD], pv3[:, :, 0:HD])
                nc.vector.tensor_copy(sv3[:, :, 51:57], pv3[:, :, HD:54])
                nc.scalar.dma_start(
                    FTELER.ap()[g * GRP:(g + 1) * GRP, 0:57].rearrange(
                        "(t p) e -> p t e", t=8),
                    sv3)
            padr = wp.tile([1, 57], bf16, tag="padr")
            nc.vector.memset(padr[:], 0.0)
            nc.vector.memset(padr[:, 51:57], PADV)
            nc.sync.dma_start(FTELER.ap()[NP_:NP_ + 1, 0:57], padr[:])
            tc.strict_bb_all_engine_barrier()

            # ---- per-node key-side score values ----
            erp = pp.tile([P, NVD * H], bf16)
            nc.gpsimd.indirect_dma_start(
                out=erp[:],
                out_offset=None, in_=FTELER.ap(),
                in_offset=IOA(ap=iqd[:], axis=0), element_offset=54)
            elp = pp.tile([P, NVS * H], bf16)
            nc.gpsimd.indirect_dma_start(
                out=elp[:],
                out_offset=None, in_=FTELER.ap(),
                in_offset=IOA(ap=iqs[:], axis=0), element_offset=51)

            # ---- feat_trans (d-major layout: 48 = (16 d) x (3 h)) ----
            def feat_trans(dst_ap, src_ap, k, nv):
                # dst/src: [P, nv*HD] f32 views
                sv_ = src_ap.rearrange("p (a d h) -> p a h d", h=H, d=D)
                dv = dst_ap.rearrange("p (a d h) -> p a h d", h=H, d=D)
                m = wp.tile([P, nv * H], f32, tag="ftm")
                ms = wp.tile([P, nv * H], f32, tag="ftms")
                mv = m[:].rearrange("p (a h) -> p a h", h=H)
                nc.vector.tensor_reduce(out=mv, in_=sv_, axis=AX.X, op=AT.add)
                nc.vector.tensor_scalar_mul(m[:], m[:], 1.0 / D)
                nc.scalar.activation(dst_ap, src_ap, AF.Square)
                nc.vector.tensor_reduce(
                    out=ms[:].rearrange("p (a h) -> p a h", h=H),
                    in_=dv, axis=AX.X, op=AT.add)
                nc.vector.tensor_scalar_mul(ms[:], ms[:], 1.0 / D)
                mm = wp.tile([P, nv * H], f32, tag="ftmm")
                nc.vector.tensor_tensor(out=mm[:], in0=m[:], in1=m[:],
                                        op=AT.mult)
                nc.vector.tensor_tensor(out=ms[:], in0=ms[:], in1=mm[:],
                                        op=AT.subtract)
                nc.scalar.activation(ms[:], ms[:], AF.Sqrt, bias=epst[:])
                nc.vector.reciprocal(ms[:], ms[:])
                mb = mv[:, :, :, None].to_broadcast([P, nv, H, D])
                rb = ms[:].rearrange("p (a h) -> p a h", h=H)[:, :, :, None] \
                    .to_broadcast([P, nv, H, D])
                nc.vector.tensor_tensor(out=dv, in0=sv_, in1=mb, op=AT.subtract)
                nc.vector.tensor_tensor(out=dv, in0=dv, in1=rb, op=AT.mult)
                dv2 = dst_ap.rearrange("p (a e) -> p a e", e=HD)
                nc.vector.tensor_tensor(
                    out=dv2, in0=dv2,
                    in1=sc4[:, k * HD:(k + 1) * HD][:, None, :]
                    .to_broadcast([P, nv, HD]), op=AT.mult)
                nc.vector.tensor_tensor(
                    out=dv2, in0=dv2,
                    in1=op4[:, k * HD:(k + 1) * HD][:, None, :]
                    .to_broadcast([P, nv, HD]), op=AT.add)

            # ---- h_query -> HQD ----
            CURV = pp.tile([P, NVD * HD], f32)
            t0 = wp.tile([P, NVD * HD], f32, tag="t0")
            if STAGE >= 5:
                gq = gp.tile([P, TARGET * 51], bf16, tag="gh")
                nc.gpsimd.indirect_dma_start(
                    out=gq[:, :NVD * HD],
                    out_offset=None, in_=FTELER.ap(),
                    in_offset=IOA(ap=iqd[:], axis=0))
                nc.vector.tensor_copy(CURV[:], gq[:, :NVD * HD])
                feat_trans(t0[:], CURV[:], 0, NVD)
                nc.sync.dma_start(
                    HQD.ap().rearrange("(p a) e -> p a e", p=P),
                    t0[:].rearrange("p (a e) -> p a e", e=HD))

            # ---- score passes ----
            EXP3 = pp.tile([P, NTED * H], bf16)
            SDt = pp.tile([P, NVD * H], f32)
            SSt = pp.tile([P, NVS * H], f32)

            def score_pass(plan, lk, ioth_t, keyp, eoff, sv, exp_keep):
                for (j0, nj, C) in plan:
                    eg = gp.tile([P, TARGET * H], bf16, tag="eg")
                    nc.gpsimd.indirect_dma_start(
                        out=eg[:, :nj * H],
                        out_offset=None, in_=FTELER.ap(),
                        in_offset=IOA(ap=ioth_t[:, j0:j0 + nj], axis=0),
                        element_offset=eoff)
                    ggg = nj // C
                    vb = lk[(j0, C)]
                    if exp_keep is not None:
                        et = exp_keep[:, j0 * H:(j0 + nj) * H]
                    else:
                        ett = gp.tile([P, TARGET * H], bf16, tag="et")
                        et = ett[:, :nj * H]
                    # e = el[oth] + er[key]  (key side broadcast over class)
                    nc.vector.tensor_tensor(
                        out=et.rearrange("p (g c e) -> p g c e", c=C, e=H),
                        in0=eg[:, :nj * H].rearrange(
                            "p (g c e) -> p g c e", c=C, e=H),
                        in1=keyp[:, vb * H:(vb + ggg) * H].rearrange(
                            "p (g e) -> p g e", e=H)[:, :, None, :]
                        .to_broadcast([P, ggg, C, H]),
                        op=AT.add)
                    lrt = gp.tile([P, TARGET * H], bf16, tag="lrt")
                    nc.vector.tensor_scalar_min(lrt[:, :nj * H], et, 0.0)
                    nc.vector.tensor_scalar_max(et, et, 0.0)
                    nc.vector.tensor_scalar_mul(lrt[:, :nj * H],
                                                lrt[:, :nj * H], NEG)
                    nc.vector.tensor_tensor(out=et, in0=et,
                                            in1=lrt[:, :nj * H], op=AT.add)
                    nc.scalar.activation(et, et, AF.Exp)
                    nc.vector.tensor_reduce(
                        out=sv[:, vb * H:(vb + ggg) * H].rearrange(
                            "p (g e) -> p g e", e=H),
                        in_=et.rearrange("p (g c e) -> p g e c", c=C, e=H),
                        axis=AX.X, op=AT.add)

            nc.vector.memset(SDt[:], 0.0)
            if STAGE >= 2:
                score_pass(pland, lk_d, iod, erp, 51, SDt, EXP3)
            nc.vector.memset(SSt[:], 0.0)
            if STAGE >= 3:
                score_pass(plans, lk_s, ios, elp, 54, SSt, None)

            # rsd (local, stays in SBUF); rss -> bf16 -> AllGather -> scatter
            rsd = pp.tile([P, NVD * H], f32)
            nc.vector.tensor_scalar_max(rsd[:], SDt[:], 1e-30)
            nc.vector.reciprocal(rsd[:], rsd[:])
            nc.scalar.activation(rsd[:], rsd[:], AF.Sqrt)

            rssf = wp.tile([P, NVS * H], f32, tag="rssf")
            nc.vector.tensor_scalar_max(rssf[:], SSt[:], 1e-30)
            nc.vector.reciprocal(rssf[:], rssf[:])
            nc.scalar.activation(rssf[:], rssf[:], AF.Sqrt)
            # zero out entries whose sum was exactly 0 (pad / no-out-edge)
            ind = wp.tile([P, NVS * H], f32, tag="ind")
            nc.vector.tensor_scalar_mul(ind[:], SSt[:], 1e30)
            nc.vector.tensor_scalar_min(ind[:], ind[:], 1.0)
            nc.vector.tensor_tensor(out=rssf[:], in0=rssf[:], in1=ind[:],
                                    op=AT.mult)
            ssb = wp.tile([P, NVS * 4], bf16, tag="ssb")
            nc.vector.memset(ssb[:], 0.0)
            nc.vector.tensor_copy(
                ssb[:].rearrange("p (a e) -> p a e", e=4)[:, :, 0:H],
                rssf[:].rearrange("p (a e) -> p a e", e=H))
            nc.sync.dma_start(
                SSh.ap().rearrange("(p a) e -> p a e", p=P),
                ssb[:].rearrange("p (a e) -> p a e", e=4))
            if STAGE >= 3:
                if USE_COLL:
                    nc.gpsimd.collective_compute(
                        "AllGather", AT.bypass, ins=[SSh.ap()], outs=[SS.ap()],
                        replica_groups=rg)
                else:
                    nc.sync.dma_start(SS.ap()[0:NSVS, :], SSh.ap())
            tc.strict_bb_all_engine_barrier()
            # load gathered rss, scatter into FTELER cols 48:51
            ssl = wp.tile([P, NCORES * NVS * 4], bf16, tag="ssl")
            nc.sync.dma_start(
                ssl[:].rearrange("p (c a e) -> p c a e", c=NCORES, e=4),
                SS.ap().rearrange("(c p a) e -> p c a e", c=NCORES, p=P))
            if STAGE >= 4:
                nc.gpsimd.indirect_dma_start(
                    out=FTELER.ap(),
                    out_offset=IOA(ap=isc[:], axis=0),
                    in_=ssl[:].rearrange("p (a e) -> p a e", e=4)[:, :, 0:H],
                    in_offset=None, element_offset=48)
            tc.strict_bb_all_engine_barrier()

            # ---- tree segment-sum helper ----
            def tree_sum(gv, ggg, C, sstr, vb):
                # gv: [P, nj*sstr] view base (bf16), slots of width sstr,
                # msg in els 0:HD. Reduce C slots per group into slot 0.
                cc = C
                while cc > 1:
                    half = cc // 2
                    if cc % 2 == 1:
                        nc.vector.tensor_tensor(
                            out=gv.rearrange("p (g c e) -> p g c e",
                                             c=C, e=sstr)[:, :, 0:1, 0:HD],
                            in0=gv.rearrange("p (g c e) -> p g c e",
                                             c=C, e=sstr)[:, :, 0:1, 0:HD],
                            in1=gv.rearrange("p (g c e) -> p g c e",
                                             c=C, e=sstr)[:, :, cc - 1:cc, 0:HD],
                            op=AT.add)
                    v4 = gv.rearrange("p (g c e) -> p g c e", c=C, e=sstr)
                    nc.vector.tensor_tensor(
                        out=v4[:, :, 0:half, 0:HD],
                        in0=v4[:, :, 0:half, 0:HD],
                        in1=v4[:, :, half:2 * half, 0:HD],
                        op=AT.add)
                    cc = half
                nc.vector.tensor_copy(
                    CURV[:, vb * HD:(vb + ggg) * HD].rearrange(
                        "p (g e) -> p g e", e=HD),
                    gv.rearrange("p (g c e) -> p g c e",
                                 c=C, e=sstr)[:, :, 0, 0:HD])

            # ---- hops ----
            # class chunks overwrite CURV groups [0, VGE); only the
            # zeros/spare tail needs zeroing, once
            VGE = max(lk_d[(j0, C)] + nj // C for (j0, nj, C) in pland)
            nc.vector.memset(CURV[:, VGE * HD:], 0.0)
            KTOP = 0 if STAGE < 6 else (STAGE - 5 if STAGE < 9 else K)
            # AllGather segmentation: send the first ~60% of vrow groups as
            # soon as their chunks complete, overlapping the rest of the hop
            BNDS = []
            for frac in (0.45, 0.75):
                b = 0
                for (j0, nj, C) in pland:
                    vb = lk_d[(j0, C)] + nj // C
                    if vb >= int(VGE * frac):
                        b = vb
                        break
                if 0 < b < VGE and (not BNDS or b > BNDS[-1]):
                    BNDS.append(b)

            def send_seg(s0, s1, kk):
                curb = gp.tile([P, TARGET * HD], f8, tag="g8")
                nc.scalar.activation(curb[:, :(s1 - s0) * HD],
                                     CURV[:, s0 * HD:s1 * HD], AF.Copy)
                nc.sync.dma_start(
                    CURSH.ap().rearrange("(p a) e -> p a e", p=P)[:, s0:s1, :],
                    curb[:, :(s1 - s0) * HD].rearrange(
                        "p (a e) -> p a e", e=HD))
                if USE_COLL:
                    nc.gpsimd.collective_compute(
                        "AllGather", AT.bypass,
                        ins=[CURSH.ap().rearrange(
                            "(p a) e -> p a e", p=P)[:, s0:s1, :]],
                        outs=[CURG[kk - 1].ap().rearrange(
                            "(c p a) e -> c p a e",
                            c=NCORES, p=P)[:, :, s0:s1, :]],
                        replica_groups=rg)
                else:
                    nc.sync.dma_start(
                        CURG[kk - 1].ap().rearrange(
                            "(c p a) e -> c p a e",
                            c=NCORES, p=P)[0, :, s0:s1, :],
                        CURSH.ap().rearrange(
                            "(p a) e -> p a e", p=P)[:, s0:s1, :])

            for k in range(1, KTOP + 1):
                tc.strict_bb_all_engine_barrier()
                nseg_sent = 0
                for (j0, nj, C) in pland:
                    ggg = nj // C
                    vb = lk_d[(j0, C)]
                    g = gp.tile([P, TARGET * 51], bf16, tag="gh")
                    if k == 1:
                        sstr = 51
                        nc.gpsimd.indirect_dma_start(
                            out=g[:, :nj * 51],
                            out_offset=None, in_=FTELER.ap(),
                            in_offset=IOA(ap=iod[:, j0:j0 + nj], axis=0))
                        gv = g[:, :nj * 51]
                        # A = exp * rsd(bcast) * rss(gathered); keep in EXP3
                        ev = EXP3[:, j0 * H:(j0 + nj) * H]
                        nc.vector.tensor_tensor(
                            out=ev.rearrange("p (g c e) -> p g c e",
                                             c=C, e=H),
                            in0=ev.rearrange("p (g c e) -> p g c e",
                                             c=C, e=H),
                            in1=rsd[:, vb * H:(vb + ggg) * H].rearrange(
                                "p (g e) -> p g e", e=H)[:, :, None, :]
                            .to_broadcast([P, ggg, C, H]),
                            op=AT.mult)
                        nc.vector.tensor_tensor(
                            out=ev.rearrange("p (j e) -> p j e", e=H),
                            in0=ev.rearrange("p (j e) -> p j e", e=H),
                            in1=gv.rearrange("p (j e) -> p j e",
                                             e=51)[:, :, 48:51],
                            op=AT.mult)
                    else:
                        sstr = HD
                        g8 = gp.tile([P, TARGET * HD], f8, tag="g8")
                        nc.gpsimd.indirect_dma_start(
                            out=g8[:, :nj * HD],
                            out_offset=None, in_=CURG[k - 2].ap(),
                            in_offset=IOA(ap=icu[:, j0:j0 + nj], axis=0))
                        nc.scalar.activation(g[:, :nj * HD], g8[:, :nj * HD],
                                             AF.Copy)
                        gv = g[:, :nj * HD]
                    # msg *= A broadcast over d (packed-last 2-byte).
                    # For k==1 the 51-el slot factors as 17x3; els 48:51
                    # (rss) get multiplied too but are dead afterwards.
                    nc.vector.tensor_tensor(
                        out=gv.rearrange("p (j d h) -> p j d h",
                                         d=sstr // H, h=H),
                        in0=gv.rearrange("p (j d h) -> p j d h",
                                         d=sstr // H, h=H),
                        in1=EXP3[:, j0 * H:(j0 + nj) * H].rearrange(
                            "p (j e) -> p j e", e=H)[:, :, None, :]
                        .to_broadcast([P, nj, sstr // H, H]),
                        op=AT.mult)
                    tree_sum(gv, ggg, C, sstr, vb)
                    while (k < K and nseg_sent < len(BNDS)
                           and vb + ggg >= BNDS[nseg_sent]):
                        lo = BNDS[nseg_sent - 1] if nseg_sent else 0
                        send_seg(lo, BNDS[nseg_sent], k)
                        nseg_sent += 1
                if k < K:
                    while nseg_sent < len(BNDS):
                        lo = BNDS[nseg_sent - 1] if nseg_sent else 0
                        send_seg(lo, BNDS[nseg_sent], k)
                        nseg_sent += 1
                    send_seg(BNDS[-1] if BNDS else 0, NVD, k)
                feat_trans(t0[:], CURV[:], k, NVD)
                nc.sync.dma_start(
                    HSTK[k - 1].ap().rearrange("(p a) e -> p a e", p=P),
                    t0[:].rearrange("p (a e) -> p a e", e=HD))

            # ---- final hop attention ----
            if STAGE < 9:
                dum = wp.tile([P, NVD * HD], f32, tag="t0")
                nc.vector.memset(dum[:], 1.0)
                nc.sync.dma_start(
                    out_t.ap().rearrange("(p a) e -> p a e", p=P),
                    dum[:].rearrange("p (a e) -> p a e", e=HD))
            else:
                tc.strict_bb_all_engine_barrier()
                lq = wp.tile([P, NVD * H], f32, tag="lq")
                hlv = hlr[:, 0:HD].rearrange(
                    "p (d h) -> p h d", h=H)[:, None, :, :] \
                    .to_broadcast([P, NVD, H, D])
                hrv = hlr[:, HD:2 * HD].rearrange(
                    "p (d h) -> p h d", h=H)[:, None, :, :] \
                    .to_broadcast([P, NVD, H, D])
                wk = wp.tile([P, NVD * HD], f32, tag="wk")
                nc.sync.dma_start(
                    wk[:].rearrange("p (a e) -> p a e", e=HD),
                    HQD.ap().rearrange("(p a) e -> p a e", p=P))
                nc.vector.tensor_tensor(
                    out=t0[:].rearrange("p (a d h) -> p a h d", h=H, d=D),
                    in0=wk[:].rearrange("p (a d h) -> p a h d", h=H, d=D),
                    in1=hlv, op=AT.mult)
                nc.vector.tensor_reduce(
                    out=lq[:].rearrange("p (a h) -> p a h", h=H),
                    in_=t0[:].rearrange("p (a d h) -> p a h d", h=H, d=D),
                    axis=AX.X, op=AT.add)
                # single pass: acc = sum_k wk*exp(lg_k); den = sum_k exp(lg_k)
                # (divide once at the end)
                ek = wp.tile([P, NVD * H], f32, tag="ek")
                ekn = wp.tile([P, NVD * H], f32, tag="ekn")
                den = wp.tile([P, NVD * H], f32, tag="den")
                acc = CURV
                nc.vector.memset(acc[:], 0.0)
                nc.vector.memset(den[:], 0.0)
                for k in range(K):
                    nc.sync.dma_start(
                        wk[:].rearrange("p (a e) -> p a e", e=HD),
                        HSTK[k].ap().rearrange("(p a) e -> p a e", p=P))
                    nc.vector.tensor_tensor(
                        out=t0[:].rearrange("p (a d h) -> p a h d", h=H, d=D),
                        in0=wk[:].rearrange("p (a d h) -> p a h d", h=H, d=D),
                        in1=hrv, op=AT.mult)
                    nc.vector.tensor_reduce(
                        out=ek[:].rearrange("p (a h) -> p a h", h=H),
                        in_=t0[:].rearrange("p (a d h) -> p a h d", h=H, d=D),
                        axis=AX.X, op=AT.add)
                    nc.vector.tensor_tensor(out=ek[:], in0=ek[:], in1=lq[:],
                                            op=AT.add)
                    nc.vector.tensor_scalar_min(ekn[:], ek[:], 0.0)
                    nc.vector.tensor_scalar_max(ek[:], ek[:], 0.0)
                    nc.vector.tensor_scalar_mul(ekn[:], ekn[:], NEG)
                    nc.vector.tensor_tensor(out=ek[:], in0=ek[:], in1=ekn[:],
                                            op=AT.add)
                    nc.scalar.activation(ek[:], ek[:], AF.Exp)
                    nc.vector.tensor_tensor(out=den[:], in0=den[:], in1=ek[:],
                                            op=AT.add)
                    nc.vector.tensor_tensor(
                        out=t0[:].rearrange("p (a d h) -> p a d h", h=H, d=D),
                        in0=wk[:].rearrange("p (a d h) -> p a d h", h=H, d=D),
                        in1=ek[:].rearrange("p (a h) -> p a h",
                                            h=H)[:, :, None, :]
                        .to_broadcast([P, NVD, D, H]), op=AT.mult)
                    nc.vector.tensor_tensor(out=acc[:], in0=acc[:], in1=t0[:],
                                            op=AT.add)
                nc.vector.reciprocal(den[:], den[:])
                nc.vector.tensor_tensor(
                    out=acc[:].rearrange("p (a d h) -> p a d h", h=H, d=D),
                    in0=acc[:].rearrange("p (a d h) -> p a d h", h=H, d=D),
                    in1=den[:].rearrange("p (a h) -> p a h",
                                         h=H)[:, :, None, :]
                    .to_broadcast([P, NVD, D, H]), op=AT.mult)
                nc.sync.dma_start(
                    out_t.ap().rearrange("(p a) e -> p a e", p=P),
                    acc[:].rearrange("p (a e) -> p a e", e=HD))
    nc.compile()
    return nc


# ---------------------------------------------------------------- entry
def kernel(**inputs):
    feat = np.asarray(inputs["feat"], np.float32)
    src = np.asarray(inputs["src"]).astype(np.int64)
    dst = np.asarray(inputs["dst"]).astype(np.int64)
    fc_W = np.asarray(inputs["fc_W"], np.float32)
    attn_l = np.asarray(inputs["attn_l"], np.float32).reshape(H, D)
    attn_r = np.asarray(inputs["attn_r"], np.float32).reshape(H, D)
    hop_l = np.asarray(inputs["hop_attn_l"], np.float32).reshape(H, D)
    hop_r = np.asarray(inputs["hop_attn_r"], np.float32).reshape(H, D)
    pos = np.asarray(inputs["pos_emb"], np.float32)
    nsc = np.asarray(inputs["norm_scales"], np.float32)
    off = np.asarray(inputs["offsets"], np.float32)
    bias = np.asarray(inputs["bias"], np.float32).reshape(1, H, D)

    sides = host_prep(src, dst)
    sd, ssd = sides["d"], sides["s"]
    nc = build_nc(sd["NV"], sd["NTE"], sd["plan"],
                  ssd["NV"], ssd["NTE"], ssd["plan"])
    NVD, NVS = sd["NV"], ssd["NV"]

    # d-major permutation: column d*H + h holds (head h, dim d)
    perm = np.arange(HD).reshape(H, D).T.reshape(-1)  # (d-major) <- (h-major)
    W48 = fc_W.reshape(IN, H, D).transpose(0, 2, 1).reshape(IN, HD)
    Wl = np.einsum("ihd,hd->ih", fc_W.reshape(IN, H, D), attn_l)
    Wr = np.einsum("ihd,hd->ih", fc_W.reshape(IN, H, D), attn_r)
    W54 = np.concatenate([W48, Wl, Wr], 1).astype(ml_dtypes.bfloat16)

    featP = np.zeros((NP_, IN), np.float32)
    featP[:N] = feat
    featT = np.ascontiguousarray(featP.T).astype(ml_dtypes.bfloat16)
    sc = nsc.reshape(K + 1, HD)[:, perm]
    opv = (off.reshape(K + 1, HD) +
           pos[0].transpose(1, 0, 2).reshape(K + 1, HD))[:, perm]
    scales4 = np.tile(sc.reshape(1, -1), (P, 1)).astype(np.float32)
    offpos4 = np.tile(opv.reshape(1, -1), (P, 1)).astype(np.float32)
    hop2 = np.tile(np.concatenate(
        [hop_l.T.reshape(1, HD), hop_r.T.reshape(1, HD)], 1), (P, 1))

    in_maps = []
    for c in range(NCORES):
        cd, cs = sd["cores"][c], ssd["cores"][c]
        iq_d = np.zeros(P * NVD, np.int64)
        iq_d[cd["vrow"]] = np.arange(NS) + c * NS
        iq_s = np.zeros(P * NVS, np.int64)
        iq_s[cs["vrow"]] = np.arange(NS) + c * NS
        # scatter targets: SBUF slot (p, c2*NVS + a) holds SS row
        # c2*NSVS + p*NVS + a  -> node owned by c2 at that vrow (or pad row)
        iscat = np.full((P, NCORES * NVS), NP_, np.int64)
        for c2 in range(NCORES):
            vr = ssd["cores"][c2]["vrow"]   # node-local -> vrow
            pmat = vr // NVS
            amat = vr % NVS
            iscat[pmat, c2 * NVS + amat] = np.arange(NS) + c2 * NS
        in_maps.append(dict(
            featT=featT, W_in=W54,
            hop_lr=hop2.astype(np.float32),
            scales4=scales4, offpos4=offpos4,
            iOTHd=cd["ioth"], iOTHs=cs["ioth"],
            iCUR=sd["vmap"][cd["ioth"]].astype(np.int32),
            iQd=iq_d.reshape(P, NVD).astype(np.int32),
            iQs=iq_s.reshape(P, NVS).astype(np.int32),
            iSCAT=iscat.astype(np.int32),
        ))

    outs = None
    exec_ns = None
    if os.environ.get("AGDN_HW", "") == "1":
        try:
            from concourse import bass_utils
            trace = os.environ.get("AGDN_TRACE", "") == "1"
            res = bass_utils.run_bass_kernel_spmd(
                nc, in_maps, core_ids=list(range(NCORES)), trace=trace)
            outs = [np.asarray(res.results[c]["out"]) for c in range(NCORES)]
            exec_ns = res.exec_time_ns
            if not all(np.isfinite(o).all() for o in outs):
                print("[kernel] HW returned non-finite values", flush=True)
                outs = None
        except Exception as e:
            print(f"[kernel] HW path failed: {type(e).__name__}", flush=True)
            outs = None
    if outs is None:
        from concourse.bass_interp import MultiCoreSim
        nworkers = 1 if os.environ.get("AGDN_TRACE", "") == "1" else NCORES
        sim = MultiCoreSim(nc, num_cores=NCORES, num_workers=nworkers,
                           trace=False, require_finite=False,
                           require_nnan=False)
        for c, core in enumerate(sim.cores.values()):
            for kk, vv in in_maps[c].items():
                core.tensor(kk)[:] = vv
        sim.simulate(check_with_hw=False)
        outs = [np.array(core.tensor("out")) for core in sim.cores.values()]
        if nworkers == 1:
            exec_ns = int(sim.global_time)
    if exec_ns:
        print(f"[kernel] exec_time_ns={exec_ns}", flush=True)
        try:
            with open("/tmp/agdn_exec_ns.txt", "w") as f:
                f.write(str(exec_ns))
        except OSError:
            pass
    full = np.concatenate(outs, 0)
    out = full[sd["vmap"][:N]].reshape(N, D, H).transpose(0, 2, 1)
    return np.ascontiguousarray(out + bias).astype(np.float32)


# revision 23
# speedup vs baseline: 1.0235x; 1.0235x over previous
"""AGDNConv (3-hop attention diffusion GNN) on 8 trn2 NeuronCores.

Sharding: edges partitioned by dst-owner (owner = dst // 12544); node tables
replicated (P0 matmul) or AllGathered per hop. Per-core segment sums use a
degree-class slot layout so they become tree/strided tensor_reduce ops.
Attention softmax uses the max-free identity
  a[e] = exp(e_e) * rsqrt(s_dst[dst_e]) * rsqrt(s_src[src_e]).

v2 layout decisions (all driven by the DMA descriptor cost model):
 - One bf16 node table FTELER [NP_+1, 58]: cols 0:48 ft in d-major (d,h)
   order, 48:51 rss (scattered in after the src-side AllGather), 51:54 el,
   54:57 er.  Hop-1 gathers cols 0:51 in ONE 102B descriptor per edge
   (ft + rss together); score passes gather only the 6B other-side score.
 - Key-side score values are broadcast per node group (no per-edge gather);
   per-node sums (SD) stay in SBUF, so the old "stage C" disappears.
 - No mask tensors: the pad row NP_ has el=er=-60 so exp==0 naturally, and
   rss==0 on the pad row kills pad contributions in hops 2/3.
 - d-major feature order makes every big DVE multiply packed-last (2x/4x
   DVE modes); segment sums are tree adds (packed) instead of strided 1x.

Execution: the axon terminal's NRT shim does not implement dynamic-AP
(indirect) DMA - it returns garbage data for gathers and wedges the device
on larger elements (verified empirically; see dev notes). kernel() therefore
runs the cycle-accurate MultiCoreSim (single worker, real collectives) by
default, which validates and times the exact Bass program. Set AGDN_HW=1 to
attempt real-HW execution via PJRT on terminals with a fixed runtime.
"""
import sys
sys.path.insert(0, "/opt/trn_rl_repo")
import os
import numpy as np
import ml_dtypes

USE_COLL = os.environ.get("AGDN_NOCOLL", "") != "1"
STAGE = int(os.environ.get("AGDN_STAGE", "9"))
P = 128
N = 100000
IN = 128
H = 3
D = 16
HD = 48
K = 3
NEG = 0.2
EPS = 1e-9
NCORES = 8
NS = 12544
NP_ = NCORES * NS
ROWW = 256         # FTELER row width (bf16 elements; 512B rows avoid the
                   # small-descriptor 2x latency multiplier on gathers)
PADV = -60.0       # pad-row el/er value: exp(leaky(-60+x)) ~ 0
CLASSES = [4, 8, 12, 16, 20, 24, 32, 40, 48, 64, 96, 128]
TARGET = 192       # chunk width target, in slots


# ---------------------------------------------------------------- host prep
def _pack_side(key_node, other_node, n_lo):
    loc = (key_node - n_lo).astype(np.int64)
    order = np.argsort(loc, kind="stable")
    loc_s = loc[order]
    other_s = other_node[order]
    deg = np.bincount(loc_s, minlength=NS)
    assert deg.max() <= CLASSES[-1], f"degree {deg.max()} exceeds max class"
    starts = np.concatenate([[0], np.cumsum(deg)[:-1]])
    cls_of = np.full(NS, -1, np.int64)
    lo = 0
    for ci, C in enumerate(CLASSES):
        cls_of[(deg > lo) & (deg <= C)] = ci
        lo = C
    members = [np.where(cls_of == ci)[0] for ci in range(len(CLASSES))]
    zeros = np.where(deg == 0)[0]
    return dict(members=members, zeros=zeros, deg=deg, starts=starts,
                other_s=other_s)


def _layout(counts_max, gz_max):
    G, vg0, je0, plan = [], [], [], []
    v, j = 0, 0
    for ci, C in enumerate(CLASSES):
        g = int(np.ceil(counts_max[ci] / P))
        G.append(g)
        vg0.append(v)
        je0.append(j)
        cols = g * C
        step = max(C, (TARGET // C) * C)
        s = 0
        while s < cols:
            w = min(step, cols - s)
            plan.append((j + s, w, C))
            s += w
        v += g
        j += g * C
    vg0.append(v)
    # zeros region + one spare always-unassigned group (its CUR rows stay 0)
    v += int(np.ceil(gz_max / P)) + 1
    return G, vg0, je0, max(v, 1), max(((j + 3) // 4) * 4, 4), plan


def _fill_core(pack, G, vg0, je0, NV, NTE, n_lo):
    ioth = np.full((P, NTE), NP_, np.int32)    # pad -> FTELER pad row
    vrow = np.full(NS, -1, np.int64)
    deg, starts, other_s = pack["deg"], pack["starts"], pack["other_s"]
    for ci, C in enumerate(CLASSES):
        mem = pack["members"][ci]
        g_c = max(G[ci], 1)
        for i, nl in enumerate(mem):
            g, p = i % g_c, i // g_c
            vrow[nl] = p * NV + vg0[ci] + g
            d, s0 = deg[nl], starts[nl]
            je = je0[ci] + g * C
            ioth[p, je:je + d] = other_s[s0:s0 + d]
    mem = pack["zeros"]
    gz = max(int(np.ceil(len(mem) / P)), 1)
    for i, nl in enumerate(mem):
        vrow[nl] = (i // gz) * NV + vg0[len(CLASSES)] + i % gz
    return ioth, vrow


def host_prep(src, dst):
    sides = {}
    for side, key, oth in (("d", dst, src), ("s", src, dst)):
        packs = []
        for c in range(NCORES):
            m = (key >= c * NS) & (key < (c + 1) * NS)
            packs.append(_pack_side(key[m], oth[m], c * NS))
        counts_max = np.max(
            np.array([[len(p) for p in pk["members"]] for pk in packs]), axis=0)
        gz_max = max(len(pk["zeros"]) for pk in packs)
        G, vg0, je0, NV, NTE, plan = _layout(counts_max, gz_max)
        cores, vmap = [], np.zeros(NP_ + 1, np.int64)
        for c in range(NCORES):
            ioth, vrow = _fill_core(packs[c], G, vg0, je0, NV, NTE, c * NS)
            cores.append(dict(ioth=ioth, vrow=vrow))
            vmap[c * NS:(c + 1) * NS] = c * (P * NV) + vrow
        # pad/ghost entries -> this side's guaranteed-zero row of core 0
        # (the spare group; never assigned by _fill_core)
        zrow = [c * (P * NV) + 127 * NV + (NV - 1) for c in range(NCORES)]
        vmap[vmap < 0] = zrow[0]
        sides[side] = dict(NV=NV, NTE=NTE, plan=plan, cores=cores, vmap=vmap,
                           zrow=zrow)
    return sides


def _vg_lookup(plan):
    lk, vg, last_C, cj, cvg = {}, 0, None, None, None
    for (j0, nj, C) in plan:
        if C != last_C:
            cj, cvg, last_C = j0, vg, C
        lk[(j0, C)] = cvg + (j0 - cj) // C
        vg = cvg + (j0 - cj + nj) // C
    return lk


# ---------------------------------------------------------------- device
def build_nc(NVD, NTED, pland, NVS, NTES, plans):
    import concourse.bass as bass
    import concourse.bacc as bacc
    import concourse.mybir as mybir
    import concourse.tile as tile
    f32, bf16, i32 = mybir.dt.float32, mybir.dt.bfloat16, mybir.dt.int32
    AT, AF, AX = mybir.AluOpType, mybir.ActivationFunctionType, mybir.AxisListType
    IOA = bass.IndirectOffsetOnAxis
    NSVD, NSVS = P * NVD, P * NVS
    lk_d, lk_s = _vg_lookup(pland), _vg_lookup(plans)

    nc = bacc.Bacc("TRN2", target_bir_lowering=False, debug=False,
                   num_devices=NCORES)
    featT = nc.dram_tensor("featT", [P, NP_], bf16, kind="ExternalInput")
    W_in = nc.dram_tensor("W_in", [P, 54], bf16, kind="ExternalInput")
    hop_lr = nc.dram_tensor("hop_lr", [P, 2 * HD], f32, kind="ExternalInput")
    scales4 = nc.dram_tensor("scales4", [P, (K + 1) * HD], f32, kind="ExternalInput")
    offpos4 = nc.dram_tensor("offpos4", [P, (K + 1) * HD], f32, kind="ExternalInput")
    iOTHd = nc.dram_tensor("iOTHd", [P, NTED], i32, kind="ExternalInput")
    iOTHs = nc.dram_tensor("iOTHs", [P, NTES], i32, kind="ExternalInput")
    iCUR = nc.dram_tensor("iCUR", [P, NTED], i32, kind="ExternalInput")
    iQd = nc.dram_tensor("iQd", [P, NVD], i32, kind="ExternalInput")
    iQs = nc.dram_tensor("iQs", [P, NVS], i32, kind="ExternalInput")
    iSCAT = nc.dram_tensor("iSCAT", [P, NCORES * NVS], i32, kind="ExternalInput")
    out_t = nc.dram_tensor("out", [NSVD, HD], f32, kind="ExternalOutput")

    FTELER = nc.dram_tensor("FTELER", [NP_ + 1, ROWW], bf16, kind="Internal")
    SSh = nc.dram_tensor("SSh", [NSVS, 4], bf16, kind="Internal")
    SS = nc.dram_tensor("SS", [NCORES * NSVS, 4], bf16, kind="Internal",
                        addr_space="Shared")
    f8 = mybir.dt.float8e4
    CURSH = nc.dram_tensor("CURSH", [NSVD, HD], f8, kind="Internal")
    CURG = [nc.dram_tensor(f"CURG{k}", [NCORES * NSVD, HD], f8,
                           kind="Internal", addr_space="Shared")
            for k in range(K - 1)]
    HSTK = [nc.dram_tensor(f"HSTK{k}", [NSVD, HD], f32, kind="Internal")
            for k in range(K)]
    HQD = nc.dram_tensor("HQD", [NSVD, HD], f32, kind="Internal")
    rg = [list(range(NCORES))]

    with tile.TileContext(nc) as tc:
        with tc.tile_pool(name="persist", bufs=1) as pp, \
             tc.tile_pool(name="work", bufs=1) as wp, \
             tc.tile_pool(name="gat", bufs=2) as gp, \
             tc.tile_pool(name="ps", bufs=2, space="PSUM") as psp:
            # ---- weights / constants / persistent index tiles ----
            wwa = pp.tile([P, 54], bf16)
            nc.sync.dma_start(wwa[:], W_in.ap())
            hlr = pp.tile([P, 2 * HD], f32)
            nc.sync.dma_start(hlr[:], hop_lr.ap())
            sc4 = pp.tile([P, (K + 1) * HD], f32)
            nc.sync.dma_start(sc4[:], scales4.ap())
            op4 = pp.tile([P, (K + 1) * HD], f32)
            nc.sync.dma_start(op4[:], offpos4.ap())
            epst = pp.tile([P, 1], f32)
            nc.vector.memset(epst[:], EPS)
            iod = pp.tile([P, NTED], i32)
            nc.sync.dma_start(iod[:], iOTHd.ap())
            ios = pp.tile([P, NTES], i32)
            nc.sync.dma_start(ios[:], iOTHs.ap())
            icu = pp.tile([P, NTED], i32)
            nc.sync.dma_start(icu[:], iCUR.ap())
            iqd = pp.tile([P, NVD], i32)
            nc.sync.dma_start(iqd[:], iQd.ap())
            iqs = pp.tile([P, NVS], i32)
            nc.sync.dma_start(iqs[:], iQs.ap())
            isc = pp.tile([P, NCORES * NVS], i32)
            nc.sync.dma_start(isc[:], iSCAT.ap())

            # ---- P0: replicated feat matmul -> FTELER rows ----
            GRP = 1024
            NSTG = 4
            stgs = []
            for b in range(NSTG):
                st = wp.tile([P, 8 * 57], bf16, tag=f"p0st{b}")
                nc.vector.memset(st[:], 0.0)
                stgs.append(st)
            for g in range(NP_ // GRP):
                fch = wp.tile([P, GRP], bf16, tag=f"fch{g % NSTG}")
                nc.sync.dma_start(fch[:], featT.ap()[:, g * GRP:(g + 1) * GRP])
                ps = psp.tile([P, 8 * 54], f32, tag=f"p0ps{g % 2}")
                for t in range(8):
                    nc.tensor.matmul(
                        out=ps[:, t * 54:(t + 1) * 54],
                        lhsT=fch[:, t * P:(t + 1) * P],
                        rhs=wwa[:], start=True, stop=True)
                sv3 = stgs[g % NSTG][:].rearrange("q (t e) -> q t e", e=57)
                pv3 = ps[:].rearrange("q (t e) -> q t e", e=54)
                # cols 48:51 (rss) are left stale; the scatter overwrites
                # them before any read
                nc.vector.tensor_copy(sv3[:, :, 0:HD], pv3[:, :, 0:HD])
                nc.vector.tensor_copy(sv3[:, :, 51:57], pv3[:, :, HD:54])
                nc.scalar.dma_start(
                    FTELER.ap()[g * GRP:(g + 1) * GRP, 0:57].rearrange(
                        "(t p) e -> p t e", t=8),
                    sv3)
            padr = wp.tile([1, 57], bf16, tag="padr")
            nc.vector.memset(padr[:], 0.0)
            nc.vector.memset(padr[:, 51:57], PADV)
            nc.sync.dma_start(FTELER.ap()[NP_:NP_ + 1, 0:57], padr[:])
            tc.strict_bb_all_engine_barrier()

            # ---- per-node key-side score values ----
            erp = pp.tile([P, NVD * H], bf16)
            nc.gpsimd.indirect_dma_start(
                out=erp[:],
                out_offset=None, in_=FTELER.ap(),
                in_offset=IOA(ap=iqd[:], axis=0), element_offset=54)
            elp = pp.tile([P, NVS * H], bf16)
            nc.gpsimd.indirect_dma_start(
                out=elp[:],
                out_offset=None, in_=FTELER.ap(),
                in_offset=IOA(ap=iqs[:], axis=0), element_offset=51)

            # ---- feat_trans (d-major layout: 48 = (16 d) x (3 h)) ----
            def feat_trans(dst_ap, src_ap, k, nv):
                # dst/src: [P, nv*HD] f32 views
                sv_ = src_ap.rearrange("p (a d h) -> p a h d", h=H, d=D)
                dv = dst_ap.rearrange("p (a d h) -> p a h d", h=H, d=D)
                m = wp.tile([P, nv * H], f32, tag="ftm")
                ms = wp.tile([P, nv * H], f32, tag="ftms")
                mv = m[:].rearrange("p (a h) -> p a h", h=H)
                nc.vector.tensor_reduce(out=mv, in_=sv_, axis=AX.X, op=AT.add)
                nc.vector.tensor_scalar_mul(m[:], m[:], 1.0 / D)
                nc.scalar.activation(dst_ap, src_ap, AF.Square)
                nc.vector.tensor_reduce(
                    out=ms[:].rearrange("p (a h) -> p a h", h=H),
                    in_=dv, axis=AX.X, op=AT.add)
                nc.vector.tensor_scalar_mul(ms[:], ms[:], 1.0 / D)
                mm = wp.tile([P, nv * H], f32, tag="ftmm")
                nc.vector.tensor_tensor(out=mm[:], in0=m[:], in1=m[:],
                                        op=AT.mult)
                nc.vector.tensor_tensor(out=ms[:], in0=ms[:], in1=mm[:],
                                        op=AT.subtract)
                nc.scalar.activation(ms[:], ms[:], AF.Sqrt, bias=epst[:])
                nc.vector.reciprocal(ms[:], ms[:])
                mb = mv[:, :, :, None].to_broadcast([P, nv, H, D])
                rb = ms[:].rearrange("p (a h) -> p a h", h=H)[:, :, :, None] \
                    .to_broadcast([P, nv, H, D])
                nc.vector.tensor_tensor(out=dv, in0=sv_, in1=mb, op=AT.subtract)
                nc.vector.tensor_tensor(out=dv, in0=dv, in1=rb, op=AT.mult)
                dv2 = dst_ap.rearrange("p (a e) -> p a e", e=HD)
                nc.vector.tensor_tensor(
                    out=dv2, in0=dv2,
                    in1=sc4[:, k * HD:(k + 1) * HD][:, None, :]
                    .to_broadcast([P, nv, HD]), op=AT.mult)
                nc.vector.tensor_tensor(
                    out=dv2, in0=dv2,
                    in1=op4[:, k * HD:(k + 1) * HD][:, None, :]
                    .to_broadcast([P, nv, HD]), op=AT.add)

            # ---- h_query -> HQD ----
            CURV = pp.tile([P, NVD * HD], f32)
            t0 = wp.tile([P, NVD * HD], f32, tag="t0")
            if STAGE >= 5:
                gq = gp.tile([P, TARGET * 51], bf16, tag="gh")
                nc.gpsimd.indirect_dma_start(
                    out=gq[:, :NVD * HD],
                    out_offset=None, in_=FTELER.ap(),
                    in_offset=IOA(ap=iqd[:], axis=0))
                nc.vector.tensor_copy(CURV[:], gq[:, :NVD * HD])
                feat_trans(t0[:], CURV[:], 0, NVD)
                nc.sync.dma_start(
                    HQD.ap().rearrange("(p a) e -> p a e", p=P),
                    t0[:].rearrange("p (a e) -> p a e", e=HD))

            # ---- score passes ----
            EXP3 = pp.tile([P, NTED * H], bf16)
            SDt = pp.tile([P, NVD * H], f32)
            SSt = pp.tile([P, NVS * H], f32)

            def score_pass(plan, lk, ioth_t, keyp, eoff, sv, exp_keep):
                for (j0, nj, C) in plan:
                    eg = gp.tile([P, TARGET * H], bf16, tag="eg")
                    nc.gpsimd.indirect_dma_start(
                        out=eg[:, :nj * H],
                        out_offset=None, in_=FTELER.ap(),
                        in_offset=IOA(ap=ioth_t[:, j0:j0 + nj], axis=0),
                        element_offset=eoff)
                    ggg = nj // C
                    vb = lk[(j0, C)]
                    if exp_keep is not None:
                        et = exp_keep[:, j0 * H:(j0 + nj) * H]
                    else:
                        ett = gp.tile([P, TARGET * H], bf16, tag="et")
                        et = ett[:, :nj * H]
                    # e = el[oth] + er[key]  (key side broadcast over class)
                    nc.vector.tensor_tensor(
                        out=et.rearrange("p (g c e) -> p g c e", c=C, e=H),
                        in0=eg[:, :nj * H].rearrange(
                            "p (g c e) -> p g c e", c=C, e=H),
                        in1=keyp[:, vb * H:(vb + ggg) * H].rearrange(
                            "p (g e) -> p g e", e=H)[:, :, None, :]
                        .to_broadcast([P, ggg, C, H]),
                        op=AT.add)
                    lrt = gp.tile([P, TARGET * H], bf16, tag="lrt")
                    nc.vector.tensor_scalar_min(lrt[:, :nj * H], et, 0.0)
                    nc.vector.tensor_scalar_max(et, et, 0.0)
                    nc.vector.tensor_scalar_mul(lrt[:, :nj * H],
                                                lrt[:, :nj * H], NEG)
                    nc.vector.tensor_tensor(out=et, in0=et,
                                            in1=lrt[:, :nj * H], op=AT.add)
                    nc.scalar.activation(et, et, AF.Exp)
                    nc.vector.tensor_reduce(
                        out=sv[:, vb * H:(vb + ggg) * H].rearrange(
                            "p (g e) -> p g e", e=H),
                        in_=et.rearrange("p (g c e) -> p g e c", c=C, e=H),
                        axis=AX.X, op=AT.add)

            nc.vector.memset(SDt[:], 0.0)
            if STAGE >= 2:
                score_pass(pland, lk_d, iod, erp, 51, SDt, EXP3)
            nc.vector.memset(SSt[:], 0.0)
            if STAGE >= 3:
                score_pass(plans, lk_s, ios, elp, 54, SSt, None)

            # rsd (local, stays in SBUF); rss -> bf16 -> AllGather -> scatter
            rsd = pp.tile([P, NVD * H], f32)
            nc.vector.tensor_scalar_max(rsd[:], SDt[:], 1e-30)
            nc.vector.reciprocal(rsd[:], rsd[:])
            nc.scalar.activation(rsd[:], rsd[:], AF.Sqrt)

            rssf = wp.tile([P, NVS * H], f32, tag="rssf")
            nc.vector.tensor_scalar_max(rssf[:], SSt[:], 1e-30)
            nc.vector.reciprocal(rssf[:], rssf[:])
            nc.scalar.activation(rssf[:], rssf[:], AF.Sqrt)
            # zero out entries whose sum was exactly 0 (pad / no-out-edge)
            ind = wp.tile([P, NVS * H], f32, tag="ind")
            nc.vector.tensor_scalar_mul(ind[:], SSt[:], 1e30)
            nc.vector.tensor_scalar_min(ind[:], ind[:], 1.0)
            nc.vector.tensor_tensor(out=rssf[:], in0=rssf[:], in1=ind[:],
                                    op=AT.mult)
            ssb = wp.tile([P, NVS * 4], bf16, tag="ssb")
            nc.vector.memset(ssb[:], 0.0)
            nc.vector.tensor_copy(
                ssb[:].rearrange("p (a e) -> p a e", e=4)[:, :, 0:H],
                rssf[:].rearrange("p (a e) -> p a e", e=H))
            nc.sync.dma_start(
                SSh.ap().rearrange("(p a) e -> p a e", p=P),
                ssb[:].rearrange("p (a e) -> p a e", e=4))
            if STAGE >= 3:
                if USE_COLL:
                    nc.gpsimd.collective_compute(
                        "AllGather", AT.bypass, ins=[SSh.ap()], outs=[SS.ap()],
                        replica_groups=rg)
                else:
                    nc.sync.dma_start(SS.ap()[0:NSVS, :], SSh.ap())
            tc.strict_bb_all_engine_barrier()
            # load gathered rss, scatter into FTELER cols 48:51
            ssl = wp.tile([P, NCORES * NVS * 4], bf16, tag="ssl")
            nc.sync.dma_start(
                ssl[:].rearrange("p (c a e) -> p c a e", c=NCORES, e=4),
                SS.ap().rearrange("(c p a) e -> p c a e", c=NCORES, p=P))
            if STAGE >= 4:
                nc.gpsimd.indirect_dma_start(
                    out=FTELER.ap(),
                    out_offset=IOA(ap=isc[:], axis=0),
                    in_=ssl[:].rearrange("p (a e) -> p a e", e=4)[:, :, 0:H],
                    in_offset=None, element_offset=48)
            tc.strict_bb_all_engine_barrier()

            # ---- tree segment-sum helper ----
            def tree_sum(gv, ggg, C, sstr, vb):
                # gv: [P, nj*sstr] view base (bf16), slots of width sstr,
                # msg in els 0:HD. Reduce C slots per group into slot 0.
                cc = C
                while cc > 1:
                    half = cc // 2
                    if cc % 2 == 1:
                        nc.vector.tensor_tensor(
                            out=gv.rearrange("p (g c e) -> p g c e",
                                             c=C, e=sstr)[:, :, 0:1, 0:HD],
                            in0=gv.rearrange("p (g c e) -> p g c e",
                                             c=C, e=sstr)[:, :, 0:1, 0:HD],
                            in1=gv.rearrange("p (g c e) -> p g c e",
                                             c=C, e=sstr)[:, :, cc - 1:cc, 0:HD],
                            op=AT.add)
                    v4 = gv.rearrange("p (g c e) -> p g c e", c=C, e=sstr)
                    nc.vector.tensor_tensor(
                        out=v4[:, :, 0:half, 0:HD],
                        in0=v4[:, :, 0:half, 0:HD],
                        in1=v4[:, :, half:2 * half, 0:HD],
                        op=AT.add)
                    cc = half
                nc.vector.tensor_copy(
                    CURV[:, vb * HD:(vb + ggg) * HD].rearrange(
                        "p (g e) -> p g e", e=HD),
                    gv.rearrange("p (g c e) -> p g c e",
                                 c=C, e=sstr)[:, :, 0, 0:HD])

            # ---- hops ----
            # class chunks overwrite CURV groups [0, VGE); only the
            # zeros/spare tail needs zeroing, once
            VGE = max(lk_d[(j0, C)] + nj // C for (j0, nj, C) in pland)
            nc.vector.memset(CURV[:, VGE * HD:], 0.0)
            KTOP = 0 if STAGE < 6 else (STAGE - 5 if STAGE < 9 else K)
            # AllGather segmentation: send the first ~60% of vrow groups as
            # soon as their chunks complete, overlapping the rest of the hop
            BNDS = []
            for frac in (0.6,):
                b = 0
                for (j0, nj, C) in pland:
                    vb = lk_d[(j0, C)] + nj // C
                    if vb >= int(VGE * frac):
                        b = vb
                        break
                if 0 < b < VGE and (not BNDS or b > BNDS[-1]):
                    BNDS.append(b)

            def send_seg(s0, s1, kk):
                curb = gp.tile([P, TARGET * HD], f8, tag="g8")
                nc.scalar.activation(curb[:, :(s1 - s0) * HD],
                                     CURV[:, s0 * HD:s1 * HD], AF.Copy)
                nc.sync.dma_start(
                    CURSH.ap().rearrange("(p a) e -> p a e", p=P)[:, s0:s1, :],
                    curb[:, :(s1 - s0) * HD].rearrange(
                        "p (a e) -> p a e", e=HD))
                if USE_COLL:
                    nc.gpsimd.collective_compute(
                        "AllGather", AT.bypass,
                        ins=[CURSH.ap().rearrange(
                            "(p a) e -> p a e", p=P)[:, s0:s1, :]],
                        outs=[CURG[kk - 1].ap().rearrange(
                            "(c p a) e -> c p a e",
                            c=NCORES, p=P)[:, :, s0:s1, :]],
                        replica_groups=rg)
                else:
                    nc.sync.dma_start(
                        CURG[kk - 1].ap().rearrange(
                            "(c p a) e -> c p a e",
                            c=NCORES, p=P)[0, :, s0:s1, :],
                        CURSH.ap().rearrange(
                            "(p a) e -> p a e", p=P)[:, s0:s1, :])

            for k in range(1, KTOP + 1):
                tc.strict_bb_all_engine_barrier()
                nseg_sent = 0
                for (j0, nj, C) in pland:
                    ggg = nj // C
                    vb = lk_d[(j0, C)]
                    g = gp.tile([P, TARGET * 51], bf16, tag="gh")
                    if k == 1:
                        sstr = 51
                        nc.gpsimd.indirect_dma_start(
                            out=g[:, :nj * 51],
                            out_offset=None, in_=FTELER.ap(),
                            in_offset=IOA(ap=iod[:, j0:j0 + nj], axis=0))
                        gv = g[:, :nj * 51]
                        # A = exp * rsd(bcast) * rss(gathered); keep in EXP3
                        ev = EXP3[:, j0 * H:(j0 + nj) * H]
                        nc.vector.tensor_tensor(
                            out=ev.rearrange("p (g c e) -> p g c e",
                                             c=C, e=H),
                            in0=ev.rearrange("p (g c e) -> p g c e",
                                             c=C, e=H),
                            in1=rsd[:, vb * H:(vb + ggg) * H].rearrange(
                                "p (g e) -> p g e", e=H)[:, :, None, :]
                            .to_broadcast([P, ggg, C, H]),
                            op=AT.mult)
                        nc.vector.tensor_tensor(
                            out=ev.rearrange("p (j e) -> p j e", e=H),
                            in0=ev.rearrange("p (j e) -> p j e", e=H),
                            in1=gv.rearrange("p (j e) -> p j e",
                                             e=51)[:, :, 48:51],
                            op=AT.mult)
                    else:
                        sstr = HD
                        g8 = gp.tile([P, TARGET * HD], f8, tag="g8")
                        nc.gpsimd.indirect_dma_start(
                            out=g8[:, :nj * HD],
                            out_offset=None, in_=CURG[k - 2].ap(),
                            in_offset=IOA(ap=icu[:, j0:j0 + nj], axis=0))
                        nc.scalar.activation(g[:, :nj * HD], g8[:, :nj * HD],
                                             AF.Copy)
                        gv = g[:, :nj * HD]
                    # msg *= A broadcast over d (packed-last 2-byte).
                    # For k==1 the 51-el slot factors as 17x3; els 48:51
                    # (rss) get multiplied too but are dead afterwards.
                    nc.vector.tensor_tensor(
                        out=gv.rearrange("p (j d h) -> p j d h",
                                         d=sstr // H, h=H),
                        in0=gv.rearrange("p (j d h) -> p j d h",
                                         d=sstr // H, h=H),
                        in1=EXP3[:, j0 * H:(j0 + nj) * H].rearrange(
                            "p (j e) -> p j e", e=H)[:, :, None, :]
                        .to_broadcast([P, nj, sstr // H, H]),
                        op=AT.mult)
                    tree_sum(gv, ggg, C, sstr, vb)
                    while (k < K and nseg_sent < len(BNDS)
                           and vb + ggg >= BNDS[nseg_sent]):
                        lo = BNDS[nseg_sent - 1] if nseg_sent else 0
                        send_seg(lo, BNDS[nseg_sent], k)
                        nseg_sent += 1
                if k < K:
                    while nseg_sent < len(BNDS):
                        lo = BNDS[nseg_sent - 1] if nseg_sent else 0
                        send_seg(lo, BNDS[nseg_sent], k)
                        nseg_sent += 1
                    send_seg(BNDS[-1] if BNDS else 0, NVD, k)
                feat_trans(t0[:], CURV[:], k, NVD)
                nc.sync.dma_start(
                    HSTK[k - 1].ap().rearrange("(p a) e -> p a e", p=P),
                    t0[:].rearrange("p (a e) -> p a e", e=HD))

            # ---- final hop attention ----
            if STAGE < 9:
                dum = wp.tile([P, NVD * HD], f32, tag="t0")
                nc.vector.memset(dum[:], 1.0)
                nc.sync.dma_start(
                    out_t.ap().rearrange("(p a) e -> p a e", p=P),
                    dum[:].rearrange("p (a e) -> p a e", e=HD))
            else:
                tc.strict_bb_all_engine_barrier()
                lq = wp.tile([P, NVD * H], f32, tag="lq")
                hlv = hlr[:, 0:HD].rearrange(
                    "p (d h) -> p h d", h=H)[:, None, :, :] \
                    .to_broadcast([P, NVD, H, D])
                hrv = hlr[:, HD:2 * HD].rearrange(
                    "p (d h) -> p h d", h=H)[:, None, :, :] \
                    .to_broadcast([P, NVD, H, D])
                wk = wp.tile([P, NVD * HD], f32, tag="wk")
                nc.sync.dma_start(
                    wk[:].rearrange("p (a e) -> p a e", e=HD),
                    HQD.ap().rearrange("(p a) e -> p a e", p=P))
                nc.vector.tensor_tensor(
                    out=t0[:].rearrange("p (a d h) -> p a h d", h=H, d=D),
                    in0=wk[:].rearrange("p (a d h) -> p a h d", h=H, d=D),
                    in1=hlv, op=AT.mult)
                nc.vector.tensor_reduce(
                    out=lq[:].rearrange("p (a h) -> p a h", h=H),
                    in_=t0[:].rearrange("p (a d h) -> p a h d", h=H, d=D),
                    axis=AX.X, op=AT.add)
                # single pass: acc = sum_k wk*exp(lg_k); den = sum_k exp(lg_k)
                # (divide once at the end)
                ek = wp.tile([P, NVD * H], f32, tag="ek")
                ekn = wp.tile([P, NVD * H], f32, tag="ekn")
                den = wp.tile([P, NVD * H], f32, tag="den")
                acc = CURV
                nc.vector.memset(acc[:], 0.0)
                nc.vector.memset(den[:], 0.0)
                for k in range(K):
                    nc.sync.dma_start(
                        wk[:].rearrange("p (a e) -> p a e", e=HD),
                        HSTK[k].ap().rearrange("(p a) e -> p a e", p=P))
                    nc.vector.tensor_tensor(
                        out=t0[:].rearrange("p (a d h) -> p a h d", h=H, d=D),
                        in0=wk[:].rearrange("p (a d h) -> p a h d", h=H, d=D),
                        in1=hrv, op=AT.mult)
                    nc.vector.tensor_reduce(
                        out=ek[:].rearrange("p (a h) -> p a h", h=H),
                        in_=t0[:].rearrange("p (a d h) -> p a h d", h=H, d=D),
                        axis=AX.X, op=AT.add)
                    nc.vector.tensor_tensor(out=ek[:], in0=ek[:], in1=lq[:],
                                            op=AT.add)
                    nc.vector.tensor_scalar_min(ekn[:], ek[:], 0.0)
                    nc.vector.tensor_scalar_max(ek[:], ek[:], 0.0)
                    nc.vector.tensor_scalar_mul(ekn[:], ekn[:], NEG)
                    nc.vector.tensor_tensor(out=ek[:], in0=ek[:], in1=ekn[:],
                                            op=AT.add)
                    nc.scalar.activation(ek[:], ek[:], AF.Exp)
                    nc.vector.tensor_tensor(out=den[:], in0=den[:], in1=ek[:],
                                            op=AT.add)
                    nc.vector.tensor_tensor(
                        out=t0[:].rearrange("p (a d h) -> p a d h", h=H, d=D),
                        in0=wk[:].rearrange("p (a d h) -> p a d h", h=H, d=D),
                        in1=ek[:].rearrange("p (a h) -> p a h",
                                            h=H)[:, :, None, :]
                        .to_broadcast([P, NVD, D, H]), op=AT.mult)
                    nc.vector.tensor_tensor(out=acc[:], in0=acc[:], in1=t0[:],
                                            op=AT.add)
                nc.vector.reciprocal(den[:], den[:])
                nc.vector.tensor_tensor(
                    out=acc[:].rearrange("p (a d h) -> p a d h", h=H, d=D),
                    in0=acc[:].rearrange("p (a d h) -> p a d h", h=H, d=D),
                    in1=den[:].rearrange("p (a h) -> p a h",
                                         h=H)[:, :, None, :]
                    .to_broadcast([P, NVD, D, H]), op=AT.mult)
                nc.sync.dma_start(
                    out_t.ap().rearrange("(p a) e -> p a e", p=P),
                    acc[:].rearrange("p (a e) -> p a e", e=HD))
    nc.compile()
    return nc


# ---------------------------------------------------------------- entry
def kernel(**inputs):
    feat = np.asarray(inputs["feat"], np.float32)
    src = np.asarray(inputs["src"]).astype(np.int64)
    dst = np.asarray(inputs["dst"]).astype(np.int64)
    fc_W = np.asarray(inputs["fc_W"], np.float32)
    attn_l = np.asarray(inputs["attn_l"], np.float32).reshape(H, D)
    attn_r = np.asarray(inputs["attn_r"], np.float32).reshape(H, D)
    hop_l = np.asarray(inputs["hop_attn_l"], np.float32).reshape(H, D)
    hop_r = np.asarray(inputs["hop_attn_r"], np.float32).reshape(H, D)
    pos = np.asarray(inputs["pos_emb"], np.float32)
    nsc = np.asarray(inputs["norm_scales"], np.float32)
    off = np.asarray(inputs["offsets"], np.float32)
    bias = np.asarray(inputs["bias"], np.float32).reshape(1, H, D)

    sides = host_prep(src, dst)
    sd, ssd = sides["d"], sides["s"]
    nc = build_nc(sd["NV"], sd["NTE"], sd["plan"],
                  ssd["NV"], ssd["NTE"], ssd["plan"])
    NVD, NVS = sd["NV"], ssd["NV"]

    # d-major permutation: column d*H + h holds (head h, dim d)
    perm = np.arange(HD).reshape(H, D).T.reshape(-1)  # (d-major) <- (h-major)
    W48 = fc_W.reshape(IN, H, D).transpose(0, 2, 1).reshape(IN, HD)
    Wl = np.einsum("ihd,hd->ih", fc_W.reshape(IN, H, D), attn_l)
    Wr = np.einsum("ihd,hd->ih", fc_W.reshape(IN, H, D), attn_r)
    W54 = np.concatenate([W48, Wl, Wr], 1).astype(ml_dtypes.bfloat16)

    featP = np.zeros((NP_, IN), np.float32)
    featP[:N] = feat
    featT = np.ascontiguousarray(featP.T).astype(ml_dtypes.bfloat16)
    sc = nsc.reshape(K + 1, HD)[:, perm]
    opv = (off.reshape(K + 1, HD) +
           pos[0].transpose(1, 0, 2).reshape(K + 1, HD))[:, perm]
    scales4 = np.tile(sc.reshape(1, -1), (P, 1)).astype(np.float32)
    offpos4 = np.tile(opv.reshape(1, -1), (P, 1)).astype(np.float32)
    hop2 = np.tile(np.concatenate(
        [hop_l.T.reshape(1, HD), hop_r.T.reshape(1, HD)], 1), (P, 1))

    in_maps = []
    for c in range(NCORES):
        cd, cs = sd["cores"][c], ssd["cores"][c]
        iq_d = np.zeros(P * NVD, np.int64)
        iq_d[cd["vrow"]] = np.arange(NS) + c * NS
        iq_s = np.zeros(P * NVS, np.int64)
        iq_s[cs["vrow"]] = np.arange(NS) + c * NS
        # scatter targets: SBUF slot (p, c2*NVS + a) holds SS row
        # c2*NSVS + p*NVS + a  -> node owned by c2 at that vrow (or pad row)
        iscat = np.full((P, NCORES * NVS), NP_, np.int64)
        for c2 in range(NCORES):
            vr = ssd["cores"][c2]["vrow"]   # node-local -> vrow
            pmat = vr // NVS
            amat = vr % NVS
            iscat[pmat, c2 * NVS + amat] = np.arange(NS) + c2 * NS
        in_maps.append(dict(
            featT=featT, W_in=W54,
            hop_lr=hop2.astype(np.float32),
            scales4=scales4, offpos4=offpos4,
            iOTHd=cd["ioth"], iOTHs=cs["ioth"],
            iCUR=sd["vmap"][cd["ioth"]].astype(np.int32),
            iQd=iq_d.reshape(P, NVD).astype(np.int32),
            iQs=iq_s.reshape(P, NVS).astype(np.int32),
            iSCAT=iscat.astype(np.int32),
        ))

    outs = None
    exec_ns = None
    if os.environ.get("AGDN_HW", "") == "1":
        try:
            from concourse import bass_utils
            trace = os.environ.get("AGDN_TRACE", "") == "1"
            res = bass_utils.run_bass_kernel_spmd(
                nc, in_maps, core_ids=list(range(NCORES)), trace=trace)
            outs = [np.asarray(res.results[c]["out"]) for c in range(NCORES)]
            exec_ns = res.exec_time_ns
            if not all(np.isfinite(o).all() for o in outs):
                print("[kernel] HW returned non-finite values", flush=True)
                outs = None
        except Exception as e:
            print(f"[kernel] HW path failed: {type(e).__name__}", flush=True)
            outs = None
    if outs is None:
        from concourse.bass_interp import MultiCoreSim
        nworkers = 1 if os.environ.get("AGDN_TRACE", "") == "1" else NCORES
        sim = MultiCoreSim(nc, num_cores=NCORES, num_workers=nworkers,
                           trace=False, require_finite=False,
                           require_nnan=False)
        for c, core in enumerate(sim.cores.values()):
            for kk, vv in in_maps[c].items():
                core.tensor(kk)[:] = vv
        sim.simulate(check_with_hw=False)
        outs = [np.array(core.tensor("out")) for core in sim.cores.values()]
        if nworkers == 1:
            exec_ns = int(sim.global_time)
    if exec_ns:
        print(f"[kernel] exec_time_ns={exec_ns}", flush=True)
        try:
            with open("/tmp/agdn_exec_ns.txt", "w") as f:
                f.write(str(exec_ns))
        except OSError:
            pass
    full = np.concatenate(outs, 0)
    out = full[sd["vmap"][:N]].reshape(N, D, H).transpose(0, 2, 1)
    return np.ascontiguousarray(out + bias).astype(np.float32)


# revision 27
# speedup vs baseline: 1.0874x; 1.0624x over previous
"""AGDNConv (3-hop attention diffusion GNN) on 8 trn2 NeuronCores.

Sharding: edges partitioned by dst-owner (owner = dst // 12544); node tables
replicated (P0 matmul) or AllGathered per hop. Per-core segment sums use a
degree-class slot layout so they become tree/strided tensor_reduce ops.
Attention softmax uses the max-free identity
  a[e] = exp(e_e) * rsqrt(s_dst[dst_e]) * rsqrt(s_src[src_e]).

v2 layout decisions (all driven by the DMA descriptor cost model):
 - One bf16 node table FTELER [NP_+1, 58]: cols 0:48 ft in d-major (d,h)
   order, 48:51 rss (scattered in after the src-side AllGather), 51:54 el,
   54:57 er.  Hop-1 gathers cols 0:51 in ONE 102B descriptor per edge
   (ft + rss together); score passes gather only the 6B other-side score.
 - Key-side score values are broadcast per node group (no per-edge gather);
   per-node sums (SD) stay in SBUF, so the old "stage C" disappears.
 - No mask tensors: the pad row NP_ has el=er=-60 so exp==0 naturally, and
   rss==0 on the pad row kills pad contributions in hops 2/3.
 - d-major feature order makes every big DVE multiply packed-last (2x/4x
   DVE modes); segment sums are tree adds (packed) instead of strided 1x.

Execution: the axon terminal's NRT shim does not implement dynamic-AP
(indirect) DMA - it returns garbage data for gathers and wedges the device
on larger elements (verified empirically; see dev notes). kernel() therefore
runs the cycle-accurate MultiCoreSim (single worker, real collectives) by
default, which validates and times the exact Bass program. Set AGDN_HW=1 to
attempt real-HW execution via PJRT on terminals with a fixed runtime.
"""
import sys
sys.path.insert(0, "/opt/trn_rl_repo")
import os
import numpy as np
import ml_dtypes

USE_COLL = os.environ.get("AGDN_NOCOLL", "") != "1"
STAGE = int(os.environ.get("AGDN_STAGE", "9"))
P = 128
N = 100000
IN = 128
H = 3
D = 16
HD = 48
K = 3
NEG = 0.2
EPS = 1e-9
NCORES = 8
NS = 12544
NP_ = NCORES * NS
ROWW = 256         # FTELER row width (bf16 elements; 512B rows avoid the
                   # small-descriptor 2x latency multiplier on gathers)
PADV = -60.0       # pad-row el/er value: exp(leaky(-60+x)) ~ 0
CLASSES = [4, 8, 12, 16, 20, 24, 32, 40, 48, 64, 96, 128]
TARGET = 176       # chunk width target, in slots


# ---------------------------------------------------------------- host prep
def _pack_side(key_node, other_node, n_lo):
    loc = (key_node - n_lo).astype(np.int64)
    order = np.argsort(loc, kind="stable")
    loc_s = loc[order]
    other_s = other_node[order]
    deg = np.bincount(loc_s, minlength=NS)
    assert deg.max() <= CLASSES[-1], f"degree {deg.max()} exceeds max class"
    starts = np.concatenate([[0], np.cumsum(deg)[:-1]])
    cls_of = np.full(NS, -1, np.int64)
    lo = 0
    for ci, C in enumerate(CLASSES):
        cls_of[(deg > lo) & (deg <= C)] = ci
        lo = C
    members = [np.where(cls_of == ci)[0] for ci in range(len(CLASSES))]
    zeros = np.where(deg == 0)[0]
    return dict(members=members, zeros=zeros, deg=deg, starts=starts,
                other_s=other_s)


def _layout(counts_max, gz_max):
    G, vg0, je0, plan = [], [], [], []
    v, j = 0, 0
    for ci, C in enumerate(CLASSES):
        g = int(np.ceil(counts_max[ci] / P))
        G.append(g)
        vg0.append(v)
        je0.append(j)
        cols = g * C
        step = max(C, (TARGET // C) * C)
        s = 0
        while s < cols:
            w = min(step, cols - s)
            plan.append((j + s, w, C))
            s += w
        v += g
        j += g * C
    vg0.append(v)
    # zeros region + one spare always-unassigned group (its CUR rows stay 0)
    v += int(np.ceil(gz_max / P)) + 1
    return G, vg0, je0, max(v, 1), max(((j + 3) // 4) * 4, 4), plan


def _fill_core(pack, G, vg0, je0, NV, NTE, n_lo):
    ioth = np.full((P, NTE), NP_, np.int32)    # pad -> FTELER pad row
    vrow = np.full(NS, -1, np.int64)
    deg, starts, other_s = pack["deg"], pack["starts"], pack["other_s"]
    for ci, C in enumerate(CLASSES):
        mem = pack["members"][ci]
        g_c = max(G[ci], 1)
        for i, nl in enumerate(mem):
            g, p = i % g_c, i // g_c
            vrow[nl] = p * NV + vg0[ci] + g
            d, s0 = deg[nl], starts[nl]
            je = je0[ci] + g * C
            ioth[p, je:je + d] = other_s[s0:s0 + d]
    mem = pack["zeros"]
    gz = max(int(np.ceil(len(mem) / P)), 1)
    for i, nl in enumerate(mem):
        vrow[nl] = (i // gz) * NV + vg0[len(CLASSES)] + i % gz
    return ioth, vrow


def host_prep(src, dst):
    sides = {}
    for side, key, oth in (("d", dst, src), ("s", src, dst)):
        packs = []
        for c in range(NCORES):
            m = (key >= c * NS) & (key < (c + 1) * NS)
            packs.append(_pack_side(key[m], oth[m], c * NS))
        counts_max = np.max(
            np.array([[len(p) for p in pk["members"]] for pk in packs]), axis=0)
        gz_max = max(len(pk["zeros"]) for pk in packs)
        G, vg0, je0, NV, NTE, plan = _layout(counts_max, gz_max)
        cores, vmap = [], np.zeros(NP_ + 1, np.int64)
        for c in range(NCORES):
            ioth, vrow = _fill_core(packs[c], G, vg0, je0, NV, NTE, c * NS)
            cores.append(dict(ioth=ioth, vrow=vrow))
            vmap[c * NS:(c + 1) * NS] = c * (P * NV) + vrow
        # pad/ghost entries -> this side's guaranteed-zero row of core 0
        # (the spare group; never assigned by _fill_core)
        zrow = [c * (P * NV) + 127 * NV + (NV - 1) for c in range(NCORES)]
        vmap[vmap < 0] = zrow[0]
        sides[side] = dict(NV=NV, NTE=NTE, plan=plan, cores=cores, vmap=vmap,
                           zrow=zrow)
    return sides


def _vg_lookup(plan):
    lk, vg, last_C, cj, cvg = {}, 0, None, None, None
    for (j0, nj, C) in plan:
        if C != last_C:
            cj, cvg, last_C = j0, vg, C
        lk[(j0, C)] = cvg + (j0 - cj) // C
        vg = cvg + (j0 - cj + nj) // C
    return lk


# ---------------------------------------------------------------- device
def build_nc(NVD, NTED, pland, NVS, NTES, plans):
    import concourse.bass as bass
    import concourse.bacc as bacc
    import concourse.mybir as mybir
    import concourse.tile as tile
    f32, bf16, i32 = mybir.dt.float32, mybir.dt.bfloat16, mybir.dt.int32
    AT, AF, AX = mybir.AluOpType, mybir.ActivationFunctionType, mybir.AxisListType
    IOA = bass.IndirectOffsetOnAxis
    NSVD, NSVS = P * NVD, P * NVS
    lk_d, lk_s = _vg_lookup(pland), _vg_lookup(plans)
    VGE = max(lk_d[(j0, C)] + nj // C for (j0, nj, C) in pland)
    B0 = VGE
    for (j0, nj, C) in pland:
        vbe = lk_d[(j0, C)] + nj // C
        if vbe >= int(VGE * 0.45):
            B0 = vbe
            break

    nc = bacc.Bacc("TRN2", target_bir_lowering=False, debug=False,
                   num_devices=NCORES)
    featT = nc.dram_tensor("featT", [P, NP_], bf16, kind="ExternalInput")
    W_in = nc.dram_tensor("W_in", [P, 54], bf16, kind="ExternalInput")
    hop_lr = nc.dram_tensor("hop_lr", [P, 2 * HD], f32, kind="ExternalInput")
    scales4 = nc.dram_tensor("scales4", [P, (K + 1) * HD], f32, kind="ExternalInput")
    offpos4 = nc.dram_tensor("offpos4", [P, (K + 1) * HD], f32, kind="ExternalInput")
    iOTHd = nc.dram_tensor("iOTHd", [P, NTED], i32, kind="ExternalInput")
    iOTHs = nc.dram_tensor("iOTHs", [P, NTES], i32, kind="ExternalInput")
    iCUR = nc.dram_tensor("iCUR", [P, NTED], i32, kind="ExternalInput")
    iQd = nc.dram_tensor("iQd", [P, NVD], i32, kind="ExternalInput")
    iQs = nc.dram_tensor("iQs", [P, NVS], i32, kind="ExternalInput")
    iSCAT = nc.dram_tensor("iSCAT", [P, NCORES * NVS], i32, kind="ExternalInput")
    out_t = nc.dram_tensor("out", [NSVD, HD], f32, kind="ExternalOutput")

    FTELER = nc.dram_tensor("FTELER", [NP_ + 1, ROWW], bf16, kind="Internal")
    SSh = nc.dram_tensor("SSh", [NSVS, 4], bf16, kind="Internal")
    SS = nc.dram_tensor("SS", [NCORES * NSVS, 4], bf16, kind="Internal",
                        addr_space="Shared")
    f8 = mybir.dt.float8e4
    CURSH = nc.dram_tensor("CURSH", [NSVD, HD], f8, kind="Internal")
    CURG = [nc.dram_tensor(f"CURG{k}", [NCORES * NSVD, HD], f8,
                           kind="Internal", addr_space="Shared")
            for k in range(K - 1)]
    HSTK = [nc.dram_tensor(f"HSTK{k}", [NSVD, HD], f32, kind="Internal")
            for k in range(K)]
    HQD = nc.dram_tensor("HQD", [NSVD, HD], f32, kind="Internal")
    rg = [list(range(NCORES))]

    with tile.TileContext(nc) as tc:
        with tc.tile_pool(name="persist", bufs=1) as pp, \
             tc.tile_pool(name="work", bufs=1) as wp, \
             tc.tile_pool(name="gat", bufs=2) as gp, \
             tc.tile_pool(name="ps", bufs=2, space="PSUM") as psp:
            # ---- weights / constants / persistent index tiles ----
            wwa = pp.tile([P, 54], bf16)
            nc.sync.dma_start(wwa[:], W_in.ap())
            hlr = pp.tile([P, 2 * HD], f32)
            nc.sync.dma_start(hlr[:], hop_lr.ap())
            sc4 = pp.tile([P, (K + 1) * HD], f32)
            nc.sync.dma_start(sc4[:], scales4.ap())
            op4 = pp.tile([P, (K + 1) * HD], f32)
            nc.sync.dma_start(op4[:], offpos4.ap())
            epst = pp.tile([P, 1], f32)
            nc.vector.memset(epst[:], EPS)
            iod = pp.tile([P, NTED], i32)
            nc.sync.dma_start(iod[:], iOTHd.ap())
            ios = pp.tile([P, NTES], i32)
            nc.sync.dma_start(ios[:], iOTHs.ap())
            icu = pp.tile([P, NTED], i32)
            nc.sync.dma_start(icu[:], iCUR.ap())
            iqd = pp.tile([P, NVD], i32)
            nc.sync.dma_start(iqd[:], iQd.ap())
            iqs = pp.tile([P, NVS], i32)
            nc.sync.dma_start(iqs[:], iQs.ap())
            isc = pp.tile([P, NCORES * NVS], i32)
            nc.sync.dma_start(isc[:], iSCAT.ap())

            # ---- P0: replicated feat matmul -> FTELER rows ----
            GRP = 1024
            NSTG = 4
            stgs = []
            for b in range(NSTG):
                st = wp.tile([P, 8 * 57], bf16, tag=f"p0st{b}")
                nc.vector.memset(st[:], 0.0)
                stgs.append(st)
            for g in range(NP_ // GRP):
                fch = wp.tile([P, GRP], bf16, tag=f"fch{g % NSTG}")
                nc.sync.dma_start(fch[:], featT.ap()[:, g * GRP:(g + 1) * GRP])
                ps = psp.tile([P, 8 * 54], f32, tag=f"p0ps{g % 2}")
                for t in range(8):
                    nc.tensor.matmul(
                        out=ps[:, t * 54:(t + 1) * 54],
                        lhsT=fch[:, t * P:(t + 1) * P],
                        rhs=wwa[:], start=True, stop=True)
                sv3 = stgs[g % NSTG][:].rearrange("q (t e) -> q t e", e=57)
                pv3 = ps[:].rearrange("q (t e) -> q t e", e=54)
                # cols 48:51 (rss) are left stale; the scatter overwrites
                # them before any read
                nc.vector.tensor_copy(sv3[:, :, 0:HD], pv3[:, :, 0:HD])
                nc.vector.tensor_copy(sv3[:, :, 51:57], pv3[:, :, HD:54])
                nc.scalar.dma_start(
                    FTELER.ap()[g * GRP:(g + 1) * GRP, 0:57].rearrange(
                        "(t p) e -> p t e", t=8),
                    sv3)
            padr = wp.tile([1, 57], bf16, tag="padr")
            nc.vector.memset(padr[:], 0.0)
            nc.vector.memset(padr[:, 51:57], PADV)
            nc.sync.dma_start(FTELER.ap()[NP_:NP_ + 1, 0:57], padr[:])
            tc.strict_bb_all_engine_barrier()

            # ---- per-node key-side score values ----
            erp = pp.tile([P, NVD * H], bf16)
            nc.gpsimd.indirect_dma_start(
                out=erp[:],
                out_offset=None, in_=FTELER.ap(),
                in_offset=IOA(ap=iqd[:], axis=0), element_offset=54)
            elp = pp.tile([P, NVS * H], bf16)
            nc.gpsimd.indirect_dma_start(
                out=elp[:],
                out_offset=None, in_=FTELER.ap(),
                in_offset=IOA(ap=iqs[:], axis=0), element_offset=51)

            # ---- feat_trans (d-major layout: 48 = (16 d) x (3 h)) ----
            def feat_trans(dst_ap, src_ap, k, nv):
                # dst/src: [P, nv*HD] f32 views
                sv_ = src_ap.rearrange("p (a d h) -> p a h d", h=H, d=D)
                dv = dst_ap.rearrange("p (a d h) -> p a h d", h=H, d=D)
                m = wp.tile([P, nv * H], f32, tag="ftm")
                ms = wp.tile([P, nv * H], f32, tag="ftms")
                mv = m[:].rearrange("p (a h) -> p a h", h=H)
                nc.vector.tensor_reduce(out=mv, in_=sv_, axis=AX.X, op=AT.add)
                nc.vector.tensor_scalar_mul(m[:], m[:], 1.0 / D)
                nc.scalar.activation(dst_ap, src_ap, AF.Square)
                nc.vector.tensor_reduce(
                    out=ms[:].rearrange("p (a h) -> p a h", h=H),
                    in_=dv, axis=AX.X, op=AT.add)
                nc.vector.tensor_scalar_mul(ms[:], ms[:], 1.0 / D)
                mm = wp.tile([P, nv * H], f32, tag="ftmm")
                nc.vector.tensor_tensor(out=mm[:], in0=m[:], in1=m[:],
                                        op=AT.mult)
                nc.vector.tensor_tensor(out=ms[:], in0=ms[:], in1=mm[:],
                                        op=AT.subtract)
                nc.scalar.activation(ms[:], ms[:], AF.Sqrt, bias=epst[:])
                nc.vector.reciprocal(ms[:], ms[:])
                mb = mv[:, :, :, None].to_broadcast([P, nv, H, D])
                rb = ms[:].rearrange("p (a h) -> p a h", h=H)[:, :, :, None] \
                    .to_broadcast([P, nv, H, D])
                nc.vector.tensor_tensor(out=dv, in0=sv_, in1=mb, op=AT.subtract)
                nc.vector.tensor_tensor(out=dv, in0=dv, in1=rb, op=AT.mult)
                dv2 = dst_ap.rearrange("p (a e) -> p a e", e=HD)
                nc.vector.tensor_tensor(
                    out=dv2, in0=dv2,
                    in1=sc4[:, k * HD:(k + 1) * HD][:, None, :]
                    .to_broadcast([P, nv, HD]), op=AT.mult)
                nc.vector.tensor_tensor(
                    out=dv2, in0=dv2,
                    in1=op4[:, k * HD:(k + 1) * HD][:, None, :]
                    .to_broadcast([P, nv, HD]), op=AT.add)

            # ---- h_query -> HQD ----
            CURVA = pp.tile([P, NVD * HD], f32)
            CURVB = pp.tile([P, (NVD - B0) * HD], f32)
            t0 = wp.tile([P, NVD * HD], f32, tag="t0")
            if STAGE >= 5:
                gq = gp.tile([P, TARGET * 51], bf16, tag="gh")
                nc.gpsimd.indirect_dma_start(
                    out=gq[:, :NVD * HD],
                    out_offset=None, in_=FTELER.ap(),
                    in_offset=IOA(ap=iqd[:], axis=0))
                nc.vector.tensor_copy(CURVA[:], gq[:, :NVD * HD])
                feat_trans(t0[:], CURVA[:], 0, NVD)
                nc.sync.dma_start(
                    HQD.ap().rearrange("(p a) e -> p a e", p=P),
                    t0[:].rearrange("p (a e) -> p a e", e=HD))

            # ---- score passes ----
            EXP3 = pp.tile([P, NTED * H], bf16)
            SDt = pp.tile([P, NVD * H], f32)
            SSt = pp.tile([P, NVS * H], f32)

            def score_pass(plan, lk, ioth_t, keyp, eoff, sv, exp_keep):
                for (j0, nj, C) in plan:
                    eg = gp.tile([P, TARGET * H], bf16, tag="eg")
                    nc.gpsimd.indirect_dma_start(
                        out=eg[:, :nj * H],
                        out_offset=None, in_=FTELER.ap(),
                        in_offset=IOA(ap=ioth_t[:, j0:j0 + nj], axis=0),
                        element_offset=eoff)
                    ggg = nj // C
                    vb = lk[(j0, C)]
                    if exp_keep is not None:
                        et = exp_keep[:, j0 * H:(j0 + nj) * H]
                    else:
                        ett = gp.tile([P, TARGET * H], bf16, tag="et")
                        et = ett[:, :nj * H]
                    # e = el[oth] + er[key]  (key side broadcast over class)
                    nc.vector.tensor_tensor(
                        out=et.rearrange("p (g c e) -> p g c e", c=C, e=H),
                        in0=eg[:, :nj * H].rearrange(
                            "p (g c e) -> p g c e", c=C, e=H),
                        in1=keyp[:, vb * H:(vb + ggg) * H].rearrange(
                            "p (g e) -> p g e", e=H)[:, :, None, :]
                        .to_broadcast([P, ggg, C, H]),
                        op=AT.add)
                    lrt = gp.tile([P, TARGET * H], bf16, tag="lrt")
                    nc.vector.tensor_scalar_min(lrt[:, :nj * H], et, 0.0)
                    nc.vector.tensor_scalar_max(et, et, 0.0)
                    nc.vector.tensor_scalar_mul(lrt[:, :nj * H],
                                                lrt[:, :nj * H], NEG)
                    nc.vector.tensor_tensor(out=et, in0=et,
                                            in1=lrt[:, :nj * H], op=AT.add)
                    nc.scalar.activation(et, et, AF.Exp)
                    nc.vector.tensor_reduce(
                        out=sv[:, vb * H:(vb + ggg) * H].rearrange(
                            "p (g e) -> p g e", e=H),
                        in_=et.rearrange("p (g c e) -> p g e c", c=C, e=H),
                        axis=AX.X, op=AT.add)

            nc.vector.memset(SDt[:], 0.0)
            if STAGE >= 2:
                score_pass(pland, lk_d, iod, erp, 51, SDt, EXP3)
            nc.vector.memset(SSt[:], 0.0)
            if STAGE >= 3:
                score_pass(plans, lk_s, ios, elp, 54, SSt, None)

            # rsd (local, stays in SBUF); rss -> bf16 -> AllGather -> scatter
            rsd = pp.tile([P, NVD * H], f32)
            nc.vector.tensor_scalar_max(rsd[:], SDt[:], 1e-30)
            nc.vector.reciprocal(rsd[:], rsd[:])
            nc.scalar.activation(rsd[:], rsd[:], AF.Sqrt)

            rssf = wp.tile([P, NVS * H], f32, tag="rssf")
            nc.vector.tensor_scalar_max(rssf[:], SSt[:], 1e-30)
            nc.vector.reciprocal(rssf[:], rssf[:])
            nc.scalar.activation(rssf[:], rssf[:], AF.Sqrt)
            # zero out entries whose sum was exactly 0 (pad / no-out-edge)
            ind = wp.tile([P, NVS * H], f32, tag="ind")
            nc.vector.tensor_scalar_mul(ind[:], SSt[:], 1e30)
            nc.vector.tensor_scalar_min(ind[:], ind[:], 1.0)
            nc.vector.tensor_tensor(out=rssf[:], in0=rssf[:], in1=ind[:],
                                    op=AT.mult)
            ssb = wp.tile([P, NVS * 4], bf16, tag="ssb")
            nc.vector.memset(ssb[:], 0.0)
            nc.vector.tensor_copy(
                ssb[:].rearrange("p (a e) -> p a e", e=4)[:, :, 0:H],
                rssf[:].rearrange("p (a e) -> p a e", e=H))
            nc.sync.dma_start(
                SSh.ap().rearrange("(p a) e -> p a e", p=P),
                ssb[:].rearrange("p (a e) -> p a e", e=4))
            if STAGE >= 3:
                if USE_COLL:
                    nc.gpsimd.collective_compute(
                        "AllGather", AT.bypass, ins=[SSh.ap()], outs=[SS.ap()],
                        replica_groups=rg)
                else:
                    nc.sync.dma_start(SS.ap()[0:NSVS, :], SSh.ap())
            tc.strict_bb_all_engine_barrier()
            # load gathered rss, scatter into FTELER cols 48:51
            ssl = wp.tile([P, NCORES * NVS * 4], bf16, tag="ssl")
            nc.sync.dma_start(
                ssl[:].rearrange("p (c a e) -> p c a e", c=NCORES, e=4),
                SS.ap().rearrange("(c p a) e -> p c a e", c=NCORES, p=P))
            if STAGE >= 4:
                nc.gpsimd.indirect_dma_start(
                    out=FTELER.ap(),
                    out_offset=IOA(ap=isc[:], axis=0),
                    in_=ssl[:].rearrange("p (a e) -> p a e", e=4)[:, :, 0:H],
                    in_offset=None, element_offset=48)
            tc.strict_bb_all_engine_barrier()

            # ---- tree segment-sum helper ----
            def tree_sum(gv, ggg, C, sstr, vb):
                # gv: [P, nj*sstr] view base (bf16), slots of width sstr,
                # msg in els 0:HD. Reduce C slots per group into slot 0.
                cc = C
                while cc > 1:
                    half = cc // 2
                    if cc % 2 == 1:
                        nc.vector.tensor_tensor(
                            out=gv.rearrange("p (g c e) -> p g c e",
                                             c=C, e=sstr)[:, :, 0:1, 0:HD],
                            in0=gv.rearrange("p (g c e) -> p g c e",
                                             c=C, e=sstr)[:, :, 0:1, 0:HD],
                            in1=gv.rearrange("p (g c e) -> p g c e",
                                             c=C, e=sstr)[:, :, cc - 1:cc, 0:HD],
                            op=AT.add)
                    v4 = gv.rearrange("p (g c e) -> p g c e", c=C, e=sstr)
                    nc.vector.tensor_tensor(
                        out=v4[:, :, 0:half, 0:HD],
                        in0=v4[:, :, 0:half, 0:HD],
                        in1=v4[:, :, half:2 * half, 0:HD],
                        op=AT.add)
                    cc = half
                dst_t = (CURVA[:, vb * HD:(vb + ggg) * HD] if vb < B0
                         else CURVB[:, (vb - B0) * HD:(vb - B0 + ggg) * HD])
                nc.vector.tensor_copy(
                    dst_t.rearrange("p (g e) -> p g e", e=HD),
                    gv.rearrange("p (g c e) -> p g c e",
                                 c=C, e=sstr)[:, :, 0, 0:HD])

            # ---- hops ----
            # class chunks overwrite CURVA/CURVB groups [0, VGE); only the
            # zeros/spare tail needs zeroing, once
            nc.vector.memset(CURVB[:, (VGE - B0) * HD:], 0.0)
            KTOP = 0 if STAGE < 6 else (STAGE - 5 if STAGE < 9 else K)

            def send_seg(s0, s1, kk):
                curb = gp.tile([P, TARGET * HD], f8, tag="g8")
                src_t = (CURVA[:, s0 * HD:s1 * HD] if s1 <= B0
                         else CURVB[:, (s0 - B0) * HD:(s1 - B0) * HD])
                nc.scalar.activation(curb[:, :(s1 - s0) * HD],
                                     src_t, AF.Copy)
                nc.sync.dma_start(
                    CURSH.ap().rearrange("(p a) e -> p a e", p=P)[:, s0:s1, :],
                    curb[:, :(s1 - s0) * HD].rearrange(
                        "p (a e) -> p a e", e=HD))
                if USE_COLL:
                    nc.gpsimd.collective_compute(
                        "AllGather", AT.bypass,
                        ins=[CURSH.ap().rearrange(
                            "(p a) e -> p a e", p=P)[:, s0:s1, :]],
                        outs=[CURG[kk - 1].ap().rearrange(
                            "(c p a) e -> c p a e",
                            c=NCORES, p=P)[:, :, s0:s1, :]],
                        replica_groups=rg)
                else:
                    nc.sync.dma_start(
                        CURG[kk - 1].ap().rearrange(
                            "(c p a) e -> c p a e",
                            c=NCORES, p=P)[0, :, s0:s1, :],
                        CURSH.ap().rearrange(
                            "(p a) e -> p a e", p=P)[:, s0:s1, :])

            for k in range(1, KTOP + 1):
                tc.strict_bb_all_engine_barrier()
                nseg_sent = 0
                for (j0, nj, C) in pland:
                    ggg = nj // C
                    vb = lk_d[(j0, C)]
                    g = gp.tile([P, TARGET * 51], bf16, tag="gh")
                    if k == 1:
                        sstr = 51
                        nc.gpsimd.indirect_dma_start(
                            out=g[:, :nj * 51],
                            out_offset=None, in_=FTELER.ap(),
                            in_offset=IOA(ap=iod[:, j0:j0 + nj], axis=0))
                        gv = g[:, :nj * 51]
                        # A = exp * rsd(bcast) * rss(gathered); keep in EXP3
                        ev = EXP3[:, j0 * H:(j0 + nj) * H]
                        nc.vector.tensor_tensor(
                            out=ev.rearrange("p (g c e) -> p g c e",
                                             c=C, e=H),
                            in0=ev.rearrange("p (g c e) -> p g c e",
                                             c=C, e=H),
                            in1=rsd[:, vb * H:(vb + ggg) * H].rearrange(
                                "p (g e) -> p g e", e=H)[:, :, None, :]
                            .to_broadcast([P, ggg, C, H]),
                            op=AT.mult)
                        nc.vector.tensor_tensor(
                            out=ev.rearrange("p (j e) -> p j e", e=H),
                            in0=ev.rearrange("p (j e) -> p j e", e=H),
                            in1=gv.rearrange("p (j e) -> p j e",
                                             e=51)[:, :, 48:51],
                            op=AT.mult)
                    else:
                        sstr = HD
                        g8 = gp.tile([P, TARGET * HD], f8, tag="g8")
                        nc.gpsimd.indirect_dma_start(
                            out=g8[:, :nj * HD],
                            out_offset=None, in_=CURG[k - 2].ap(),
                            in_offset=IOA(ap=icu[:, j0:j0 + nj], axis=0))
                        nc.scalar.activation(g[:, :nj * HD], g8[:, :nj * HD],
                                             AF.Copy)
                        gv = g[:, :nj * HD]
                    # msg *= A broadcast over d (packed-last 2-byte).
                    # For k==1 the 51-el slot factors as 17x3; els 48:51
                    # (rss) get multiplied too but are dead afterwards.
                    nc.vector.tensor_tensor(
                        out=gv.rearrange("p (j d h) -> p j d h",
                                         d=sstr // H, h=H),
                        in0=gv.rearrange("p (j d h) -> p j d h",
                                         d=sstr // H, h=H),
                        in1=EXP3[:, j0 * H:(j0 + nj) * H].rearrange(
                            "p (j e) -> p j e", e=H)[:, :, None, :]
                        .to_broadcast([P, nj, sstr // H, H]),
                        op=AT.mult)
                    tree_sum(gv, ggg, C, sstr, vb)
                    if k < K and nseg_sent == 0 and vb + ggg >= B0:
                        send_seg(0, B0, k)
                        nseg_sent = 1
                if k < K:
                    if nseg_sent == 0:
                        send_seg(0, B0, k)
                    send_seg(B0, NVD, k)
                feat_trans(t0[:, :B0 * HD], CURVA[:, :B0 * HD], k, B0)
                feat_trans(t0[:, B0 * HD:], CURVB[:], k, NVD - B0)
                nc.sync.dma_start(
                    HSTK[k - 1].ap().rearrange("(p a) e -> p a e", p=P),
                    t0[:].rearrange("p (a e) -> p a e", e=HD))

            # ---- final hop attention ----
            if STAGE < 9:
                dum = wp.tile([P, NVD * HD], f32, tag="t0")
                nc.vector.memset(dum[:], 1.0)
                nc.sync.dma_start(
                    out_t.ap().rearrange("(p a) e -> p a e", p=P),
                    dum[:].rearrange("p (a e) -> p a e", e=HD))
            else:
                tc.strict_bb_all_engine_barrier()
                lq = wp.tile([P, NVD * H], f32, tag="lq")
                hlv = hlr[:, 0:HD].rearrange(
                    "p (d h) -> p h d", h=H)[:, None, :, :] \
                    .to_broadcast([P, NVD, H, D])
                hrv = hlr[:, HD:2 * HD].rearrange(
                    "p (d h) -> p h d", h=H)[:, None, :, :] \
                    .to_broadcast([P, NVD, H, D])
                wk = wp.tile([P, NVD * HD], f32, tag="wk")
                nc.sync.dma_start(
                    wk[:].rearrange("p (a e) -> p a e", e=HD),
                    HQD.ap().rearrange("(p a) e -> p a e", p=P))
                nc.vector.tensor_tensor(
                    out=t0[:].rearrange("p (a d h) -> p a h d", h=H, d=D),
                    in0=wk[:].rearrange("p (a d h) -> p a h d", h=H, d=D),
                    in1=hlv, op=AT.mult)
                nc.vector.tensor_reduce(
                    out=lq[:].rearrange("p (a h) -> p a h", h=H),
                    in_=t0[:].rearrange("p (a d h) -> p a h d", h=H, d=D),
                    axis=AX.X, op=AT.add)
                # single pass: acc = sum_k wk*exp(lg_k); den = sum_k exp(lg_k)
                # (divide once at the end)
                ek = wp.tile([P, NVD * H], f32, tag="ek")
                ekn = wp.tile([P, NVD * H], f32, tag="ekn")
                den = wp.tile([P, NVD * H], f32, tag="den")
                acc = CURVA
                nc.vector.memset(acc[:], 0.0)
                nc.vector.memset(den[:], 0.0)
                for k in range(K):
                    nc.sync.dma_start(
                        wk[:].rearrange("p (a e) -> p a e", e=HD),
                        HSTK[k].ap().rearrange("(p a) e -> p a e", p=P))
                    nc.vector.tensor_tensor(
                        out=t0[:].rearrange("p (a d h) -> p a h d", h=H, d=D),
                        in0=wk[:].rearrange("p (a d h) -> p a h d", h=H, d=D),
                        in1=hrv, op=AT.mult)
                    nc.vector.tensor_reduce(
                        out=ek[:].rearrange("p (a h) -> p a h", h=H),
                        in_=t0[:].rearrange("p (a d h) -> p a h d", h=H, d=D),
                        axis=AX.X, op=AT.add)
                    nc.vector.tensor_tensor(out=ek[:], in0=ek[:], in1=lq[:],
                                            op=AT.add)
                    nc.vector.tensor_scalar_min(ekn[:], ek[:], 0.0)
                    nc.vector.tensor_scalar_max(ek[:], ek[:], 0.0)
                    nc.vector.tensor_scalar_mul(ekn[:], ekn[:], NEG)
                    nc.vector.tensor_tensor(out=ek[:], in0=ek[:], in1=ekn[:],
                                            op=AT.add)
                    nc.scalar.activation(ek[:], ek[:], AF.Exp)
                    nc.vector.tensor_tensor(out=den[:], in0=den[:], in1=ek[:],
                                            op=AT.add)
                    nc.vector.tensor_tensor(
                        out=t0[:].rearrange("p (a d h) -> p a d h", h=H, d=D),
                        in0=wk[:].rearrange("p (a d h) -> p a d h", h=H, d=D),
                        in1=ek[:].rearrange("p (a h) -> p a h",
                                            h=H)[:, :, None, :]
                        .to_broadcast([P, NVD, D, H]), op=AT.mult)
                    nc.vector.tensor_tensor(out=acc[:], in0=acc[:], in1=t0[:],
                                            op=AT.add)
                nc.vector.reciprocal(den[:], den[:])
                nc.vector.tensor_tensor(
                    out=acc[:].rearrange("p (a d h) -> p a d h", h=H, d=D),
                    in0=acc[:].rearrange("p (a d h) -> p a d h", h=H, d=D),
                    in1=den[:].rearrange("p (a h) -> p a h",
                                         h=H)[:, :, None, :]
                    .to_broadcast([P, NVD, D, H]), op=AT.mult)
                nc.sync.dma_start(
                    out_t.ap().rearrange("(p a) e -> p a e", p=P),
                    acc[:].rearrange("p (a e) -> p a e", e=HD))
    nc.compile()
    return nc


# ---------------------------------------------------------------- entry
def kernel(**inputs):
    feat = np.asarray(inputs["feat"], np.float32)
    src = np.asarray(inputs["src"]).astype(np.int64)
    dst = np.asarray(inputs["dst"]).astype(np.int64)
    fc_W = np.asarray(inputs["fc_W"], np.float32)
    attn_l = np.asarray(inputs["attn_l"], np.float32).reshape(H, D)
    attn_r = np.asarray(inputs["attn_r"], np.float32).reshape(H, D)
    hop_l = np.asarray(inputs["hop_attn_l"], np.float32).reshape(H, D)
    hop_r = np.asarray(inputs["hop_attn_r"], np.float32).reshape(H, D)
    pos = np.asarray(inputs["pos_emb"], np.float32)
    nsc = np.asarray(inputs["norm_scales"], np.float32)
    off = np.asarray(inputs["offsets"], np.float32)
    bias = np.asarray(inputs["bias"], np.float32).reshape(1, H, D)

    sides = host_prep(src, dst)
    sd, ssd = sides["d"], sides["s"]
    nc = build_nc(sd["NV"], sd["NTE"], sd["plan"],
                  ssd["NV"], ssd["NTE"], ssd["plan"])
    NVD, NVS = sd["NV"], ssd["NV"]

    # d-major permutation: column d*H + h holds (head h, dim d)
    perm = np.arange(HD).reshape(H, D).T.reshape(-1)  # (d-major) <- (h-major)
    W48 = fc_W.reshape(IN, H, D).transpose(0, 2, 1).reshape(IN, HD)
    Wl = np.einsum("ihd,hd->ih", fc_W.reshape(IN, H, D), attn_l)
    Wr = np.einsum("ihd,hd->ih", fc_W.reshape(IN, H, D), attn_r)
    W54 = np.concatenate([W48, Wl, Wr], 1).astype(ml_dtypes.bfloat16)

    featP = np.zeros((NP_, IN), np.float32)
    featP[:N] = feat
    featT = np.ascontiguousarray(featP.T).astype(ml_dtypes.bfloat16)
    sc = nsc.reshape(K + 1, HD)[:, perm]
    opv = (off.reshape(K + 1, HD) +
           pos[0].transpose(1, 0, 2).reshape(K + 1, HD))[:, perm]
    scales4 = np.tile(sc.reshape(1, -1), (P, 1)).astype(np.float32)
    offpos4 = np.tile(opv.reshape(1, -1), (P, 1)).astype(np.float32)
    hop2 = np.tile(np.concatenate(
        [hop_l.T.reshape(1, HD), hop_r.T.reshape(1, HD)], 1), (P, 1))

    in_maps = []
    for c in range(NCORES):
        cd, cs = sd["cores"][c], ssd["cores"][c]
        iq_d = np.zeros(P * NVD, np.int64)
        iq_d[cd["vrow"]] = np.arange(NS) + c * NS
        iq_s = np.zeros(P * NVS, np.int64)
        iq_s[cs["vrow"]] = np.arange(NS) + c * NS
        # scatter targets: SBUF slot (p, c2*NVS + a) holds SS row
        # c2*NSVS + p*NVS + a  -> node owned by c2 at that vrow (or pad row)
        iscat = np.full((P, NCORES * NVS), NP_, np.int64)
        for c2 in range(NCORES):
            vr = ssd["cores"][c2]["vrow"]   # node-local -> vrow
            pmat = vr // NVS
            amat = vr % NVS
            iscat[pmat, c2 * NVS + amat] = np.arange(NS) + c2 * NS
        in_maps.append(dict(
            featT=featT, W_in=W54,
            hop_lr=hop2.astype(np.float32),
            scales4=scales4, offpos4=offpos4,
            iOTHd=cd["ioth"], iOTHs=cs["ioth"],
            iCUR=sd["vmap"][cd["ioth"]].astype(np.int32),
            iQd=iq_d.reshape(P, NVD).astype(np.int32),
            iQs=iq_s.reshape(P, NVS).astype(np.int32),
            iSCAT=iscat.astype(np.int32),
        ))

    outs = None
    exec_ns = None
    if os.environ.get("AGDN_HW", "") == "1":
        try:
            from concourse import bass_utils
            trace = os.environ.get("AGDN_TRACE", "") == "1"
            res = bass_utils.run_bass_kernel_spmd(
                nc, in_maps, core_ids=list(range(NCORES)), trace=trace)
            outs = [np.asarray(res.results[c]["out"]) for c in range(NCORES)]
            exec_ns = res.exec_time_ns
            if not all(np.isfinite(o).all() for o in outs):
                print("[kernel] HW returned non-finite values", flush=True)
                outs = None
        except Exception as e:
            print(f"[kernel] HW path failed: {type(e).__name__}", flush=True)
            outs = None
    if outs is None:
        from concourse.bass_interp import MultiCoreSim
        nworkers = 1 if os.environ.get("AGDN_TRACE", "") == "1" else NCORES
        sim = MultiCoreSim(nc, num_cores=NCORES, num_workers=nworkers,
                           trace=False, require_finite=False,
                           require_nnan=False)
        for c, core in enumerate(sim.cores.values()):
            for kk, vv in in_maps[c].items():
                core.tensor(kk)[:] = vv
        sim.simulate(check_with_hw=False)
        outs = [np.array(core.tensor("out")) for core in sim.cores.values()]
        if nworkers == 1:
            exec_ns = int(sim.global_time)
    if exec_ns:
        print(f"[kernel] exec_time_ns={exec_ns}", flush=True)
        try:
            with open("/tmp/agdn_exec_ns.txt", "w") as f:
                f.write(str(exec_ns))
        except OSError:
            pass
    full = np.concatenate(outs, 0)
    out = full[sd["vmap"][:N]].reshape(N, D, H).transpose(0, 2, 1)
    return np.ascontiguousarray(out + bias).astype(np.float32)


# revision 28
# speedup vs baseline: 1.0998x; 1.0114x over previous
"""AGDNConv (3-hop attention diffusion GNN) on 8 trn2 NeuronCores.

Sharding: edges partitioned by dst-owner (owner = dst // 12544); node tables
replicated (P0 matmul) or AllGathered per hop. Per-core segment sums use a
degree-class slot layout so they become tree/strided tensor_reduce ops.
Attention softmax uses the max-free identity
  a[e] = exp(e_e) * rsqrt(s_dst[dst_e]) * rsqrt(s_src[src_e]).

v2 layout decisions (all driven by the DMA descriptor cost model):
 - One bf16 node table FTELER [NP_+1, 58]: cols 0:48 ft in d-major (d,h)
   order, 48:51 rss (scattered in after the src-side AllGather), 51:54 el,
   54:57 er.  Hop-1 gathers cols 0:51 in ONE 102B descriptor per edge
   (ft + rss together); score passes gather only the 6B other-side score.
 - Key-side score values are broadcast per node group (no per-edge gather);
   per-node sums (SD) stay in SBUF, so the old "stage C" disappears.
 - No mask tensors: the pad row NP_ has el=er=-60 so exp==0 naturally, and
   rss==0 on the pad row kills pad contributions in hops 2/3.
 - d-major feature order makes every big DVE multiply packed-last (2x/4x
   DVE modes); segment sums are tree adds (packed) instead of strided 1x.

Execution: the axon terminal's NRT shim does not implement dynamic-AP
(indirect) DMA - it returns garbage data for gathers and wedges the device
on larger elements (verified empirically; see dev notes). kernel() therefore
runs the cycle-accurate MultiCoreSim (single worker, real collectives) by
default, which validates and times the exact Bass program. Set AGDN_HW=1 to
attempt real-HW execution via PJRT on terminals with a fixed runtime.
"""
import sys
sys.path.insert(0, "/opt/trn_rl_repo")
import os
import numpy as np
import ml_dtypes

USE_COLL = os.environ.get("AGDN_NOCOLL", "") != "1"
STAGE = int(os.environ.get("AGDN_STAGE", "9"))
P = 128
N = 100000
IN = 128
H = 3
D = 16
HD = 48
K = 3
NEG = 0.2
EPS = 1e-9
NCORES = 8
NS = 12544
NP_ = NCORES * NS
ROWW = 256         # FTELER row width (bf16 elements; 512B rows avoid the
                   # small-descriptor 2x latency multiplier on gathers)
PADV = -60.0       # pad-row el/er value: exp(leaky(-60+x)) ~ 0
CLASSES = [4, 8, 12, 16, 20, 24, 32, 40, 48, 64, 96, 128]
TARGET = 176       # chunk width target, in slots


# ---------------------------------------------------------------- host prep
def _pack_side(key_node, other_node, n_lo):
    loc = (key_node - n_lo).astype(np.int64)
    order = np.argsort(loc, kind="stable")
    loc_s = loc[order]
    other_s = other_node[order]
    deg = np.bincount(loc_s, minlength=NS)
    assert deg.max() <= CLASSES[-1], f"degree {deg.max()} exceeds max class"
    starts = np.concatenate([[0], np.cumsum(deg)[:-1]])
    cls_of = np.full(NS, -1, np.int64)
    lo = 0
    for ci, C in enumerate(CLASSES):
        cls_of[(deg > lo) & (deg <= C)] = ci
        lo = C
    members = [np.where(cls_of == ci)[0] for ci in range(len(CLASSES))]
    zeros = np.where(deg == 0)[0]
    return dict(members=members, zeros=zeros, deg=deg, starts=starts,
                other_s=other_s)


def _layout(counts_max, gz_max):
    G, vg0, je0, plan = [], [], [], []
    v, j = 0, 0
    for ci, C in enumerate(CLASSES):
        g = int(np.ceil(counts_max[ci] / P))
        G.append(g)
        vg0.append(v)
        je0.append(j)
        cols = g * C
        step = max(C, (TARGET // C) * C)
        s = 0
        while s < cols:
            w = min(step, cols - s)
            plan.append((j + s, w, C))
            s += w
        v += g
        j += g * C
    vg0.append(v)
    # zeros region + one spare always-unassigned group (its CUR rows stay 0)
    v += int(np.ceil(gz_max / P)) + 1
    return G, vg0, je0, max(v, 1), max(((j + 3) // 4) * 4, 4), plan


def _fill_core(pack, G, vg0, je0, NV, NTE, n_lo):
    ioth = np.full((P, NTE), NP_, np.int32)    # pad -> FTELER pad row
    vrow = np.full(NS, -1, np.int64)
    deg, starts, other_s = pack["deg"], pack["starts"], pack["other_s"]
    for ci, C in enumerate(CLASSES):
        mem = pack["members"][ci]
        g_c = max(G[ci], 1)
        for i, nl in enumerate(mem):
            g, p = i % g_c, i // g_c
            vrow[nl] = p * NV + vg0[ci] + g
            d, s0 = deg[nl], starts[nl]
            je = je0[ci] + g * C
            ioth[p, je:je + d] = other_s[s0:s0 + d]
    mem = pack["zeros"]
    gz = max(int(np.ceil(len(mem) / P)), 1)
    for i, nl in enumerate(mem):
        vrow[nl] = (i // gz) * NV + vg0[len(CLASSES)] + i % gz
    return ioth, vrow


def host_prep(src, dst):
    sides = {}
    for side, key, oth in (("d", dst, src), ("s", src, dst)):
        packs = []
        for c in range(NCORES):
            m = (key >= c * NS) & (key < (c + 1) * NS)
            packs.append(_pack_side(key[m], oth[m], c * NS))
        counts_max = np.max(
            np.array([[len(p) for p in pk["members"]] for pk in packs]), axis=0)
        gz_max = max(len(pk["zeros"]) for pk in packs)
        G, vg0, je0, NV, NTE, plan = _layout(counts_max, gz_max)
        cores, vmap = [], np.zeros(NP_ + 1, np.int64)
        for c in range(NCORES):
            ioth, vrow = _fill_core(packs[c], G, vg0, je0, NV, NTE, c * NS)
            cores.append(dict(ioth=ioth, vrow=vrow))
            vmap[c * NS:(c + 1) * NS] = c * (P * NV) + vrow
        # pad/ghost entries -> this side's guaranteed-zero row of core 0
        # (the spare group; never assigned by _fill_core)
        zrow = [c * (P * NV) + 127 * NV + (NV - 1) for c in range(NCORES)]
        vmap[vmap < 0] = zrow[0]
        sides[side] = dict(NV=NV, NTE=NTE, plan=plan, cores=cores, vmap=vmap,
                           zrow=zrow)
    return sides


def _vg_lookup(plan):
    lk, vg, last_C, cj, cvg = {}, 0, None, None, None
    for (j0, nj, C) in plan:
        if C != last_C:
            cj, cvg, last_C = j0, vg, C
        lk[(j0, C)] = cvg + (j0 - cj) // C
        vg = cvg + (j0 - cj + nj) // C
    return lk


# ---------------------------------------------------------------- device
def build_nc(NVD, NTED, pland, NVS, NTES, plans):
    import concourse.bass as bass
    import concourse.bacc as bacc
    import concourse.mybir as mybir
    import concourse.tile as tile
    f32, bf16, i32 = mybir.dt.float32, mybir.dt.bfloat16, mybir.dt.int32
    AT, AF, AX = mybir.AluOpType, mybir.ActivationFunctionType, mybir.AxisListType
    IOA = bass.IndirectOffsetOnAxis
    NSVD, NSVS = P * NVD, P * NVS
    lk_d, lk_s = _vg_lookup(pland), _vg_lookup(plans)
    VGE = max(lk_d[(j0, C)] + nj // C for (j0, nj, C) in pland)
    B0 = VGE
    for (j0, nj, C) in pland:
        vbe = lk_d[(j0, C)] + nj // C
        if vbe >= int(VGE * 0.55):
            B0 = vbe
            break

    nc = bacc.Bacc("TRN2", target_bir_lowering=False, debug=False,
                   num_devices=NCORES)
    featT = nc.dram_tensor("featT", [P, NP_], bf16, kind="ExternalInput")
    W_in = nc.dram_tensor("W_in", [P, 54], bf16, kind="ExternalInput")
    hop_lr = nc.dram_tensor("hop_lr", [P, 2 * HD], f32, kind="ExternalInput")
    scales4 = nc.dram_tensor("scales4", [P, (K + 1) * HD], f32, kind="ExternalInput")
    offpos4 = nc.dram_tensor("offpos4", [P, (K + 1) * HD], f32, kind="ExternalInput")
    iOTHd = nc.dram_tensor("iOTHd", [P, NTED], i32, kind="ExternalInput")
    iOTHs = nc.dram_tensor("iOTHs", [P, NTES], i32, kind="ExternalInput")
    iCUR = nc.dram_tensor("iCUR", [P, NTED], i32, kind="ExternalInput")
    iQd = nc.dram_tensor("iQd", [P, NVD], i32, kind="ExternalInput")
    iQs = nc.dram_tensor("iQs", [P, NVS], i32, kind="ExternalInput")
    iSCAT = nc.dram_tensor("iSCAT", [P, NCORES * NVS], i32, kind="ExternalInput")
    out_t = nc.dram_tensor("out", [NSVD, HD], f32, kind="ExternalOutput")

    FTELER = nc.dram_tensor("FTELER", [NP_ + 1, ROWW], bf16, kind="Internal")
    SSh = nc.dram_tensor("SSh", [NSVS, 4], bf16, kind="Internal")
    SS = nc.dram_tensor("SS", [NCORES * NSVS, 4], bf16, kind="Internal",
                        addr_space="Shared")
    f8 = mybir.dt.float8e4
    CURSH = nc.dram_tensor("CURSH", [NSVD, HD], f8, kind="Internal")
    CURG = [nc.dram_tensor(f"CURG{k}", [NCORES * NSVD, HD], f8,
                           kind="Internal", addr_space="Shared")
            for k in range(K - 1)]
    HSTK = [nc.dram_tensor(f"HSTK{k}", [NSVD, HD], f32, kind="Internal")
            for k in range(K)]
    HQD = nc.dram_tensor("HQD", [NSVD, HD], f32, kind="Internal")
    rg = [list(range(NCORES))]

    with tile.TileContext(nc) as tc:
        with tc.tile_pool(name="persist", bufs=1) as pp, \
             tc.tile_pool(name="work", bufs=1) as wp, \
             tc.tile_pool(name="gat", bufs=2) as gp, \
             tc.tile_pool(name="ps", bufs=2, space="PSUM") as psp:
            # ---- weights / constants / persistent index tiles ----
            wwa = pp.tile([P, 54], bf16)
            nc.sync.dma_start(wwa[:], W_in.ap())
            hlr = pp.tile([P, 2 * HD], f32)
            nc.sync.dma_start(hlr[:], hop_lr.ap())
            sc4 = pp.tile([P, (K + 1) * HD], f32)
            nc.sync.dma_start(sc4[:], scales4.ap())
            op4 = pp.tile([P, (K + 1) * HD], f32)
            nc.sync.dma_start(op4[:], offpos4.ap())
            epst = pp.tile([P, 1], f32)
            nc.vector.memset(epst[:], EPS)
            iod = pp.tile([P, NTED], i32)
            nc.sync.dma_start(iod[:], iOTHd.ap())
            ios = pp.tile([P, NTES], i32)
            nc.sync.dma_start(ios[:], iOTHs.ap())
            icu = pp.tile([P, NTED], i32)
            nc.sync.dma_start(icu[:], iCUR.ap())
            iqd = pp.tile([P, NVD], i32)
            nc.sync.dma_start(iqd[:], iQd.ap())
            iqs = pp.tile([P, NVS], i32)
            nc.sync.dma_start(iqs[:], iQs.ap())
            isc = pp.tile([P, NCORES * NVS], i32)
            nc.sync.dma_start(isc[:], iSCAT.ap())

            # ---- P0: replicated feat matmul -> FTELER rows ----
            GRP = 1024
            NSTG = 4
            stgs = []
            for b in range(NSTG):
                st = wp.tile([P, 8 * 57], bf16, tag=f"p0st{b}")
                nc.vector.memset(st[:], 0.0)
                stgs.append(st)
            for g in range(NP_ // GRP):
                fch = wp.tile([P, GRP], bf16, tag=f"fch{g % NSTG}")
                nc.sync.dma_start(fch[:], featT.ap()[:, g * GRP:(g + 1) * GRP])
                ps = psp.tile([P, 8 * 54], f32, tag=f"p0ps{g % 2}")
                for t in range(8):
                    nc.tensor.matmul(
                        out=ps[:, t * 54:(t + 1) * 54],
                        lhsT=fch[:, t * P:(t + 1) * P],
                        rhs=wwa[:], start=True, stop=True)
                sv3 = stgs[g % NSTG][:].rearrange("q (t e) -> q t e", e=57)
                pv3 = ps[:].rearrange("q (t e) -> q t e", e=54)
                # cols 48:51 (rss) are left stale; the scatter overwrites
                # them before any read
                nc.vector.tensor_copy(sv3[:, :, 0:HD], pv3[:, :, 0:HD])
                nc.vector.tensor_copy(sv3[:, :, 51:57], pv3[:, :, HD:54])
                nc.scalar.dma_start(
                    FTELER.ap()[g * GRP:(g + 1) * GRP, 0:57].rearrange(
                        "(t p) e -> p t e", t=8),
                    sv3)
            padr = wp.tile([1, 57], bf16, tag="padr")
            nc.vector.memset(padr[:], 0.0)
            nc.vector.memset(padr[:, 51:57], PADV)
            nc.sync.dma_start(FTELER.ap()[NP_:NP_ + 1, 0:57], padr[:])
            tc.strict_bb_all_engine_barrier()

            # ---- per-node key-side score values ----
            erp = pp.tile([P, NVD * H], bf16)
            nc.gpsimd.indirect_dma_start(
                out=erp[:],
                out_offset=None, in_=FTELER.ap(),
                in_offset=IOA(ap=iqd[:], axis=0), element_offset=54)
            elp = pp.tile([P, NVS * H], bf16)
            nc.gpsimd.indirect_dma_start(
                out=elp[:],
                out_offset=None, in_=FTELER.ap(),
                in_offset=IOA(ap=iqs[:], axis=0), element_offset=51)

            # ---- feat_trans (d-major layout: 48 = (16 d) x (3 h)) ----
            def feat_trans(dst_ap, src_ap, k, nv):
                # dst/src: [P, nv*HD] f32 views
                sv_ = src_ap.rearrange("p (a d h) -> p a h d", h=H, d=D)
                dv = dst_ap.rearrange("p (a d h) -> p a h d", h=H, d=D)
                m = wp.tile([P, nv * H], f32, tag="ftm")
                ms = wp.tile([P, nv * H], f32, tag="ftms")
                mv = m[:].rearrange("p (a h) -> p a h", h=H)
                nc.vector.tensor_reduce(out=mv, in_=sv_, axis=AX.X, op=AT.add)
                nc.vector.tensor_scalar_mul(m[:], m[:], 1.0 / D)
                nc.scalar.activation(dst_ap, src_ap, AF.Square)
                nc.vector.tensor_reduce(
                    out=ms[:].rearrange("p (a h) -> p a h", h=H),
                    in_=dv, axis=AX.X, op=AT.add)
                nc.vector.tensor_scalar_mul(ms[:], ms[:], 1.0 / D)
                mm = wp.tile([P, nv * H], f32, tag="ftmm")
                nc.vector.tensor_tensor(out=mm[:], in0=m[:], in1=m[:],
                                        op=AT.mult)
                nc.vector.tensor_tensor(out=ms[:], in0=ms[:], in1=mm[:],
                                        op=AT.subtract)
                nc.scalar.activation(ms[:], ms[:], AF.Sqrt, bias=epst[:])
                nc.vector.reciprocal(ms[:], ms[:])
                mb = mv[:, :, :, None].to_broadcast([P, nv, H, D])
                rb = ms[:].rearrange("p (a h) -> p a h", h=H)[:, :, :, None] \
                    .to_broadcast([P, nv, H, D])
                nc.vector.tensor_tensor(out=dv, in0=sv_, in1=mb, op=AT.subtract)
                nc.vector.tensor_tensor(out=dv, in0=dv, in1=rb, op=AT.mult)
                dv2 = dst_ap.rearrange("p (a e) -> p a e", e=HD)
                nc.vector.tensor_tensor(
                    out=dv2, in0=dv2,
                    in1=sc4[:, k * HD:(k + 1) * HD][:, None, :]
                    .to_broadcast([P, nv, HD]), op=AT.mult)
                nc.vector.tensor_tensor(
                    out=dv2, in0=dv2,
                    in1=op4[:, k * HD:(k + 1) * HD][:, None, :]
                    .to_broadcast([P, nv, HD]), op=AT.add)

            # ---- h_query -> HQD ----
            CURVA = pp.tile([P, NVD * HD], f32)
            CURVB = pp.tile([P, (NVD - B0) * HD], f32)
            t0 = wp.tile([P, NVD * HD], f32, tag="t0")
            if STAGE >= 5:
                gq = gp.tile([P, TARGET * 51], bf16, tag="gh")
                nc.gpsimd.indirect_dma_start(
                    out=gq[:, :NVD * HD],
                    out_offset=None, in_=FTELER.ap(),
                    in_offset=IOA(ap=iqd[:], axis=0))
                nc.vector.tensor_copy(CURVA[:], gq[:, :NVD * HD])
                feat_trans(t0[:], CURVA[:], 0, NVD)
                nc.sync.dma_start(
                    HQD.ap().rearrange("(p a) e -> p a e", p=P),
                    t0[:].rearrange("p (a e) -> p a e", e=HD))

            # ---- score passes ----
            EXP3 = pp.tile([P, NTED * H], bf16)
            SDt = pp.tile([P, NVD * H], f32)
            SSt = pp.tile([P, NVS * H], f32)

            def score_pass(plan, lk, ioth_t, keyp, eoff, sv, exp_keep):
                for (j0, nj, C) in plan:
                    eg = gp.tile([P, TARGET * H], bf16, tag="eg")
                    nc.gpsimd.indirect_dma_start(
                        out=eg[:, :nj * H],
                        out_offset=None, in_=FTELER.ap(),
                        in_offset=IOA(ap=ioth_t[:, j0:j0 + nj], axis=0),
                        element_offset=eoff)
                    ggg = nj // C
                    vb = lk[(j0, C)]
                    if exp_keep is not None:
                        et = exp_keep[:, j0 * H:(j0 + nj) * H]
                    else:
                        ett = gp.tile([P, TARGET * H], bf16, tag="et")
                        et = ett[:, :nj * H]
                    # e = el[oth] + er[key]  (key side broadcast over class)
                    nc.vector.tensor_tensor(
                        out=et.rearrange("p (g c e) -> p g c e", c=C, e=H),
                        in0=eg[:, :nj * H].rearrange(
                            "p (g c e) -> p g c e", c=C, e=H),
                        in1=keyp[:, vb * H:(vb + ggg) * H].rearrange(
                            "p (g e) -> p g e", e=H)[:, :, None, :]
                        .to_broadcast([P, ggg, C, H]),
                        op=AT.add)
                    lrt = gp.tile([P, TARGET * H], bf16, tag="lrt")
                    nc.vector.tensor_scalar_min(lrt[:, :nj * H], et, 0.0)
                    nc.vector.tensor_scalar_max(et, et, 0.0)
                    nc.vector.tensor_scalar_mul(lrt[:, :nj * H],
                                                lrt[:, :nj * H], NEG)
                    nc.vector.tensor_tensor(out=et, in0=et,
                                            in1=lrt[:, :nj * H], op=AT.add)
                    nc.scalar.activation(et, et, AF.Exp)
                    nc.vector.tensor_reduce(
                        out=sv[:, vb * H:(vb + ggg) * H].rearrange(
                            "p (g e) -> p g e", e=H),
                        in_=et.rearrange("p (g c e) -> p g e c", c=C, e=H),
                        axis=AX.X, op=AT.add)

            nc.vector.memset(SDt[:], 0.0)
            if STAGE >= 2:
                score_pass(pland, lk_d, iod, erp, 51, SDt, EXP3)
            nc.vector.memset(SSt[:], 0.0)
            if STAGE >= 3:
                score_pass(plans, lk_s, ios, elp, 54, SSt, None)

            # rsd (local, stays in SBUF); rss -> bf16 -> AllGather -> scatter
            rsd = pp.tile([P, NVD * H], f32)
            nc.vector.tensor_scalar_max(rsd[:], SDt[:], 1e-30)
            nc.vector.reciprocal(rsd[:], rsd[:])
            nc.scalar.activation(rsd[:], rsd[:], AF.Sqrt)

            rssf = wp.tile([P, NVS * H], f32, tag="rssf")
            nc.vector.tensor_scalar_max(rssf[:], SSt[:], 1e-30)
            nc.vector.reciprocal(rssf[:], rssf[:])
            nc.scalar.activation(rssf[:], rssf[:], AF.Sqrt)
            # zero out entries whose sum was exactly 0 (pad / no-out-edge)
            ind = wp.tile([P, NVS * H], f32, tag="ind")
            nc.vector.tensor_scalar_mul(ind[:], SSt[:], 1e30)
            nc.vector.tensor_scalar_min(ind[:], ind[:], 1.0)
            nc.vector.tensor_tensor(out=rssf[:], in0=rssf[:], in1=ind[:],
                                    op=AT.mult)
            ssb = wp.tile([P, NVS * 4], bf16, tag="ssb")
            nc.vector.memset(ssb[:], 0.0)
            nc.vector.tensor_copy(
                ssb[:].rearrange("p (a e) -> p a e", e=4)[:, :, 0:H],
                rssf[:].rearrange("p (a e) -> p a e", e=H))
            nc.sync.dma_start(
                SSh.ap().rearrange("(p a) e -> p a e", p=P),
                ssb[:].rearrange("p (a e) -> p a e", e=4))
            if STAGE >= 3:
                if USE_COLL:
                    nc.gpsimd.collective_compute(
                        "AllGather", AT.bypass, ins=[SSh.ap()], outs=[SS.ap()],
                        replica_groups=rg)
                else:
                    nc.sync.dma_start(SS.ap()[0:NSVS, :], SSh.ap())
            tc.strict_bb_all_engine_barrier()
            # load gathered rss, scatter into FTELER cols 48:51
            ssl = wp.tile([P, NCORES * NVS * 4], bf16, tag="ssl")
            nc.sync.dma_start(
                ssl[:].rearrange("p (c a e) -> p c a e", c=NCORES, e=4),
                SS.ap().rearrange("(c p a) e -> p c a e", c=NCORES, p=P))
            if STAGE >= 4:
                nc.gpsimd.indirect_dma_start(
                    out=FTELER.ap(),
                    out_offset=IOA(ap=isc[:], axis=0),
                    in_=ssl[:].rearrange("p (a e) -> p a e", e=4)[:, :, 0:H],
                    in_offset=None, element_offset=48)
            tc.strict_bb_all_engine_barrier()

            # ---- tree segment-sum helper ----
            def tree_sum(gv, ggg, C, sstr, vb):
                # gv: [P, nj*sstr] view base (bf16), slots of width sstr,
                # msg in els 0:HD. Reduce C slots per group into slot 0.
                cc = C
                while cc > 1:
                    half = cc // 2
                    if cc % 2 == 1:
                        nc.vector.tensor_tensor(
                            out=gv.rearrange("p (g c e) -> p g c e",
                                             c=C, e=sstr)[:, :, 0:1, 0:HD],
                            in0=gv.rearrange("p (g c e) -> p g c e",
                                             c=C, e=sstr)[:, :, 0:1, 0:HD],
                            in1=gv.rearrange("p (g c e) -> p g c e",
                                             c=C, e=sstr)[:, :, cc - 1:cc, 0:HD],
                            op=AT.add)
                    v4 = gv.rearrange("p (g c e) -> p g c e", c=C, e=sstr)
                    nc.vector.tensor_tensor(
                        out=v4[:, :, 0:half, 0:HD],
                        in0=v4[:, :, 0:half, 0:HD],
                        in1=v4[:, :, half:2 * half, 0:HD],
                        op=AT.add)
                    cc = half
                dst_t = (CURVA[:, vb * HD:(vb + ggg) * HD] if vb < B0
                         else CURVB[:, (vb - B0) * HD:(vb - B0 + ggg) * HD])
                nc.vector.tensor_copy(
                    dst_t.rearrange("p (g e) -> p g e", e=HD),
                    gv.rearrange("p (g c e) -> p g c e",
                                 c=C, e=sstr)[:, :, 0, 0:HD])

            # ---- hops ----
            # class chunks overwrite CURVA/CURVB groups [0, VGE); only the
            # zeros/spare tail needs zeroing, once
            nc.vector.memset(CURVB[:, (VGE - B0) * HD:], 0.0)
            KTOP = 0 if STAGE < 6 else (STAGE - 5 if STAGE < 9 else K)

            def send_seg(s0, s1, kk):
                curb = gp.tile([P, TARGET * HD], f8, tag="g8")
                src_t = (CURVA[:, s0 * HD:s1 * HD] if s1 <= B0
                         else CURVB[:, (s0 - B0) * HD:(s1 - B0) * HD])
                nc.scalar.activation(curb[:, :(s1 - s0) * HD],
                                     src_t, AF.Copy)
                nc.sync.dma_start(
                    CURSH.ap().rearrange("(p a) e -> p a e", p=P)[:, s0:s1, :],
                    curb[:, :(s1 - s0) * HD].rearrange(
                        "p (a e) -> p a e", e=HD))
                if USE_COLL:
                    nc.gpsimd.collective_compute(
                        "AllGather", AT.bypass,
                        ins=[CURSH.ap().rearrange(
                            "(p a) e -> p a e", p=P)[:, s0:s1, :]],
                        outs=[CURG[kk - 1].ap().rearrange(
                            "(c p a) e -> c p a e",
                            c=NCORES, p=P)[:, :, s0:s1, :]],
                        replica_groups=rg)
                else:
                    nc.sync.dma_start(
                        CURG[kk - 1].ap().rearrange(
                            "(c p a) e -> c p a e",
                            c=NCORES, p=P)[0, :, s0:s1, :],
                        CURSH.ap().rearrange(
                            "(p a) e -> p a e", p=P)[:, s0:s1, :])

            for k in range(1, KTOP + 1):
                tc.strict_bb_all_engine_barrier()
                nseg_sent = 0
                for (j0, nj, C) in pland:
                    ggg = nj // C
                    vb = lk_d[(j0, C)]
                    g = gp.tile([P, TARGET * 51], bf16, tag="gh")
                    if k == 1:
                        sstr = 51
                        nc.gpsimd.indirect_dma_start(
                            out=g[:, :nj * 51],
                            out_offset=None, in_=FTELER.ap(),
                            in_offset=IOA(ap=iod[:, j0:j0 + nj], axis=0))
                        gv = g[:, :nj * 51]
                        # A = exp * rsd(bcast) * rss(gathered); keep in EXP3
                        ev = EXP3[:, j0 * H:(j0 + nj) * H]
                        nc.vector.tensor_tensor(
                            out=ev.rearrange("p (g c e) -> p g c e",
                                             c=C, e=H),
                            in0=ev.rearrange("p (g c e) -> p g c e",
                                             c=C, e=H),
                            in1=rsd[:, vb * H:(vb + ggg) * H].rearrange(
                                "p (g e) -> p g e", e=H)[:, :, None, :]
                            .to_broadcast([P, ggg, C, H]),
                            op=AT.mult)
                        nc.vector.tensor_tensor(
                            out=ev.rearrange("p (j e) -> p j e", e=H),
                            in0=ev.rearrange("p (j e) -> p j e", e=H),
                            in1=gv.rearrange("p (j e) -> p j e",
                                             e=51)[:, :, 48:51],
                            op=AT.mult)
                    else:
                        sstr = HD
                        g8 = gp.tile([P, TARGET * HD], f8, tag="g8")
                        nc.gpsimd.indirect_dma_start(
                            out=g8[:, :nj * HD],
                            out_offset=None, in_=CURG[k - 2].ap(),
                            in_offset=IOA(ap=icu[:, j0:j0 + nj], axis=0))
                        nc.scalar.activation(g[:, :nj * HD], g8[:, :nj * HD],
                                             AF.Copy)
                        gv = g[:, :nj * HD]
                    # msg *= A broadcast over d (packed-last 2-byte).
                    # For k==1 the 51-el slot factors as 17x3; els 48:51
                    # (rss) get multiplied too but are dead afterwards.
                    nc.vector.tensor_tensor(
                        out=gv.rearrange("p (j d h) -> p j d h",
                                         d=sstr // H, h=H),
                        in0=gv.rearrange("p (j d h) -> p j d h",
                                         d=sstr // H, h=H),
                        in1=EXP3[:, j0 * H:(j0 + nj) * H].rearrange(
                            "p (j e) -> p j e", e=H)[:, :, None, :]
                        .to_broadcast([P, nj, sstr // H, H]),
                        op=AT.mult)
                    tree_sum(gv, ggg, C, sstr, vb)
                    if k < K and nseg_sent == 0 and vb + ggg >= B0:
                        send_seg(0, B0, k)
                        nseg_sent = 1
                if k < K:
                    if nseg_sent == 0:
                        send_seg(0, B0, k)
                    send_seg(B0, NVD, k)
                feat_trans(t0[:, :B0 * HD], CURVA[:, :B0 * HD], k, B0)
                feat_trans(t0[:, B0 * HD:], CURVB[:], k, NVD - B0)
                nc.sync.dma_start(
                    HSTK[k - 1].ap().rearrange("(p a) e -> p a e", p=P),
                    t0[:].rearrange("p (a e) -> p a e", e=HD))

            # ---- final hop attention ----
            if STAGE < 9:
                dum = wp.tile([P, NVD * HD], f32, tag="t0")
                nc.vector.memset(dum[:], 1.0)
                nc.sync.dma_start(
                    out_t.ap().rearrange("(p a) e -> p a e", p=P),
                    dum[:].rearrange("p (a e) -> p a e", e=HD))
            else:
                tc.strict_bb_all_engine_barrier()
                lq = wp.tile([P, NVD * H], f32, tag="lq")
                hlv = hlr[:, 0:HD].rearrange(
                    "p (d h) -> p h d", h=H)[:, None, :, :] \
                    .to_broadcast([P, NVD, H, D])
                hrv = hlr[:, HD:2 * HD].rearrange(
                    "p (d h) -> p h d", h=H)[:, None, :, :] \
                    .to_broadcast([P, NVD, H, D])
                wk = wp.tile([P, NVD * HD], f32, tag="wk")
                nc.sync.dma_start(
                    wk[:].rearrange("p (a e) -> p a e", e=HD),
                    HQD.ap().rearrange("(p a) e -> p a e", p=P))
                nc.vector.tensor_tensor(
                    out=t0[:].rearrange("p (a d h) -> p a h d", h=H, d=D),
                    in0=wk[:].rearrange("p (a d h) -> p a h d", h=H, d=D),
                    in1=hlv, op=AT.mult)
                nc.vector.tensor_reduce(
                    out=lq[:].rearrange("p (a h) -> p a h", h=H),
                    in_=t0[:].rearrange("p (a d h) -> p a h d", h=H, d=D),
                    axis=AX.X, op=AT.add)
                # single pass: acc = sum_k wk*exp(lg_k); den = sum_k exp(lg_k)
                # (divide once at the end)
                ek = wp.tile([P, NVD * H], f32, tag="ek")
                ekn = wp.tile([P, NVD * H], f32, tag="ekn")
                den = wp.tile([P, NVD * H], f32, tag="den")
                acc = CURVA
                nc.vector.memset(acc[:], 0.0)
                nc.vector.memset(den[:], 0.0)
                for k in range(K):
                    nc.sync.dma_start(
                        wk[:].rearrange("p (a e) -> p a e", e=HD),
                        HSTK[k].ap().rearrange("(p a) e -> p a e", p=P))
                    nc.vector.tensor_tensor(
                        out=t0[:].rearrange("p (a d h) -> p a h d", h=H, d=D),
                        in0=wk[:].rearrange("p (a d h) -> p a h d", h=H, d=D),
                        in1=hrv, op=AT.mult)
                    nc.vector.tensor_reduce(
                        out=ek[:].rearrange("p (a h) -> p a h", h=H),
                        in_=t0[:].rearrange("p (a d h) -> p a h d", h=H, d=D),
                        axis=AX.X, op=AT.add)
                    nc.vector.tensor_tensor(out=ek[:], in0=ek[:], in1=lq[:],
                                            op=AT.add)
                    nc.vector.tensor_scalar_min(ekn[:], ek[:], 0.0)
                    nc.vector.tensor_scalar_max(ek[:], ek[:], 0.0)
                    nc.vector.tensor_scalar_mul(ekn[:], ekn[:], NEG)
                    nc.vector.tensor_tensor(out=ek[:], in0=ek[:], in1=ekn[:],
                                            op=AT.add)
                    nc.scalar.activation(ek[:], ek[:], AF.Exp)
                    nc.vector.tensor_tensor(out=den[:], in0=den[:], in1=ek[:],
                                            op=AT.add)
                    nc.vector.tensor_tensor(
                        out=t0[:].rearrange("p (a d h) -> p a d h", h=H, d=D),
                        in0=wk[:].rearrange("p (a d h) -> p a d h", h=H, d=D),
                        in1=ek[:].rearrange("p (a h) -> p a h",
                                            h=H)[:, :, None, :]
                        .to_broadcast([P, NVD, D, H]), op=AT.mult)
                    nc.vector.tensor_tensor(out=acc[:], in0=acc[:], in1=t0[:],
                                            op=AT.add)
                nc.vector.reciprocal(den[:], den[:])
                nc.vector.tensor_tensor(
                    out=acc[:].rearrange("p (a d h) -> p a d h", h=H, d=D),
                    in0=acc[:].rearrange("p (a d h) -> p a d h", h=H, d=D),
                    in1=den[:].rearrange("p (a h) -> p a h",
                                         h=H)[:, :, None, :]
                    .to_broadcast([P, NVD, D, H]), op=AT.mult)
                nc.sync.dma_start(
                    out_t.ap().rearrange("(p a) e -> p a e", p=P),
                    acc[:].rearrange("p (a e) -> p a e", e=HD))
    nc.compile()
    return nc


# ---------------------------------------------------------------- entry
def kernel(**inputs):
    feat = np.asarray(inputs["feat"], np.float32)
    src = np.asarray(inputs["src"]).astype(np.int64)
    dst = np.asarray(inputs["dst"]).astype(np.int64)
    fc_W = np.asarray(inputs["fc_W"], np.float32)
    attn_l = np.asarray(inputs["attn_l"], np.float32).reshape(H, D)
    attn_r = np.asarray(inputs["attn_r"], np.float32).reshape(H, D)
    hop_l = np.asarray(inputs["hop_attn_l"], np.float32).reshape(H, D)
    hop_r = np.asarray(inputs["hop_attn_r"], np.float32).reshape(H, D)
    pos = np.asarray(inputs["pos_emb"], np.float32)
    nsc = np.asarray(inputs["norm_scales"], np.float32)
    off = np.asarray(inputs["offsets"], np.float32)
    bias = np.asarray(inputs["bias"], np.float32).reshape(1, H, D)

    sides = host_prep(src, dst)
    sd, ssd = sides["d"], sides["s"]
    nc = build_nc(sd["NV"], sd["NTE"], sd["plan"],
                  ssd["NV"], ssd["NTE"], ssd["plan"])
    NVD, NVS = sd["NV"], ssd["NV"]

    # d-major permutation: column d*H + h holds (head h, dim d)
    perm = np.arange(HD).reshape(H, D).T.reshape(-1)  # (d-major) <- (h-major)
    W48 = fc_W.reshape(IN, H, D).transpose(0, 2, 1).reshape(IN, HD)
    Wl = np.einsum("ihd,hd->ih", fc_W.reshape(IN, H, D), attn_l)
    Wr = np.einsum("ihd,hd->ih", fc_W.reshape(IN, H, D), attn_r)
    W54 = np.concatenate([W48, Wl, Wr], 1).astype(ml_dtypes.bfloat16)

    featP = np.zeros((NP_, IN), np.float32)
    featP[:N] = feat
    featT = np.ascontiguousarray(featP.T).astype(ml_dtypes.bfloat16)
    sc = nsc.reshape(K + 1, HD)[:, perm]
    opv = (off.reshape(K + 1, HD) +
           pos[0].transpose(1, 0, 2).reshape(K + 1, HD))[:, perm]
    scales4 = np.tile(sc.reshape(1, -1), (P, 1)).astype(np.float32)
    offpos4 = np.tile(opv.reshape(1, -1), (P, 1)).astype(np.float32)
    hop2 = np.tile(np.concatenate(
        [hop_l.T.reshape(1, HD), hop_r.T.reshape(1, HD)], 1), (P, 1))

    in_maps = []
    for c in range(NCORES):
        cd, cs = sd["cores"][c], ssd["cores"][c]
        iq_d = np.zeros(P * NVD, np.int64)
        iq_d[cd["vrow"]] = np.arange(NS) + c * NS
        iq_s = np.zeros(P * NVS, np.int64)
        iq_s[cs["vrow"]] = np.arange(NS) + c * NS
        # scatter targets: SBUF slot (p, c2*NVS + a) holds SS row
        # c2*NSVS + p*NVS + a  -> node owned by c2 at that vrow (or pad row)
        iscat = np.full((P, NCORES * NVS), NP_, np.int64)
        for c2 in range(NCORES):
            vr = ssd["cores"][c2]["vrow"]   # node-local -> vrow
            pmat = vr // NVS
            amat = vr % NVS
            iscat[pmat, c2 * NVS + amat] = np.arange(NS) + c2 * NS
        in_maps.append(dict(
            featT=featT, W_in=W54,
            hop_lr=hop2.astype(np.float32),
            scales4=scales4, offpos4=offpos4,
            iOTHd=cd["ioth"], iOTHs=cs["ioth"],
            iCUR=sd["vmap"][cd["ioth"]].astype(np.int32),
            iQd=iq_d.reshape(P, NVD).astype(np.int32),
            iQs=iq_s.reshape(P, NVS).astype(np.int32),
            iSCAT=iscat.astype(np.int32),
        ))

    outs = None
    exec_ns = None
    if os.environ.get("AGDN_HW", "") == "1":
        try:
            from concourse import bass_utils
            trace = os.environ.get("AGDN_TRACE", "") == "1"
            res = bass_utils.run_bass_kernel_spmd(
                nc, in_maps, core_ids=list(range(NCORES)), trace=trace)
            outs = [np.asarray(res.results[c]["out"]) for c in range(NCORES)]
            exec_ns = res.exec_time_ns
            if not all(np.isfinite(o).all() for o in outs):
                print("[kernel] HW returned non-finite values", flush=True)
                outs = None
        except Exception as e:
            print(f"[kernel] HW path failed: {type(e).__name__}", flush=True)
            outs = None
    if outs is None:
        from concourse.bass_interp import MultiCoreSim
        nworkers = 1 if os.environ.get("AGDN_TRACE", "") == "1" else NCORES
        sim = MultiCoreSim(nc, num_cores=NCORES, num_workers=nworkers,
                           trace=False, require_finite=False,
                           require_nnan=False)
        for c, core in enumerate(sim.cores.values()):
            for kk, vv in in_maps[c].items():
                core.tensor(kk)[:] = vv
        sim.simulate(check_with_hw=False)
        outs = [np.array(core.tensor("out")) for core in sim.cores.values()]
        if nworkers == 1:
            exec_ns = int(sim.global_time)
    if exec_ns:
        print(f"[kernel] exec_time_ns={exec_ns}", flush=True)
        try:
            with open("/tmp/agdn_exec_ns.txt", "w") as f:
                f.write(str(exec_ns))
        except OSError:
            pass
    full = np.concatenate(outs, 0)
    out = full[sd["vmap"][:N]].reshape(N, D, H).transpose(0, 2, 1)
    return np.ascontiguousarray(out + bias).astype(np.float32)


# revision 29
# speedup vs baseline: 1.1008x; 1.0009x over previous
"""AGDNConv (3-hop attention diffusion GNN) on 8 trn2 NeuronCores.

Sharding: edges partitioned by dst-owner (owner = dst // 12544); node tables
replicated (P0 matmul) or AllGathered per hop. Per-core segment sums use a
degree-class slot layout so they become tree/strided tensor_reduce ops.
Attention softmax uses the max-free identity
  a[e] = exp(e_e) * rsqrt(s_dst[dst_e]) * rsqrt(s_src[src_e]).

v2 layout decisions (all driven by the DMA descriptor cost model):
 - One bf16 node table FTELER [NP_+1, 58]: cols 0:48 ft in d-major (d,h)
   order, 48:51 rss (scattered in after the src-side AllGather), 51:54 el,
   54:57 er.  Hop-1 gathers cols 0:51 in ONE 102B descriptor per edge
   (ft + rss together); score passes gather only the 6B other-side score.
 - Key-side score values are broadcast per node group (no per-edge gather);
   per-node sums (SD) stay in SBUF, so the old "stage C" disappears.
 - No mask tensors: the pad row NP_ has el=er=-60 so exp==0 naturally, and
   rss==0 on the pad row kills pad contributions in hops 2/3.
 - d-major feature order makes every big DVE multiply packed-last (2x/4x
   DVE modes); segment sums are tree adds (packed) instead of strided 1x.

Execution: the axon terminal's NRT shim does not implement dynamic-AP
(indirect) DMA - it returns garbage data for gathers and wedges the device
on larger elements (verified empirically; see dev notes). kernel() therefore
runs the cycle-accurate MultiCoreSim (single worker, real collectives) by
default, which validates and times the exact Bass program. Set AGDN_HW=1 to
attempt real-HW execution via PJRT on terminals with a fixed runtime.
"""
import sys
sys.path.insert(0, "/opt/trn_rl_repo")
import os
import numpy as np
import ml_dtypes

USE_COLL = os.environ.get("AGDN_NOCOLL", "") != "1"
STAGE = int(os.environ.get("AGDN_STAGE", "9"))
P = 128
N = 100000
IN = 128
H = 3
D = 16
HD = 48
K = 3
NEG = 0.2
EPS = 1e-9
NCORES = 8
NS = 12544
NP_ = NCORES * NS
ROWW = 256         # FTELER row width (bf16 elements; 512B rows avoid the
                   # small-descriptor 2x latency multiplier on gathers)
PADV = -60.0       # pad-row el/er value: exp(leaky(-60+x)) ~ 0
CLASSES = [4, 8, 12, 16, 20, 24, 32, 40, 48, 64, 96, 128]
TARGET = 176       # chunk width target, in slots


# ---------------------------------------------------------------- host prep
def _pack_side(key_node, other_node, n_lo):
    loc = (key_node - n_lo).astype(np.int64)
    order = np.argsort(loc, kind="stable")
    loc_s = loc[order]
    other_s = other_node[order]
    deg = np.bincount(loc_s, minlength=NS)
    assert deg.max() <= CLASSES[-1], f"degree {deg.max()} exceeds max class"
    starts = np.concatenate([[0], np.cumsum(deg)[:-1]])
    cls_of = np.full(NS, -1, np.int64)
    lo = 0
    for ci, C in enumerate(CLASSES):
        cls_of[(deg > lo) & (deg <= C)] = ci
        lo = C
    members = [np.where(cls_of == ci)[0] for ci in range(len(CLASSES))]
    zeros = np.where(deg == 0)[0]
    return dict(members=members, zeros=zeros, deg=deg, starts=starts,
                other_s=other_s)


def _layout(counts_max, gz_max):
    G, vg0, je0, plan = [], [], [], []
    v, j = 0, 0
    for ci, C in enumerate(CLASSES):
        g = int(np.ceil(counts_max[ci] / P))
        G.append(g)
        vg0.append(v)
        je0.append(j)
        cols = g * C
        step = max(C, (TARGET // C) * C)
        s = 0
        while s < cols:
            w = min(step, cols - s)
            plan.append((j + s, w, C))
            s += w
        v += g
        j += g * C
    vg0.append(v)
    # zeros region + one spare always-unassigned group (its CUR rows stay 0)
    v += int(np.ceil(gz_max / P)) + 1
    return G, vg0, je0, max(v, 1), max(((j + 3) // 4) * 4, 4), plan


def _fill_core(pack, G, vg0, je0, NV, NTE, n_lo):
    ioth = np.full((P, NTE), NP_, np.int32)    # pad -> FTELER pad row
    vrow = np.full(NS, -1, np.int64)
    deg, starts, other_s = pack["deg"], pack["starts"], pack["other_s"]
    for ci, C in enumerate(CLASSES):
        mem = pack["members"][ci]
        g_c = max(G[ci], 1)
        for i, nl in enumerate(mem):
            g, p = i % g_c, i // g_c
            vrow[nl] = p * NV + vg0[ci] + g
            d, s0 = deg[nl], starts[nl]
            je = je0[ci] + g * C
            ioth[p, je:je + d] = other_s[s0:s0 + d]
    mem = pack["zeros"]
    gz = max(int(np.ceil(len(mem) / P)), 1)
    for i, nl in enumerate(mem):
        vrow[nl] = (i // gz) * NV + vg0[len(CLASSES)] + i % gz
    return ioth, vrow


def host_prep(src, dst):
    sides = {}
    for side, key, oth in (("d", dst, src), ("s", src, dst)):
        packs = []
        for c in range(NCORES):
            m = (key >= c * NS) & (key < (c + 1) * NS)
            packs.append(_pack_side(key[m], oth[m], c * NS))
        counts_max = np.max(
            np.array([[len(p) for p in pk["members"]] for pk in packs]), axis=0)
        gz_max = max(len(pk["zeros"]) for pk in packs)
        G, vg0, je0, NV, NTE, plan = _layout(counts_max, gz_max)
        cores, vmap = [], np.zeros(NP_ + 1, np.int64)
        for c in range(NCORES):
            ioth, vrow = _fill_core(packs[c], G, vg0, je0, NV, NTE, c * NS)
            cores.append(dict(ioth=ioth, vrow=vrow))
            vmap[c * NS:(c + 1) * NS] = c * (P * NV) + vrow
        # pad/ghost entries -> this side's guaranteed-zero row of core 0
        # (the spare group; never assigned by _fill_core)
        zrow = [c * (P * NV) + 127 * NV + (NV - 1) for c in range(NCORES)]
        vmap[vmap < 0] = zrow[0]
        sides[side] = dict(NV=NV, NTE=NTE, plan=plan, cores=cores, vmap=vmap,
                           zrow=zrow)
    return sides


def _vg_lookup(plan):
    lk, vg, last_C, cj, cvg = {}, 0, None, None, None
    for (j0, nj, C) in plan:
        if C != last_C:
            cj, cvg, last_C = j0, vg, C
        lk[(j0, C)] = cvg + (j0 - cj) // C
        vg = cvg + (j0 - cj + nj) // C
    return lk


# ---------------------------------------------------------------- device
def build_nc(NVD, NTED, pland, NVS, NTES, plans):
    import concourse.bass as bass
    import concourse.bacc as bacc
    import concourse.mybir as mybir
    import concourse.tile as tile
    f32, bf16, i32 = mybir.dt.float32, mybir.dt.bfloat16, mybir.dt.int32
    AT, AF, AX = mybir.AluOpType, mybir.ActivationFunctionType, mybir.AxisListType
    IOA = bass.IndirectOffsetOnAxis
    NSVD, NSVS = P * NVD, P * NVS
    lk_d, lk_s = _vg_lookup(pland), _vg_lookup(plans)
    VGE = max(lk_d[(j0, C)] + nj // C for (j0, nj, C) in pland)
    B0 = VGE
    for (j0, nj, C) in pland:
        vbe = lk_d[(j0, C)] + nj // C
        if vbe >= int(VGE * 0.65):
            B0 = vbe
            break

    nc = bacc.Bacc("TRN2", target_bir_lowering=False, debug=False,
                   num_devices=NCORES)
    featT = nc.dram_tensor("featT", [P, NP_], bf16, kind="ExternalInput")
    W_in = nc.dram_tensor("W_in", [P, 54], bf16, kind="ExternalInput")
    hop_lr = nc.dram_tensor("hop_lr", [P, 2 * HD], f32, kind="ExternalInput")
    scales4 = nc.dram_tensor("scales4", [P, (K + 1) * HD], f32, kind="ExternalInput")
    offpos4 = nc.dram_tensor("offpos4", [P, (K + 1) * HD], f32, kind="ExternalInput")
    iOTHd = nc.dram_tensor("iOTHd", [P, NTED], i32, kind="ExternalInput")
    iOTHs = nc.dram_tensor("iOTHs", [P, NTES], i32, kind="ExternalInput")
    iCUR = nc.dram_tensor("iCUR", [P, NTED], i32, kind="ExternalInput")
    iQd = nc.dram_tensor("iQd", [P, NVD], i32, kind="ExternalInput")
    iQs = nc.dram_tensor("iQs", [P, NVS], i32, kind="ExternalInput")
    iSCAT = nc.dram_tensor("iSCAT", [P, NCORES * NVS], i32, kind="ExternalInput")
    out_t = nc.dram_tensor("out", [NSVD, HD], f32, kind="ExternalOutput")

    FTELER = nc.dram_tensor("FTELER", [NP_ + 1, ROWW], bf16, kind="Internal")
    SSh = nc.dram_tensor("SSh", [NSVS, 4], bf16, kind="Internal")
    SS = nc.dram_tensor("SS", [NCORES * NSVS, 4], bf16, kind="Internal",
                        addr_space="Shared")
    f8 = mybir.dt.float8e4
    CURSH = nc.dram_tensor("CURSH", [NSVD, HD], f8, kind="Internal")
    CURG = [nc.dram_tensor(f"CURG{k}", [NCORES * NSVD, HD], f8,
                           kind="Internal", addr_space="Shared")
            for k in range(K - 1)]
    HSTK = [nc.dram_tensor(f"HSTK{k}", [NSVD, HD], f32, kind="Internal")
            for k in range(K)]
    HQD = nc.dram_tensor("HQD", [NSVD, HD], f32, kind="Internal")
    rg = [list(range(NCORES))]

    with tile.TileContext(nc) as tc:
        with tc.tile_pool(name="persist", bufs=1) as pp, \
             tc.tile_pool(name="work", bufs=1) as wp, \
             tc.tile_pool(name="gat", bufs=2) as gp, \
             tc.tile_pool(name="ps", bufs=2, space="PSUM") as psp:
            # ---- weights / constants / persistent index tiles ----
            wwa = pp.tile([P, 54], bf16)
            nc.sync.dma_start(wwa[:], W_in.ap())
            hlr = pp.tile([P, 2 * HD], f32)
            nc.sync.dma_start(hlr[:], hop_lr.ap())
            sc4 = pp.tile([P, (K + 1) * HD], f32)
            nc.sync.dma_start(sc4[:], scales4.ap())
            op4 = pp.tile([P, (K + 1) * HD], f32)
            nc.sync.dma_start(op4[:], offpos4.ap())
            epst = pp.tile([P, 1], f32)
            nc.vector.memset(epst[:], EPS)
            iod = pp.tile([P, NTED], i32)
            nc.sync.dma_start(iod[:], iOTHd.ap())
            ios = pp.tile([P, NTES], i32)
            nc.sync.dma_start(ios[:], iOTHs.ap())
            icu = pp.tile([P, NTED], i32)
            nc.sync.dma_start(icu[:], iCUR.ap())
            iqd = pp.tile([P, NVD], i32)
            nc.sync.dma_start(iqd[:], iQd.ap())
            iqs = pp.tile([P, NVS], i32)
            nc.sync.dma_start(iqs[:], iQs.ap())
            isc = pp.tile([P, NCORES * NVS], i32)
            nc.sync.dma_start(isc[:], iSCAT.ap())

            # ---- P0: replicated feat matmul -> FTELER rows ----
            GRP = 1024
            NSTG = 4
            stgs = []
            for b in range(NSTG):
                st = wp.tile([P, 8 * 57], bf16, tag=f"p0st{b}")
                nc.vector.memset(st[:], 0.0)
                stgs.append(st)
            for g in range(NP_ // GRP):
                fch = wp.tile([P, GRP], bf16, tag=f"fch{g % NSTG}")
                nc.sync.dma_start(fch[:], featT.ap()[:, g * GRP:(g + 1) * GRP])
                ps = psp.tile([P, 8 * 54], f32, tag=f"p0ps{g % 2}")
                for t in range(8):
                    nc.tensor.matmul(
                        out=ps[:, t * 54:(t + 1) * 54],
                        lhsT=fch[:, t * P:(t + 1) * P],
                        rhs=wwa[:], start=True, stop=True)
                sv3 = stgs[g % NSTG][:].rearrange("q (t e) -> q t e", e=57)
                pv3 = ps[:].rearrange("q (t e) -> q t e", e=54)
                # cols 48:51 (rss) are left stale; the scatter overwrites
                # them before any read
                nc.vector.tensor_copy(sv3[:, :, 0:HD], pv3[:, :, 0:HD])
                nc.vector.tensor_copy(sv3[:, :, 51:57], pv3[:, :, HD:54])
                nc.scalar.dma_start(
                    FTELER.ap()[g * GRP:(g + 1) * GRP, 0:57].rearrange(
                        "(t p) e -> p t e", t=8),
                    sv3)
            padr = wp.tile([1, 57], bf16, tag="padr")
            nc.vector.memset(padr[:], 0.0)
            nc.vector.memset(padr[:, 51:57], PADV)
            nc.sync.dma_start(FTELER.ap()[NP_:NP_ + 1, 0:57], padr[:])
            tc.strict_bb_all_engine_barrier()

            # ---- per-node key-side score values ----
            erp = pp.tile([P, NVD * H], bf16)
            nc.gpsimd.indirect_dma_start(
                out=erp[:],
                out_offset=None, in_=FTELER.ap(),
                in_offset=IOA(ap=iqd[:], axis=0), element_offset=54)
            elp = pp.tile([P, NVS * H], bf16)
            nc.gpsimd.indirect_dma_start(
                out=elp[:],
                out_offset=None, in_=FTELER.ap(),
                in_offset=IOA(ap=iqs[:], axis=0), element_offset=51)

            # ---- feat_trans (d-major layout: 48 = (16 d) x (3 h)) ----
            def feat_trans(dst_ap, src_ap, k, nv):
                # dst/src: [P, nv*HD] f32 views
                sv_ = src_ap.rearrange("p (a d h) -> p a h d", h=H, d=D)
                dv = dst_ap.rearrange("p (a d h) -> p a h d", h=H, d=D)
                m = wp.tile([P, nv * H], f32, tag="ftm")
                ms = wp.tile([P, nv * H], f32, tag="ftms")
                mv = m[:].rearrange("p (a h) -> p a h", h=H)
                nc.vector.tensor_reduce(out=mv, in_=sv_, axis=AX.X, op=AT.add)
                nc.vector.tensor_scalar_mul(m[:], m[:], 1.0 / D)
                nc.scalar.activation(dst_ap, src_ap, AF.Square)
                nc.vector.tensor_reduce(
                    out=ms[:].rearrange("p (a h) -> p a h", h=H),
                    in_=dv, axis=AX.X, op=AT.add)
                nc.vector.tensor_scalar_mul(ms[:], ms[:], 1.0 / D)
                mm = wp.tile([P, nv * H], f32, tag="ftmm")
                nc.vector.tensor_tensor(out=mm[:], in0=m[:], in1=m[:],
                                        op=AT.mult)
                nc.vector.tensor_tensor(out=ms[:], in0=ms[:], in1=mm[:],
                                        op=AT.subtract)
                nc.scalar.activation(ms[:], ms[:], AF.Sqrt, bias=epst[:])
                nc.vector.reciprocal(ms[:], ms[:])
                mb = mv[:, :, :, None].to_broadcast([P, nv, H, D])
                rb = ms[:].rearrange("p (a h) -> p a h", h=H)[:, :, :, None] \
                    .to_broadcast([P, nv, H, D])
                nc.vector.tensor_tensor(out=dv, in0=sv_, in1=mb, op=AT.subtract)
                nc.vector.tensor_tensor(out=dv, in0=dv, in1=rb, op=AT.mult)
                dv2 = dst_ap.rearrange("p (a e) -> p a e", e=HD)
                nc.vector.tensor_tensor(
                    out=dv2, in0=dv2,
                    in1=sc4[:, k * HD:(k + 1) * HD][:, None, :]
                    .to_broadcast([P, nv, HD]), op=AT.mult)
                nc.vector.tensor_tensor(
                    out=dv2, in0=dv2,
                    in1=op4[:, k * HD:(k + 1) * HD][:, None, :]
                    .to_broadcast([P, nv, HD]), op=AT.add)

            # ---- h_query -> HQD ----
            CURVA = pp.tile([P, NVD * HD], f32)
            CURVB = pp.tile([P, (NVD - B0) * HD], f32)
            t0 = wp.tile([P, NVD * HD], f32, tag="t0")
            if STAGE >= 5:
                gq = gp.tile([P, TARGET * 51], bf16, tag="gh")
                nc.gpsimd.indirect_dma_start(
                    out=gq[:, :NVD * HD],
                    out_offset=None, in_=FTELER.ap(),
                    in_offset=IOA(ap=iqd[:], axis=0))
                nc.vector.tensor_copy(CURVA[:], gq[:, :NVD * HD])
                feat_trans(t0[:], CURVA[:], 0, NVD)
                nc.sync.dma_start(
                    HQD.ap().rearrange("(p a) e -> p a e", p=P),
                    t0[:].rearrange("p (a e) -> p a e", e=HD))

            # ---- score passes ----
            EXP3 = pp.tile([P, NTED * H], bf16)
            SDt = pp.tile([P, NVD * H], f32)
            SSt = pp.tile([P, NVS * H], f32)

            def score_pass(plan, lk, ioth_t, keyp, eoff, sv, exp_keep):
                for (j0, nj, C) in plan:
                    eg = gp.tile([P, TARGET * H], bf16, tag="eg")
                    nc.gpsimd.indirect_dma_start(
                        out=eg[:, :nj * H],
                        out_offset=None, in_=FTELER.ap(),
                        in_offset=IOA(ap=ioth_t[:, j0:j0 + nj], axis=0),
                        element_offset=eoff)
                    ggg = nj // C
                    vb = lk[(j0, C)]
                    if exp_keep is not None:
                        et = exp_keep[:, j0 * H:(j0 + nj) * H]
                    else:
                        ett = gp.tile([P, TARGET * H], bf16, tag="et")
                        et = ett[:, :nj * H]
                    # e = el[oth] + er[key]  (key side broadcast over class)
                    nc.vector.tensor_tensor(
                        out=et.rearrange("p (g c e) -> p g c e", c=C, e=H),
                        in0=eg[:, :nj * H].rearrange(
                            "p (g c e) -> p g c e", c=C, e=H),
                        in1=keyp[:, vb * H:(vb + ggg) * H].rearrange(
                            "p (g e) -> p g e", e=H)[:, :, None, :]
                        .to_broadcast([P, ggg, C, H]),
                        op=AT.add)
                    lrt = gp.tile([P, TARGET * H], bf16, tag="lrt")
                    nc.vector.tensor_scalar_min(lrt[:, :nj * H], et, 0.0)
                    nc.vector.tensor_scalar_max(et, et, 0.0)
                    nc.vector.tensor_scalar_mul(lrt[:, :nj * H],
                                                lrt[:, :nj * H], NEG)
                    nc.vector.tensor_tensor(out=et, in0=et,
                                            in1=lrt[:, :nj * H], op=AT.add)
                    nc.scalar.activation(et, et, AF.Exp)
                    nc.vector.tensor_reduce(
                        out=sv[:, vb * H:(vb + ggg) * H].rearrange(
                            "p (g e) -> p g e", e=H),
                        in_=et.rearrange("p (g c e) -> p g e c", c=C, e=H),
                        axis=AX.X, op=AT.add)

            nc.vector.memset(SDt[:], 0.0)
            if STAGE >= 2:
                score_pass(pland, lk_d, iod, erp, 51, SDt, EXP3)
            nc.vector.memset(SSt[:], 0.0)
            if STAGE >= 3:
                score_pass(plans, lk_s, ios, elp, 54, SSt, None)

            # rsd (local, stays in SBUF); rss -> bf16 -> AllGather -> scatter
            rsd = pp.tile([P, NVD * H], f32)
            nc.vector.tensor_scalar_max(rsd[:], SDt[:], 1e-30)
            nc.vector.reciprocal(rsd[:], rsd[:])
            nc.scalar.activation(rsd[:], rsd[:], AF.Sqrt)

            rssf = wp.tile([P, NVS * H], f32, tag="rssf")
            nc.vector.tensor_scalar_max(rssf[:], SSt[:], 1e-30)
            nc.vector.reciprocal(rssf[:], rssf[:])
            nc.scalar.activation(rssf[:], rssf[:], AF.Sqrt)
            # zero out entries whose sum was exactly 0 (pad / no-out-edge)
            ind = wp.tile([P, NVS * H], f32, tag="ind")
            nc.vector.tensor_scalar_mul(ind[:], SSt[:], 1e30)
            nc.vector.tensor_scalar_min(ind[:], ind[:], 1.0)
            nc.vector.tensor_tensor(out=rssf[:], in0=rssf[:], in1=ind[:],
                                    op=AT.mult)
            ssb = wp.tile([P, NVS * 4], bf16, tag="ssb")
            nc.vector.memset(ssb[:], 0.0)
            nc.vector.tensor_copy(
                ssb[:].rearrange("p (a e) -> p a e", e=4)[:, :, 0:H],
                rssf[:].rearrange("p (a e) -> p a e", e=H))
            nc.sync.dma_start(
                SSh.ap().rearrange("(p a) e -> p a e", p=P),
                ssb[:].rearrange("p (a e) -> p a e", e=4))
            if STAGE >= 3:
                if USE_COLL:
                    nc.gpsimd.collective_compute(
                        "AllGather", AT.bypass, ins=[SSh.ap()], outs=[SS.ap()],
                        replica_groups=rg)
                else:
                    nc.sync.dma_start(SS.ap()[0:NSVS, :], SSh.ap())
            tc.strict_bb_all_engine_barrier()
            # load gathered rss, scatter into FTELER cols 48:51
            ssl = wp.tile([P, NCORES * NVS * 4], bf16, tag="ssl")
            nc.sync.dma_start(
                ssl[:].rearrange("p (c a e) -> p c a e", c=NCORES, e=4),
                SS.ap().rearrange("(c p a) e -> p c a e", c=NCORES, p=P))
            if STAGE >= 4:
                nc.gpsimd.indirect_dma_start(
                    out=FTELER.ap(),
                    out_offset=IOA(ap=isc[:], axis=0),
                    in_=ssl[:].rearrange("p (a e) -> p a e", e=4)[:, :, 0:H],
                    in_offset=None, element_offset=48)
            tc.strict_bb_all_engine_barrier()

            # ---- tree segment-sum helper ----
            def tree_sum(gv, ggg, C, sstr, vb):
                # gv: [P, nj*sstr] view base (bf16), slots of width sstr,
                # msg in els 0:HD. Reduce C slots per group into slot 0.
                cc = C
                while cc > 1:
                    half = cc // 2
                    if cc % 2 == 1:
                        nc.vector.tensor_tensor(
                            out=gv.rearrange("p (g c e) -> p g c e",
                                             c=C, e=sstr)[:, :, 0:1, 0:HD],
                            in0=gv.rearrange("p (g c e) -> p g c e",
                                             c=C, e=sstr)[:, :, 0:1, 0:HD],
                            in1=gv.rearrange("p (g c e) -> p g c e",
                                             c=C, e=sstr)[:, :, cc - 1:cc, 0:HD],
                            op=AT.add)
                    v4 = gv.rearrange("p (g c e) -> p g c e", c=C, e=sstr)
                    nc.vector.tensor_tensor(
                        out=v4[:, :, 0:half, 0:HD],
                        in0=v4[:, :, 0:half, 0:HD],
                        in1=v4[:, :, half:2 * half, 0:HD],
                        op=AT.add)
                    cc = half
                dst_t = (CURVA[:, vb * HD:(vb + ggg) * HD] if vb < B0
                         else CURVB[:, (vb - B0) * HD:(vb - B0 + ggg) * HD])
                nc.vector.tensor_copy(
                    dst_t.rearrange("p (g e) -> p g e", e=HD),
                    gv.rearrange("p (g c e) -> p g c e",
                                 c=C, e=sstr)[:, :, 0, 0:HD])

            # ---- hops ----
            # class chunks overwrite CURVA/CURVB groups [0, VGE); only the
            # zeros/spare tail needs zeroing, once
            nc.vector.memset(CURVB[:, (VGE - B0) * HD:], 0.0)
            KTOP = 0 if STAGE < 6 else (STAGE - 5 if STAGE < 9 else K)

            def send_seg(s0, s1, kk):
                curb = gp.tile([P, TARGET * HD], f8, tag="g8")
                src_t = (CURVA[:, s0 * HD:s1 * HD] if s1 <= B0
                         else CURVB[:, (s0 - B0) * HD:(s1 - B0) * HD])
                nc.scalar.activation(curb[:, :(s1 - s0) * HD],
                                     src_t, AF.Copy)
                nc.sync.dma_start(
                    CURSH.ap().rearrange("(p a) e -> p a e", p=P)[:, s0:s1, :],
                    curb[:, :(s1 - s0) * HD].rearrange(
                        "p (a e) -> p a e", e=HD))
                if USE_COLL:
                    nc.gpsimd.collective_compute(
                        "AllGather", AT.bypass,
                        ins=[CURSH.ap().rearrange(
                            "(p a) e -> p a e", p=P)[:, s0:s1, :]],
                        outs=[CURG[kk - 1].ap().rearrange(
                            "(c p a) e -> c p a e",
                            c=NCORES, p=P)[:, :, s0:s1, :]],
                        replica_groups=rg)
                else:
                    nc.sync.dma_start(
                        CURG[kk - 1].ap().rearrange(
                            "(c p a) e -> c p a e",
                            c=NCORES, p=P)[0, :, s0:s1, :],
                        CURSH.ap().rearrange(
                            "(p a) e -> p a e", p=P)[:, s0:s1, :])

            for k in range(1, KTOP + 1):
                tc.strict_bb_all_engine_barrier()
                nseg_sent = 0
                for (j0, nj, C) in pland:
                    ggg = nj // C
                    vb = lk_d[(j0, C)]
                    g = gp.tile([P, TARGET * 51], bf16, tag="gh")
                    if k == 1:
                        sstr = 51
                        nc.gpsimd.indirect_dma_start(
                            out=g[:, :nj * 51],
                            out_offset=None, in_=FTELER.ap(),
                            in_offset=IOA(ap=iod[:, j0:j0 + nj], axis=0))
                        gv = g[:, :nj * 51]
                        # A = exp * rsd(bcast) * rss(gathered); keep in EXP3
                        ev = EXP3[:, j0 * H:(j0 + nj) * H]
                        nc.vector.tensor_tensor(
                            out=ev.rearrange("p (g c e) -> p g c e",
                                             c=C, e=H),
                            in0=ev.rearrange("p (g c e) -> p g c e",
                                             c=C, e=H),
                            in1=rsd[:, vb * H:(vb + ggg) * H].rearrange(
                                "p (g e) -> p g e", e=H)[:, :, None, :]
                            .to_broadcast([P, ggg, C, H]),
                            op=AT.mult)
                        nc.vector.tensor_tensor(
                            out=ev.rearrange("p (j e) -> p j e", e=H),
                            in0=ev.rearrange("p (j e) -> p j e", e=H),
                            in1=gv.rearrange("p (j e) -> p j e",
                                             e=51)[:, :, 48:51],
                            op=AT.mult)
                    else:
                        sstr = HD
                        g8 = gp.tile([P, TARGET * HD], f8, tag="g8")
                        nc.gpsimd.indirect_dma_start(
                            out=g8[:, :nj * HD],
                            out_offset=None, in_=CURG[k - 2].ap(),
                            in_offset=IOA(ap=icu[:, j0:j0 + nj], axis=0))
                        nc.scalar.activation(g[:, :nj * HD], g8[:, :nj * HD],
                                             AF.Copy)
                        gv = g[:, :nj * HD]
                    # msg *= A broadcast over d (packed-last 2-byte).
                    # For k==1 the 51-el slot factors as 17x3; els 48:51
                    # (rss) get multiplied too but are dead afterwards.
                    nc.vector.tensor_tensor(
                        out=gv.rearrange("p (j d h) -> p j d h",
                                         d=sstr // H, h=H),
                        in0=gv.rearrange("p (j d h) -> p j d h",
                                         d=sstr // H, h=H),
                        in1=EXP3[:, j0 * H:(j0 + nj) * H].rearrange(
                            "p (j e) -> p j e", e=H)[:, :, None, :]
                        .to_broadcast([P, nj, sstr // H, H]),
                        op=AT.mult)
                    tree_sum(gv, ggg, C, sstr, vb)
                    if k < K and nseg_sent == 0 and vb + ggg >= B0:
                        send_seg(0, B0, k)
                        nseg_sent = 1
                if k < K:
                    if nseg_sent == 0:
                        send_seg(0, B0, k)
                    send_seg(B0, NVD, k)
                feat_trans(t0[:, :B0 * HD], CURVA[:, :B0 * HD], k, B0)
                feat_trans(t0[:, B0 * HD:], CURVB[:], k, NVD - B0)
                nc.sync.dma_start(
                    HSTK[k - 1].ap().rearrange("(p a) e -> p a e", p=P),
                    t0[:].rearrange("p (a e) -> p a e", e=HD))

            # ---- final hop attention ----
            if STAGE < 9:
                dum = wp.tile([P, NVD * HD], f32, tag="t0")
                nc.vector.memset(dum[:], 1.0)
                nc.sync.dma_start(
                    out_t.ap().rearrange("(p a) e -> p a e", p=P),
                    dum[:].rearrange("p (a e) -> p a e", e=HD))
            else:
                tc.strict_bb_all_engine_barrier()
                lq = wp.tile([P, NVD * H], f32, tag="lq")
                hlv = hlr[:, 0:HD].rearrange(
                    "p (d h) -> p h d", h=H)[:, None, :, :] \
                    .to_broadcast([P, NVD, H, D])
                hrv = hlr[:, HD:2 * HD].rearrange(
                    "p (d h) -> p h d", h=H)[:, None, :, :] \
                    .to_broadcast([P, NVD, H, D])
                wk = wp.tile([P, NVD * HD], f32, tag="wk")
                nc.sync.dma_start(
                    wk[:].rearrange("p (a e) -> p a e", e=HD),
                    HQD.ap().rearrange("(p a) e -> p a e", p=P))
                nc.vector.tensor_tensor(
                    out=t0[:].rearrange("p (a d h) -> p a h d", h=H, d=D),
                    in0=wk[:].rearrange("p (a d h) -> p a h d", h=H, d=D),
                    in1=hlv, op=AT.mult)
                nc.vector.tensor_reduce(
                    out=lq[:].rearrange("p (a h) -> p a h", h=H),
                    in_=t0[:].rearrange("p (a d h) -> p a h d", h=H, d=D),
                    axis=AX.X, op=AT.add)
                # single pass: acc = sum_k wk*exp(lg_k); den = sum_k exp(lg_k)
                # (divide once at the end)
                ek = wp.tile([P, NVD * H], f32, tag="ek")
                ekn = wp.tile([P, NVD * H], f32, tag="ekn")
                den = wp.tile([P, NVD * H], f32, tag="den")
                acc = CURVA
                nc.vector.memset(acc[:], 0.0)
                nc.vector.memset(den[:], 0.0)
                for k in range(K):
                    nc.sync.dma_start(
                        wk[:].rearrange("p (a e) -> p a e", e=HD),
                        HSTK[k].ap().rearrange("(p a) e -> p a e", p=P))
                    nc.vector.tensor_tensor(
                        out=t0[:].rearrange("p (a d h) -> p a h d", h=H, d=D),
                        in0=wk[:].rearrange("p (a d h) -> p a h d", h=H, d=D),
                        in1=hrv, op=AT.mult)
                    nc.vector.tensor_reduce(
                        out=ek[:].rearrange("p (a h) -> p a h", h=H),
                        in_=t0[:].rearrange("p (a d h) -> p a h d", h=H, d=D),
                        axis=AX.X, op=AT.add)
                    nc.vector.tensor_tensor(out=ek[:], in0=ek[:], in1=lq[:],
                                            op=AT.add)
                    nc.vector.tensor_scalar_min(ekn[:], ek[:], 0.0)
                    nc.vector.tensor_scalar_max(ek[:], ek[:], 0.0)
                    nc.vector.tensor_scalar_mul(ekn[:], ekn[:], NEG)
                    nc.vector.tensor_tensor(out=ek[:], in0=ek[:], in1=ekn[:],
                                            op=AT.add)
                    nc.scalar.activation(ek[:], ek[:], AF.Exp)
                    nc.vector.tensor_tensor(out=den[:], in0=den[:], in1=ek[:],
                                            op=AT.add)
                    nc.vector.tensor_tensor(
                        out=t0[:].rearrange("p (a d h) -> p a d h", h=H, d=D),
                        in0=wk[:].rearrange("p (a d h) -> p a d h", h=H, d=D),
                        in1=ek[:].rearrange("p (a h) -> p a h",
                                            h=H)[:, :, None, :]
                        .to_broadcast([P, NVD, D, H]), op=AT.mult)
                    nc.vector.tensor_tensor(out=acc[:], in0=acc[:], in1=t0[:],
                                            op=AT.add)
                nc.vector.reciprocal(den[:], den[:])
                nc.vector.tensor_tensor(
                    out=acc[:].rearrange("p (a d h) -> p a d h", h=H, d=D),
                    in0=acc[:].rearrange("p (a d h) -> p a d h", h=H, d=D),
                    in1=den[:].rearrange("p (a h) -> p a h",
                                         h=H)[:, :, None, :]
                    .to_broadcast([P, NVD, D, H]), op=AT.mult)
                nc.sync.dma_start(
                    out_t.ap().rearrange("(p a) e -> p a e", p=P),
                    acc[:].rearrange("p (a e) -> p a e", e=HD))
    nc.compile()
    return nc


# ---------------------------------------------------------------- entry
def kernel(**inputs):
    feat = np.asarray(inputs["feat"], np.float32)
    src = np.asarray(inputs["src"]).astype(np.int64)
    dst = np.asarray(inputs["dst"]).astype(np.int64)
    fc_W = np.asarray(inputs["fc_W"], np.float32)
    attn_l = np.asarray(inputs["attn_l"], np.float32).reshape(H, D)
    attn_r = np.asarray(inputs["attn_r"], np.float32).reshape(H, D)
    hop_l = np.asarray(inputs["hop_attn_l"], np.float32).reshape(H, D)
    hop_r = np.asarray(inputs["hop_attn_r"], np.float32).reshape(H, D)
    pos = np.asarray(inputs["pos_emb"], np.float32)
    nsc = np.asarray(inputs["norm_scales"], np.float32)
    off = np.asarray(inputs["offsets"], np.float32)
    bias = np.asarray(inputs["bias"], np.float32).reshape(1, H, D)

    sides = host_prep(src, dst)
    sd, ssd = sides["d"], sides["s"]
    nc = build_nc(sd["NV"], sd["NTE"], sd["plan"],
                  ssd["NV"], ssd["NTE"], ssd["plan"])
    NVD, NVS = sd["NV"], ssd["NV"]

    # d-major permutation: column d*H + h holds (head h, dim d)
    perm = np.arange(HD).reshape(H, D).T.reshape(-1)  # (d-major) <- (h-major)
    W48 = fc_W.reshape(IN, H, D).transpose(0, 2, 1).reshape(IN, HD)
    Wl = np.einsum("ihd,hd->ih", fc_W.reshape(IN, H, D), attn_l)
    Wr = np.einsum("ihd,hd->ih", fc_W.reshape(IN, H, D), attn_r)
    W54 = np.concatenate([W48, Wl, Wr], 1).astype(ml_dtypes.bfloat16)

    featP = np.zeros((NP_, IN), np.float32)
    featP[:N] = feat
    featT = np.ascontiguousarray(featP.T).astype(ml_dtypes.bfloat16)
    sc = nsc.reshape(K + 1, HD)[:, perm]
    opv = (off.reshape(K + 1, HD) +
           pos[0].transpose(1, 0, 2).reshape(K + 1, HD))[:, perm]
    scales4 = np.tile(sc.reshape(1, -1), (P, 1)).astype(np.float32)
    offpos4 = np.tile(opv.reshape(1, -1), (P, 1)).astype(np.float32)
    hop2 = np.tile(np.concatenate(
        [hop_l.T.reshape(1, HD), hop_r.T.reshape(1, HD)], 1), (P, 1))

    in_maps = []
    for c in range(NCORES):
        cd, cs = sd["cores"][c], ssd["cores"][c]
        iq_d = np.zeros(P * NVD, np.int64)
        iq_d[cd["vrow"]] = np.arange(NS) + c * NS
        iq_s = np.zeros(P * NVS, np.int64)
        iq_s[cs["vrow"]] = np.arange(NS) + c * NS
        # scatter targets: SBUF slot (p, c2*NVS + a) holds SS row
        # c2*NSVS + p*NVS + a  -> node owned by c2 at that vrow (or pad row)
        iscat = np.full((P, NCORES * NVS), NP_, np.int64)
        for c2 in range(NCORES):
            vr = ssd["cores"][c2]["vrow"]   # node-local -> vrow
            pmat = vr // NVS
            amat = vr % NVS
            iscat[pmat, c2 * NVS + amat] = np.arange(NS) + c2 * NS
        in_maps.append(dict(
            featT=featT, W_in=W54,
            hop_lr=hop2.astype(np.float32),
            scales4=scales4, offpos4=offpos4,
            iOTHd=cd["ioth"], iOTHs=cs["ioth"],
            iCUR=sd["vmap"][cd["ioth"]].astype(np.int32),
            iQd=iq_d.reshape(P, NVD).astype(np.int32),
            iQs=iq_s.reshape(P, NVS).astype(np.int32),
            iSCAT=iscat.astype(np.int32),
        ))

    outs = None
    exec_ns = None
    if os.environ.get("AGDN_HW", "") == "1":
        try:
            from concourse import bass_utils
            trace = os.environ.get("AGDN_TRACE", "") == "1"
            res = bass_utils.run_bass_kernel_spmd(
                nc, in_maps, core_ids=list(range(NCORES)), trace=trace)
            outs = [np.asarray(res.results[c]["out"]) for c in range(NCORES)]
            exec_ns = res.exec_time_ns
            if not all(np.isfinite(o).all() for o in outs):
                print("[kernel] HW returned non-finite values", flush=True)
                outs = None
        except Exception as e:
            print(f"[kernel] HW path failed: {type(e).__name__}", flush=True)
            outs = None
    if outs is None:
        from concourse.bass_interp import MultiCoreSim
        nworkers = 1 if os.environ.get("AGDN_TRACE", "") == "1" else NCORES
        sim = MultiCoreSim(nc, num_cores=NCORES, num_workers=nworkers,
                           trace=False, require_finite=False,
                           require_nnan=False)
        for c, core in enumerate(sim.cores.values()):
            for kk, vv in in_maps[c].items():
                core.tensor(kk)[:] = vv
        sim.simulate(check_with_hw=False)
        outs = [np.array(core.tensor("out")) for core in sim.cores.values()]
        if nworkers == 1:
            exec_ns = int(sim.global_time)
    if exec_ns:
        print(f"[kernel] exec_time_ns={exec_ns}", flush=True)
        try:
            with open("/tmp/agdn_exec_ns.txt", "w") as f:
                f.write(str(exec_ns))
        except OSError:
            pass
    full = np.concatenate(outs, 0)
    out = full[sd["vmap"][:N]].reshape(N, D, H).transpose(0, 2, 1)
    return np.ascontiguousarray(out + bias).astype(np.float32)


# revision 30
# speedup vs baseline: 1.1055x; 1.0044x over previous
"""AGDNConv (3-hop attention diffusion GNN) on 8 trn2 NeuronCores.

Sharding: edges partitioned by dst-owner (owner = dst // 12544); node tables
replicated (P0 matmul) or AllGathered per hop. Per-core segment sums use a
degree-class slot layout so they become tree/strided tensor_reduce ops.
Attention softmax uses the max-free identity
  a[e] = exp(e_e) * rsqrt(s_dst[dst_e]) * rsqrt(s_src[src_e]).

v2 layout decisions (all driven by the DMA descriptor cost model):
 - One bf16 node table FTELER [NP_+1, 58]: cols 0:48 ft in d-major (d,h)
   order, 48:51 rss (scattered in after the src-side AllGather), 51:54 el,
   54:57 er.  Hop-1 gathers cols 0:51 in ONE 102B descriptor per edge
   (ft + rss together); score passes gather only the 6B other-side score.
 - Key-side score values are broadcast per node group (no per-edge gather);
   per-node sums (SD) stay in SBUF, so the old "stage C" disappears.
 - No mask tensors: the pad row NP_ has el=er=-60 so exp==0 naturally, and
   rss==0 on the pad row kills pad contributions in hops 2/3.
 - d-major feature order makes every big DVE multiply packed-last (2x/4x
   DVE modes); segment sums are tree adds (packed) instead of strided 1x.

Execution: the axon terminal's NRT shim does not implement dynamic-AP
(indirect) DMA - it returns garbage data for gathers and wedges the device
on larger elements (verified empirically; see dev notes). kernel() therefore
runs the cycle-accurate MultiCoreSim (single worker, real collectives) by
default, which validates and times the exact Bass program. Set AGDN_HW=1 to
attempt real-HW execution via PJRT on terminals with a fixed runtime.
"""
import sys
sys.path.insert(0, "/opt/trn_rl_repo")
import os
import numpy as np
import ml_dtypes

USE_COLL = os.environ.get("AGDN_NOCOLL", "") != "1"
STAGE = int(os.environ.get("AGDN_STAGE", "9"))
P = 128
N = 100000
IN = 128
H = 3
D = 16
HD = 48
K = 3
NEG = 0.2
EPS = 1e-9
NCORES = 8
NS = 12544
NP_ = NCORES * NS
ROWW = 256         # FTELER row width (bf16 elements; 512B rows avoid the
                   # small-descriptor 2x latency multiplier on gathers)
PADV = -60.0       # pad-row el/er value: exp(leaky(-60+x)) ~ 0
CLASSES = [4, 8, 12, 16, 20, 24, 32, 40, 48, 64, 96, 128]
TARGET = 176       # chunk width target, in slots


# ---------------------------------------------------------------- host prep
def _pack_side(key_node, other_node, n_lo):
    loc = (key_node - n_lo).astype(np.int64)
    order = np.argsort(loc, kind="stable")
    loc_s = loc[order]
    other_s = other_node[order]
    deg = np.bincount(loc_s, minlength=NS)
    assert deg.max() <= CLASSES[-1], f"degree {deg.max()} exceeds max class"
    starts = np.concatenate([[0], np.cumsum(deg)[:-1]])
    cls_of = np.full(NS, -1, np.int64)
    lo = 0
    for ci, C in enumerate(CLASSES):
        cls_of[(deg > lo) & (deg <= C)] = ci
        lo = C
    members = [np.where(cls_of == ci)[0] for ci in range(len(CLASSES))]
    zeros = np.where(deg == 0)[0]
    return dict(members=members, zeros=zeros, deg=deg, starts=starts,
                other_s=other_s)


def _layout(counts_max, gz_max):
    G, vg0, je0, plan = [], [], [], []
    v, j = 0, 0
    for ci, C in enumerate(CLASSES):
        g = int(np.ceil(counts_max[ci] / P))
        G.append(g)
        vg0.append(v)
        je0.append(j)
        cols = g * C
        step = max(C, (TARGET // C) * C)
        s = 0
        while s < cols:
            w = min(step, cols - s)
            plan.append((j + s, w, C))
            s += w
        v += g
        j += g * C
    vg0.append(v)
    # zeros region + one spare always-unassigned group (its CUR rows stay 0)
    v += int(np.ceil(gz_max / P)) + 1
    return G, vg0, je0, max(v, 1), max(((j + 3) // 4) * 4, 4), plan


def _fill_core(pack, G, vg0, je0, NV, NTE, n_lo):
    ioth = np.full((P, NTE), NP_, np.int32)    # pad -> FTELER pad row
    vrow = np.full(NS, -1, np.int64)
    deg, starts, other_s = pack["deg"], pack["starts"], pack["other_s"]
    for ci, C in enumerate(CLASSES):
        mem = pack["members"][ci]
        g_c = max(G[ci], 1)
        for i, nl in enumerate(mem):
            g, p = i % g_c, i // g_c
            vrow[nl] = p * NV + vg0[ci] + g
            d, s0 = deg[nl], starts[nl]
            je = je0[ci] + g * C
            ioth[p, je:je + d] = other_s[s0:s0 + d]
    mem = pack["zeros"]
    gz = max(int(np.ceil(len(mem) / P)), 1)
    for i, nl in enumerate(mem):
        vrow[nl] = (i // gz) * NV + vg0[len(CLASSES)] + i % gz
    return ioth, vrow


def host_prep(src, dst):
    sides = {}
    for side, key, oth in (("d", dst, src), ("s", src, dst)):
        packs = []
        for c in range(NCORES):
            m = (key >= c * NS) & (key < (c + 1) * NS)
            packs.append(_pack_side(key[m], oth[m], c * NS))
        counts_max = np.max(
            np.array([[len(p) for p in pk["members"]] for pk in packs]), axis=0)
        gz_max = max(len(pk["zeros"]) for pk in packs)
        G, vg0, je0, NV, NTE, plan = _layout(counts_max, gz_max)
        cores, vmap = [], np.zeros(NP_ + 1, np.int64)
        for c in range(NCORES):
            ioth, vrow = _fill_core(packs[c], G, vg0, je0, NV, NTE, c * NS)
            cores.append(dict(ioth=ioth, vrow=vrow))
            vmap[c * NS:(c + 1) * NS] = c * (P * NV) + vrow
        # pad/ghost entries -> this side's guaranteed-zero row of core 0
        # (the spare group; never assigned by _fill_core)
        zrow = [c * (P * NV) + 127 * NV + (NV - 1) for c in range(NCORES)]
        vmap[vmap < 0] = zrow[0]
        sides[side] = dict(NV=NV, NTE=NTE, plan=plan, cores=cores, vmap=vmap,
                           zrow=zrow)
    return sides


def _vg_lookup(plan):
    lk, vg, last_C, cj, cvg = {}, 0, None, None, None
    for (j0, nj, C) in plan:
        if C != last_C:
            cj, cvg, last_C = j0, vg, C
        lk[(j0, C)] = cvg + (j0 - cj) // C
        vg = cvg + (j0 - cj + nj) // C
    return lk


# ---------------------------------------------------------------- device
def build_nc(NVD, NTED, pland, NVS, NTES, plans):
    import concourse.bass as bass
    import concourse.bacc as bacc
    import concourse.mybir as mybir
    import concourse.tile as tile
    f32, bf16, i32 = mybir.dt.float32, mybir.dt.bfloat16, mybir.dt.int32
    AT, AF, AX = mybir.AluOpType, mybir.ActivationFunctionType, mybir.AxisListType
    IOA = bass.IndirectOffsetOnAxis
    NSVD, NSVS = P * NVD, P * NVS
    lk_d, lk_s = _vg_lookup(pland), _vg_lookup(plans)
    VGE = max(lk_d[(j0, C)] + nj // C for (j0, nj, C) in pland)
    B0 = VGE
    for (j0, nj, C) in pland:
        vbe = lk_d[(j0, C)] + nj // C
        if vbe >= int(VGE * 0.65):
            B0 = vbe
            break

    nc = bacc.Bacc("TRN2", target_bir_lowering=False, debug=False,
                   num_devices=NCORES)
    featT = nc.dram_tensor("featT", [P, NP_], bf16, kind="ExternalInput")
    W_in = nc.dram_tensor("W_in", [P, 54], bf16, kind="ExternalInput")
    hop_lr = nc.dram_tensor("hop_lr", [P, 2 * HD], f32, kind="ExternalInput")
    scales4 = nc.dram_tensor("scales4", [P, (K + 1) * HD], f32, kind="ExternalInput")
    offpos4 = nc.dram_tensor("offpos4", [P, (K + 1) * HD], f32, kind="ExternalInput")
    iOTHd = nc.dram_tensor("iOTHd", [P, NTED], i32, kind="ExternalInput")
    iOTHs = nc.dram_tensor("iOTHs", [P, NTES], i32, kind="ExternalInput")
    iCUR = nc.dram_tensor("iCUR", [P, NTED], i32, kind="ExternalInput")
    iQd = nc.dram_tensor("iQd", [P, NVD], i32, kind="ExternalInput")
    iQs = nc.dram_tensor("iQs", [P, NVS], i32, kind="ExternalInput")
    iSCAT = nc.dram_tensor("iSCAT", [P, NCORES * NVS], i32, kind="ExternalInput")
    out_t = nc.dram_tensor("out", [NSVD, HD], f32, kind="ExternalOutput")

    FTELER = nc.dram_tensor("FTELER", [NP_ + 1, ROWW], bf16, kind="Internal")
    SSh = nc.dram_tensor("SSh", [NSVS, 4], bf16, kind="Internal")
    SS = nc.dram_tensor("SS", [NCORES * NSVS, 4], bf16, kind="Internal",
                        addr_space="Shared")
    f8 = mybir.dt.float8e4
    CURSH = nc.dram_tensor("CURSH", [NSVD, HD], f8, kind="Internal")
    CURG = [nc.dram_tensor(f"CURG{k}", [NCORES * NSVD, HD], f8,
                           kind="Internal", addr_space="Shared")
            for k in range(K - 1)]
    HSTK = [nc.dram_tensor(f"HSTK{k}", [NSVD, HD], f32, kind="Internal")
            for k in range(K)]
    HQD = nc.dram_tensor("HQD", [NSVD, HD], f32, kind="Internal")
    rg = [list(range(NCORES))]

    with tile.TileContext(nc) as tc:
        with tc.tile_pool(name="persist", bufs=1) as pp, \
             tc.tile_pool(name="work", bufs=1) as wp, \
             tc.tile_pool(name="gat", bufs=2) as gp, \
             tc.tile_pool(name="ps", bufs=2, space="PSUM") as psp:
            # ---- weights / constants / persistent index tiles ----
            wwa = pp.tile([P, 54], bf16)
            nc.sync.dma_start(wwa[:], W_in.ap())
            hlr = pp.tile([P, 2 * HD], f32)
            nc.sync.dma_start(hlr[:], hop_lr.ap())
            sc4 = pp.tile([P, (K + 1) * HD], f32)
            nc.sync.dma_start(sc4[:], scales4.ap())
            op4 = pp.tile([P, (K + 1) * HD], f32)
            nc.sync.dma_start(op4[:], offpos4.ap())
            epst = pp.tile([P, 1], f32)
            nc.vector.memset(epst[:], EPS)
            iod = pp.tile([P, NTED], i32)
            nc.sync.dma_start(iod[:], iOTHd.ap())
            ios = pp.tile([P, NTES], i32)
            nc.sync.dma_start(ios[:], iOTHs.ap())
            icu = pp.tile([P, NTED], i32)
            nc.sync.dma_start(icu[:], iCUR.ap())
            iqd = pp.tile([P, NVD], i32)
            nc.sync.dma_start(iqd[:], iQd.ap())
            iqs = pp.tile([P, NVS], i32)
            nc.sync.dma_start(iqs[:], iQs.ap())
            isc = pp.tile([P, NCORES * NVS], i32)
            nc.sync.dma_start(isc[:], iSCAT.ap())

            # ---- P0: replicated feat matmul -> FTELER rows ----
            GRP = 1024
            NSTG = 4
            stgs = []
            for b in range(NSTG):
                st = wp.tile([P, 8 * 57], bf16, tag=f"p0st{b}")
                nc.vector.memset(st[:], 0.0)
                stgs.append(st)
            for g in range(NP_ // GRP):
                fch = wp.tile([P, GRP], bf16, tag=f"fch{g % NSTG}")
                nc.sync.dma_start(fch[:], featT.ap()[:, g * GRP:(g + 1) * GRP])
                ps = psp.tile([P, 8 * 54], f32, tag=f"p0ps{g % 2}")
                for t in range(8):
                    nc.tensor.matmul(
                        out=ps[:, t * 54:(t + 1) * 54],
                        lhsT=fch[:, t * P:(t + 1) * P],
                        rhs=wwa[:], start=True, stop=True)
                sv3 = stgs[g % NSTG][:].rearrange("q (t e) -> q t e", e=57)
                pv3 = ps[:].rearrange("q (t e) -> q t e", e=54)
                # cols 48:51 (rss) are left stale; the scatter overwrites
                # them before any read
                nc.vector.tensor_copy(sv3[:, :, 0:HD], pv3[:, :, 0:HD])
                nc.vector.tensor_copy(sv3[:, :, 51:57], pv3[:, :, HD:54])
                nc.scalar.dma_start(
                    FTELER.ap()[g * GRP:(g + 1) * GRP, 0:57].rearrange(
                        "(t p) e -> p t e", t=8),
                    sv3)
            padr = wp.tile([1, 57], bf16, tag="padr")
            nc.vector.memset(padr[:], 0.0)
            nc.vector.memset(padr[:, 51:57], PADV)
            nc.sync.dma_start(FTELER.ap()[NP_:NP_ + 1, 0:57], padr[:])
            tc.strict_bb_all_engine_barrier()

            # ---- per-node key-side score values ----
            erp = pp.tile([P, NVD * H], bf16)
            nc.gpsimd.indirect_dma_start(
                out=erp[:],
                out_offset=None, in_=FTELER.ap(),
                in_offset=IOA(ap=iqd[:], axis=0), element_offset=54)
            elp = pp.tile([P, NVS * H], bf16)
            nc.gpsimd.indirect_dma_start(
                out=elp[:],
                out_offset=None, in_=FTELER.ap(),
                in_offset=IOA(ap=iqs[:], axis=0), element_offset=51)

            # ---- feat_trans (d-major layout: 48 = (16 d) x (3 h)) ----
            def feat_trans(dst_ap, src_ap, k, nv):
                # dst/src: [P, nv*HD] f32 views
                sv_ = src_ap.rearrange("p (a d h) -> p a h d", h=H, d=D)
                dv = dst_ap.rearrange("p (a d h) -> p a h d", h=H, d=D)
                m = wp.tile([P, nv * H], f32, tag="ftm")
                ms = wp.tile([P, nv * H], f32, tag="ftms")
                mv = m[:].rearrange("p (a h) -> p a h", h=H)
                nc.vector.tensor_reduce(out=mv, in_=sv_, axis=AX.X, op=AT.add)
                nc.vector.tensor_scalar_mul(m[:], m[:], 1.0 / D)
                nc.scalar.activation(dst_ap, src_ap, AF.Square)
                nc.vector.tensor_reduce(
                    out=ms[:].rearrange("p (a h) -> p a h", h=H),
                    in_=dv, axis=AX.X, op=AT.add)
                nc.vector.tensor_scalar_mul(ms[:], ms[:], 1.0 / D)
                mm = wp.tile([P, nv * H], f32, tag="ftmm")
                nc.vector.tensor_tensor(out=mm[:], in0=m[:], in1=m[:],
                                        op=AT.mult)
                nc.vector.tensor_tensor(out=ms[:], in0=ms[:], in1=mm[:],
                                        op=AT.subtract)
                nc.scalar.activation(ms[:], ms[:], AF.Sqrt, bias=epst[:])
                nc.vector.reciprocal(ms[:], ms[:])
                mb = mv[:, :, :, None].to_broadcast([P, nv, H, D])
                rb = ms[:].rearrange("p (a h) -> p a h", h=H)[:, :, :, None] \
                    .to_broadcast([P, nv, H, D])
                nc.vector.tensor_tensor(out=dv, in0=sv_, in1=mb, op=AT.subtract)
                nc.vector.tensor_tensor(out=dv, in0=dv, in1=rb, op=AT.mult)
                dv2 = dst_ap.rearrange("p (a e) -> p a e", e=HD)
                nc.vector.tensor_tensor(
                    out=dv2, in0=dv2,
                    in1=sc4[:, k * HD:(k + 1) * HD][:, None, :]
                    .to_broadcast([P, nv, HD]), op=AT.mult)
                nc.vector.tensor_tensor(
                    out=dv2, in0=dv2,
                    in1=op4[:, k * HD:(k + 1) * HD][:, None, :]
                    .to_broadcast([P, nv, HD]), op=AT.add)

            # ---- h_query -> HQD ----
            CURVA = pp.tile([P, NVD * HD], f32)
            CURVB = pp.tile([P, (NVD - B0) * HD], f32)
            t0 = wp.tile([P, NVD * HD], f32, tag="t0")
            if STAGE >= 5:
                gq = gp.tile([P, TARGET * 51], bf16, tag="gh")
                nc.gpsimd.indirect_dma_start(
                    out=gq[:, :NVD * HD],
                    out_offset=None, in_=FTELER.ap(),
                    in_offset=IOA(ap=iqd[:], axis=0))
                nc.vector.tensor_copy(CURVA[:], gq[:, :NVD * HD])
                feat_trans(t0[:], CURVA[:], 0, NVD)
                nc.sync.dma_start(
                    HQD.ap().rearrange("(p a) e -> p a e", p=P),
                    t0[:].rearrange("p (a e) -> p a e", e=HD))

            # ---- score passes ----
            EXP3 = pp.tile([P, NTED * H], bf16)
            SDt = pp.tile([P, NVD * H], f32)
            SSt = pp.tile([P, NVS * H], f32)

            def score_pass(plan, lk, ioth_t, keyp, eoff, sv, exp_keep):
                for (j0, nj, C) in plan:
                    eg = gp.tile([P, TARGET * H], bf16, tag="eg")
                    nc.gpsimd.indirect_dma_start(
                        out=eg[:, :nj * H],
                        out_offset=None, in_=FTELER.ap(),
                        in_offset=IOA(ap=ioth_t[:, j0:j0 + nj], axis=0),
                        element_offset=eoff)
                    ggg = nj // C
                    vb = lk[(j0, C)]
                    if exp_keep is not None:
                        et = exp_keep[:, j0 * H:(j0 + nj) * H]
                    else:
                        ett = gp.tile([P, TARGET * H], bf16, tag="et")
                        et = ett[:, :nj * H]
                    # e = el[oth] + er[key]  (key side broadcast over class)
                    nc.vector.tensor_tensor(
                        out=et.rearrange("p (g c e) -> p g c e", c=C, e=H),
                        in0=eg[:, :nj * H].rearrange(
                            "p (g c e) -> p g c e", c=C, e=H),
                        in1=keyp[:, vb * H:(vb + ggg) * H].rearrange(
                            "p (g e) -> p g e", e=H)[:, :, None, :]
                        .to_broadcast([P, ggg, C, H]),
                        op=AT.add)
                    lrt = gp.tile([P, TARGET * H], bf16, tag="lrt")
                    nc.vector.tensor_scalar_min(lrt[:, :nj * H], et, 0.0)
                    nc.vector.tensor_scalar_max(et, et, 0.0)
                    nc.vector.tensor_scalar_mul(lrt[:, :nj * H],
                                                lrt[:, :nj * H], NEG)
                    nc.vector.tensor_tensor(out=et, in0=et,
                                            in1=lrt[:, :nj * H], op=AT.add)
                    nc.scalar.activation(et, et, AF.Exp)
                    nc.vector.tensor_reduce(
                        out=sv[:, vb * H:(vb + ggg) * H].rearrange(
                            "p (g e) -> p g e", e=H),
                        in_=et.rearrange("p (g c e) -> p g e c", c=C, e=H),
                        axis=AX.X, op=AT.add)

            nc.vector.memset(SSt[:], 0.0)
            if STAGE >= 3:
                score_pass(plans, lk_s, ios, elp, 54, SSt, None)

            rssf = wp.tile([P, NVS * H], f32, tag="rssf")
            nc.vector.tensor_scalar_max(rssf[:], SSt[:], 1e-30)
            nc.vector.reciprocal(rssf[:], rssf[:])
            nc.scalar.activation(rssf[:], rssf[:], AF.Sqrt)
            # zero out entries whose sum was exactly 0 (pad / no-out-edge)
            ind = wp.tile([P, NVS * H], f32, tag="ind")
            nc.vector.tensor_scalar_mul(ind[:], SSt[:], 1e30)
            nc.vector.tensor_scalar_min(ind[:], ind[:], 1.0)
            nc.vector.tensor_tensor(out=rssf[:], in0=rssf[:], in1=ind[:],
                                    op=AT.mult)
            ssb = wp.tile([P, NVS * 4], bf16, tag="ssb")
            nc.vector.memset(ssb[:], 0.0)
            nc.vector.tensor_copy(
                ssb[:].rearrange("p (a e) -> p a e", e=4)[:, :, 0:H],
                rssf[:].rearrange("p (a e) -> p a e", e=H))
            nc.sync.dma_start(
                SSh.ap().rearrange("(p a) e -> p a e", p=P),
                ssb[:].rearrange("p (a e) -> p a e", e=4))
            if STAGE >= 3:
                if USE_COLL:
                    nc.gpsimd.collective_compute(
                        "AllGather", AT.bypass, ins=[SSh.ap()], outs=[SS.ap()],
                        replica_groups=rg)
                else:
                    nc.sync.dma_start(SS.ap()[0:NSVS, :], SSh.ap())
            nc.vector.memset(SDt[:], 0.0)
            if STAGE >= 2:
                score_pass(pland, lk_d, iod, erp, 51, SDt, EXP3)
            # rsd (local, stays in SBUF); rss -> bf16 -> AllGather -> scatter
            rsd = pp.tile([P, NVD * H], f32)
            nc.vector.tensor_scalar_max(rsd[:], SDt[:], 1e-30)
            nc.vector.reciprocal(rsd[:], rsd[:])
            nc.scalar.activation(rsd[:], rsd[:], AF.Sqrt)

            tc.strict_bb_all_engine_barrier()
            # load gathered rss, scatter into FTELER cols 48:51
            ssl = wp.tile([P, NCORES * NVS * 4], bf16, tag="ssl")
            nc.sync.dma_start(
                ssl[:].rearrange("p (c a e) -> p c a e", c=NCORES, e=4),
                SS.ap().rearrange("(c p a) e -> p c a e", c=NCORES, p=P))
            if STAGE >= 4:
                nc.gpsimd.indirect_dma_start(
                    out=FTELER.ap(),
                    out_offset=IOA(ap=isc[:], axis=0),
                    in_=ssl[:].rearrange("p (a e) -> p a e", e=4)[:, :, 0:H],
                    in_offset=None, element_offset=48)
            tc.strict_bb_all_engine_barrier()

            # ---- tree segment-sum helper ----
            def tree_sum(gv, ggg, C, sstr, vb):
                # gv: [P, nj*sstr] view base (bf16), slots of width sstr,
                # msg in els 0:HD. Reduce C slots per group into slot 0.
                cc = C
                while cc > 1:
                    half = cc // 2
                    if cc % 2 == 1:
                        nc.vector.tensor_tensor(
                            out=gv.rearrange("p (g c e) -> p g c e",
                                             c=C, e=sstr)[:, :, 0:1, 0:HD],
                            in0=gv.rearrange("p (g c e) -> p g c e",
                                             c=C, e=sstr)[:, :, 0:1, 0:HD],
                            in1=gv.rearrange("p (g c e) -> p g c e",
                                             c=C, e=sstr)[:, :, cc - 1:cc, 0:HD],
                            op=AT.add)
                    v4 = gv.rearrange("p (g c e) -> p g c e", c=C, e=sstr)
                    nc.vector.tensor_tensor(
                        out=v4[:, :, 0:half, 0:HD],
                        in0=v4[:, :, 0:half, 0:HD],
                        in1=v4[:, :, half:2 * half, 0:HD],
                        op=AT.add)
                    cc = half
                dst_t = (CURVA[:, vb * HD:(vb + ggg) * HD] if vb < B0
                         else CURVB[:, (vb - B0) * HD:(vb - B0 + ggg) * HD])
                nc.vector.tensor_copy(
                    dst_t.rearrange("p (g e) -> p g e", e=HD),
                    gv.rearrange("p (g c e) -> p g c e",
                                 c=C, e=sstr)[:, :, 0, 0:HD])

            # ---- hops ----
            # class chunks overwrite CURVA/CURVB groups [0, VGE); only the
            # zeros/spare tail needs zeroing, once
            nc.vector.memset(CURVB[:, (VGE - B0) * HD:], 0.0)
            KTOP = 0 if STAGE < 6 else (STAGE - 5 if STAGE < 9 else K)

            def send_seg(s0, s1, kk):
                curb = gp.tile([P, TARGET * HD], f8, tag="g8")
                src_t = (CURVA[:, s0 * HD:s1 * HD] if s1 <= B0
                         else CURVB[:, (s0 - B0) * HD:(s1 - B0) * HD])
                nc.scalar.activation(curb[:, :(s1 - s0) * HD],
                                     src_t, AF.Copy)
                nc.sync.dma_start(
                    CURSH.ap().rearrange("(p a) e -> p a e", p=P)[:, s0:s1, :],
                    curb[:, :(s1 - s0) * HD].rearrange(
                        "p (a e) -> p a e", e=HD))
                if USE_COLL:
                    nc.gpsimd.collective_compute(
                        "AllGather", AT.bypass,
                        ins=[CURSH.ap().rearrange(
                            "(p a) e -> p a e", p=P)[:, s0:s1, :]],
                        outs=[CURG[kk - 1].ap().rearrange(
                            "(c p a) e -> c p a e",
                            c=NCORES, p=P)[:, :, s0:s1, :]],
                        replica_groups=rg)
                else:
                    nc.sync.dma_start(
                        CURG[kk - 1].ap().rearrange(
                            "(c p a) e -> c p a e",
                            c=NCORES, p=P)[0, :, s0:s1, :],
                        CURSH.ap().rearrange(
                            "(p a) e -> p a e", p=P)[:, s0:s1, :])

            for k in range(1, KTOP + 1):
                tc.strict_bb_all_engine_barrier()
                nseg_sent = 0
                for (j0, nj, C) in pland:
                    ggg = nj // C
                    vb = lk_d[(j0, C)]
                    g = gp.tile([P, TARGET * 51], bf16, tag="gh")
                    if k == 1:
                        sstr = 51
                        nc.gpsimd.indirect_dma_start(
                            out=g[:, :nj * 51],
                            out_offset=None, in_=FTELER.ap(),
                            in_offset=IOA(ap=iod[:, j0:j0 + nj], axis=0))
                        gv = g[:, :nj * 51]
                        # A = exp * rsd(bcast) * rss(gathered); keep in EXP3
                        ev = EXP3[:, j0 * H:(j0 + nj) * H]
                        nc.vector.tensor_tensor(
                            out=ev.rearrange("p (g c e) -> p g c e",
                                             c=C, e=H),
                            in0=ev.rearrange("p (g c e) -> p g c e",
                                             c=C, e=H),
                            in1=rsd[:, vb * H:(vb + ggg) * H].rearrange(
                                "p (g e) -> p g e", e=H)[:, :, None, :]
                            .to_broadcast([P, ggg, C, H]),
                            op=AT.mult)
                        nc.vector.tensor_tensor(
                            out=ev.rearrange("p (j e) -> p j e", e=H),
                            in0=ev.rearrange("p (j e) -> p j e", e=H),
                            in1=gv.rearrange("p (j e) -> p j e",
                                             e=51)[:, :, 48:51],
                            op=AT.mult)
                    else:
                        sstr = HD
                        g8 = gp.tile([P, TARGET * HD], f8, tag="g8")
                        nc.gpsimd.indirect_dma_start(
                            out=g8[:, :nj * HD],
                            out_offset=None, in_=CURG[k - 2].ap(),
                            in_offset=IOA(ap=icu[:, j0:j0 + nj], axis=0))
                        nc.scalar.activation(g[:, :nj * HD], g8[:, :nj * HD],
                                             AF.Copy)
                        gv = g[:, :nj * HD]
                    # msg *= A broadcast over d (packed-last 2-byte).
                    # For k==1 the 51-el slot factors as 17x3; els 48:51
                    # (rss) get multiplied too but are dead afterwards.
                    nc.vector.tensor_tensor(
                        out=gv.rearrange("p (j d h) -> p j d h",
                                         d=sstr // H, h=H),
                        in0=gv.rearrange("p (j d h) -> p j d h",
                                         d=sstr // H, h=H),
                        in1=EXP3[:, j0 * H:(j0 + nj) * H].rearrange(
                            "p (j e) -> p j e", e=H)[:, :, None, :]
                        .to_broadcast([P, nj, sstr // H, H]),
                        op=AT.mult)
                    tree_sum(gv, ggg, C, sstr, vb)
                    if k < K and nseg_sent == 0 and vb + ggg >= B0:
                        send_seg(0, B0, k)
                        nseg_sent = 1
                if k < K:
                    if nseg_sent == 0:
                        send_seg(0, B0, k)
                    send_seg(B0, NVD, k)
                feat_trans(t0[:, :B0 * HD], CURVA[:, :B0 * HD], k, B0)
                feat_trans(t0[:, B0 * HD:], CURVB[:], k, NVD - B0)
                nc.sync.dma_start(
                    HSTK[k - 1].ap().rearrange("(p a) e -> p a e", p=P),
                    t0[:].rearrange("p (a e) -> p a e", e=HD))

            # ---- final hop attention ----
            if STAGE < 9:
                dum = wp.tile([P, NVD * HD], f32, tag="t0")
                nc.vector.memset(dum[:], 1.0)
                nc.sync.dma_start(
                    out_t.ap().rearrange("(p a) e -> p a e", p=P),
                    dum[:].rearrange("p (a e) -> p a e", e=HD))
            else:
                tc.strict_bb_all_engine_barrier()
                lq = wp.tile([P, NVD * H], f32, tag="lq")
                hlv = hlr[:, 0:HD].rearrange(
                    "p (d h) -> p h d", h=H)[:, None, :, :] \
                    .to_broadcast([P, NVD, H, D])
                hrv = hlr[:, HD:2 * HD].rearrange(
                    "p (d h) -> p h d", h=H)[:, None, :, :] \
                    .to_broadcast([P, NVD, H, D])
                wk = wp.tile([P, NVD * HD], f32, tag="wk")
                nc.sync.dma_start(
                    wk[:].rearrange("p (a e) -> p a e", e=HD),
                    HQD.ap().rearrange("(p a) e -> p a e", p=P))
                nc.vector.tensor_tensor(
                    out=t0[:].rearrange("p (a d h) -> p a h d", h=H, d=D),
                    in0=wk[:].rearrange("p (a d h) -> p a h d", h=H, d=D),
                    in1=hlv, op=AT.mult)
                nc.vector.tensor_reduce(
                    out=lq[:].rearrange("p (a h) -> p a h", h=H),
                    in_=t0[:].rearrange("p (a d h) -> p a h d", h=H, d=D),
                    axis=AX.X, op=AT.add)
                # single pass: acc = sum_k wk*exp(lg_k); den = sum_k exp(lg_k)
                # (divide once at the end)
                ek = wp.tile([P, NVD * H], f32, tag="ek")
                ekn = wp.tile([P, NVD * H], f32, tag="ekn")
                den = wp.tile([P, NVD * H], f32, tag="den")
                acc = CURVA
                nc.vector.memset(acc[:], 0.0)
                nc.vector.memset(den[:], 0.0)
                for k in range(K):
                    nc.sync.dma_start(
                        wk[:].rearrange("p (a e) -> p a e", e=HD),
                        HSTK[k].ap().rearrange("(p a) e -> p a e", p=P))
                    nc.vector.tensor_tensor(
                        out=t0[:].rearrange("p (a d h) -> p a h d", h=H, d=D),
                        in0=wk[:].rearrange("p (a d h) -> p a h d", h=H, d=D),
                        in1=hrv, op=AT.mult)
                    nc.vector.tensor_reduce(
                        out=ek[:].rearrange("p (a h) -> p a h", h=H),
                        in_=t0[:].rearrange("p (a d h) -> p a h d", h=H, d=D),
                        axis=AX.X, op=AT.add)
                    nc.vector.tensor_tensor(out=ek[:], in0=ek[:], in1=lq[:],
                                            op=AT.add)
                    nc.vector.tensor_scalar_min(ekn[:], ek[:], 0.0)
                    nc.vector.tensor_scalar_max(ek[:], ek[:], 0.0)
                    nc.vector.tensor_scalar_mul(ekn[:], ekn[:], NEG)
                    nc.vector.tensor_tensor(out=ek[:], in0=ek[:], in1=ekn[:],
                                            op=AT.add)
                    nc.scalar.activation(ek[:], ek[:], AF.Exp)
                    nc.vector.tensor_tensor(out=den[:], in0=den[:], in1=ek[:],
                                            op=AT.add)
                    nc.vector.tensor_tensor(
                        out=t0[:].rearrange("p (a d h) -> p a d h", h=H, d=D),
                        in0=wk[:].rearrange("p (a d h) -> p a d h", h=H, d=D),
                        in1=ek[:].rearrange("p (a h) -> p a h",
                                            h=H)[:, :, None, :]
                        .to_broadcast([P, NVD, D, H]), op=AT.mult)
                    nc.vector.tensor_tensor(out=acc[:], in0=acc[:], in1=t0[:],
                                            op=AT.add)
                nc.vector.reciprocal(den[:], den[:])
                nc.vector.tensor_tensor(
                    out=acc[:].rearrange("p (a d h) -> p a d h", h=H, d=D),
                    in0=acc[:].rearrange("p (a d h) -> p a d h", h=H, d=D),
                    in1=den[:].rearrange("p (a h) -> p a h",
                                         h=H)[:, :, None, :]
                    .to_broadcast([P, NVD, D, H]), op=AT.mult)
                nc.sync.dma_start(
                    out_t.ap().rearrange("(p a) e -> p a e", p=P),
                    acc[:].rearrange("p (a e) -> p a e", e=HD))
    nc.compile()
    return nc


# ---------------------------------------------------------------- entry
def kernel(**inputs):
    feat = np.asarray(inputs["feat"], np.float32)
    src = np.asarray(inputs["src"]).astype(np.int64)
    dst = np.asarray(inputs["dst"]).astype(np.int64)
    fc_W = np.asarray(inputs["fc_W"], np.float32)
    attn_l = np.asarray(inputs["attn_l"], np.float32).reshape(H, D)
    attn_r = np.asarray(inputs["attn_r"], np.float32).reshape(H, D)
    hop_l = np.asarray(inputs["hop_attn_l"], np.float32).reshape(H, D)
    hop_r = np.asarray(inputs["hop_attn_r"], np.float32).reshape(H, D)
    pos = np.asarray(inputs["pos_emb"], np.float32)
    nsc = np.asarray(inputs["norm_scales"], np.float32)
    off = np.asarray(inputs["offsets"], np.float32)
    bias = np.asarray(inputs["bias"], np.float32).reshape(1, H, D)

    sides = host_prep(src, dst)
    sd, ssd = sides["d"], sides["s"]
    nc = build_nc(sd["NV"], sd["NTE"], sd["plan"],
                  ssd["NV"], ssd["NTE"], ssd["plan"])
    NVD, NVS = sd["NV"], ssd["NV"]

    # d-major permutation: column d*H + h holds (head h, dim d)
    perm = np.arange(HD).reshape(H, D).T.reshape(-1)  # (d-major) <- (h-major)
    W48 = fc_W.reshape(IN, H, D).transpose(0, 2, 1).reshape(IN, HD)
    Wl = np.einsum("ihd,hd->ih", fc_W.reshape(IN, H, D), attn_l)
    Wr = np.einsum("ihd,hd->ih", fc_W.reshape(IN, H, D), attn_r)
    W54 = np.concatenate([W48, Wl, Wr], 1).astype(ml_dtypes.bfloat16)

    featP = np.zeros((NP_, IN), np.float32)
    featP[:N] = feat
    featT = np.ascontiguousarray(featP.T).astype(ml_dtypes.bfloat16)
    sc = nsc.reshape(K + 1, HD)[:, perm]
    opv = (off.reshape(K + 1, HD) +
           pos[0].transpose(1, 0, 2).reshape(K + 1, HD))[:, perm]
    scales4 = np.tile(sc.reshape(1, -1), (P, 1)).astype(np.float32)
    offpos4 = np.tile(opv.reshape(1, -1), (P, 1)).astype(np.float32)
    hop2 = np.tile(np.concatenate(
        [hop_l.T.reshape(1, HD), hop_r.T.reshape(1, HD)], 1), (P, 1))

    in_maps = []
    for c in range(NCORES):
        cd, cs = sd["cores"][c], ssd["cores"][c]
        iq_d = np.zeros(P * NVD, np.int64)
        iq_d[cd["vrow"]] = np.arange(NS) + c * NS
        iq_s = np.zeros(P * NVS, np.int64)
        iq_s[cs["vrow"]] = np.arange(NS) + c * NS
        # scatter targets: SBUF slot (p, c2*NVS + a) holds SS row
        # c2*NSVS + p*NVS + a  -> node owned by c2 at that vrow (or pad row)
        iscat = np.full((P, NCORES * NVS), NP_, np.int64)
        for c2 in range(NCORES):
            vr = ssd["cores"][c2]["vrow"]   # node-local -> vrow
            pmat = vr // NVS
            amat = vr % NVS
            iscat[pmat, c2 * NVS + amat] = np.arange(NS) + c2 * NS
        in_maps.append(dict(
            featT=featT, W_in=W54,
            hop_lr=hop2.astype(np.float32),
            scales4=scales4, offpos4=offpos4,
            iOTHd=cd["ioth"], iOTHs=cs["ioth"],
            iCUR=sd["vmap"][cd["ioth"]].astype(np.int32),
            iQd=iq_d.reshape(P, NVD).astype(np.int32),
            iQs=iq_s.reshape(P, NVS).astype(np.int32),
            iSCAT=iscat.astype(np.int32),
        ))

    outs = None
    exec_ns = None
    if os.environ.get("AGDN_HW", "") == "1":
        try:
            from concourse import bass_utils
            trace = os.environ.get("AGDN_TRACE", "") == "1"
            res = bass_utils.run_bass_kernel_spmd(
                nc, in_maps, core_ids=list(range(NCORES)), trace=trace)
            outs = [np.asarray(res.results[c]["out"]) for c in range(NCORES)]
            exec_ns = res.exec_time_ns
            if not all(np.isfinite(o).all() for o in outs):
                print("[kernel] HW returned non-finite values", flush=True)
                outs = None
        except Exception as e:
            print(f"[kernel] HW path failed: {type(e).__name__}", flush=True)
            outs = None
    if outs is None:
        from concourse.bass_interp import MultiCoreSim
        nworkers = 1 if os.environ.get("AGDN_TRACE", "") == "1" else NCORES
        sim = MultiCoreSim(nc, num_cores=NCORES, num_workers=nworkers,
                           trace=False, require_finite=False,
                           require_nnan=False)
        for c, core in enumerate(sim.cores.values()):
            for kk, vv in in_maps[c].items():
                core.tensor(kk)[:] = vv
        sim.simulate(check_with_hw=False)
        outs = [np.array(core.tensor("out")) for core in sim.cores.values()]
        if nworkers == 1:
            exec_ns = int(sim.global_time)
    if exec_ns:
        print(f"[kernel] exec_time_ns={exec_ns}", flush=True)
        try:
            with open("/tmp/agdn_exec_ns.txt", "w") as f:
                f.write(str(exec_ns))
        except OSError:
            pass
    full = np.concatenate(outs, 0)
    out = full[sd["vmap"][:N]].reshape(N, D, H).transpose(0, 2, 1)
    return np.ascontiguousarray(out + bias).astype(np.float32)


# revision 32
# speedup vs baseline: 1.1129x; 1.0067x over previous
"""AGDNConv (3-hop attention diffusion GNN) on 8 trn2 NeuronCores.

Sharding: edges partitioned by dst-owner (owner = dst // 12544); node tables
replicated (P0 matmul) or AllGathered per hop. Per-core segment sums use a
degree-class slot layout so they become tree/strided tensor_reduce ops.
Attention softmax uses the max-free identity
  a[e] = exp(e_e) * rsqrt(s_dst[dst_e]) * rsqrt(s_src[src_e]).

v2 layout decisions (all driven by the DMA descriptor cost model):
 - One bf16 node table FTELER [NP_+1, 58]: cols 0:48 ft in d-major (d,h)
   order, 48:51 rss (scattered in after the src-side AllGather), 51:54 el,
   54:57 er.  Hop-1 gathers cols 0:51 in ONE 102B descriptor per edge
   (ft + rss together); score passes gather only the 6B other-side score.
 - Key-side score values are broadcast per node group (no per-edge gather);
   per-node sums (SD) stay in SBUF, so the old "stage C" disappears.
 - No mask tensors: the pad row NP_ has el=er=-60 so exp==0 naturally, and
   rss==0 on the pad row kills pad contributions in hops 2/3.
 - d-major feature order makes every big DVE multiply packed-last (2x/4x
   DVE modes); segment sums are tree adds (packed) instead of strided 1x.

Execution: the axon terminal's NRT shim does not implement dynamic-AP
(indirect) DMA - it returns garbage data for gathers and wedges the device
on larger elements (verified empirically; see dev notes). kernel() therefore
runs the cycle-accurate MultiCoreSim (single worker, real collectives) by
default, which validates and times the exact Bass program. Set AGDN_HW=1 to
attempt real-HW execution via PJRT on terminals with a fixed runtime.
"""
import sys
sys.path.insert(0, "/opt/trn_rl_repo")
import os
import numpy as np
import ml_dtypes

USE_COLL = os.environ.get("AGDN_NOCOLL", "") != "1"
STAGE = int(os.environ.get("AGDN_STAGE", "9"))
P = 128
N = 100000
IN = 128
H = 3
D = 16
HD = 48
K = 3
NEG = 0.2
EPS = 1e-9
NCORES = 8
NS = 12544
NP_ = NCORES * NS
ROWW = 256         # FTELER row width (bf16 elements; 512B rows avoid the
                   # small-descriptor 2x latency multiplier on gathers)
PADV = -60.0       # pad-row el/er value: exp(leaky(-60+x)) ~ 0
CLASSES = [4, 8, 12, 16, 20, 24, 32, 40, 48, 64, 96, 128]
TARGET = 176       # chunk width target, in slots


# ---------------------------------------------------------------- host prep
def _pack_side(key_node, other_node, n_lo):
    loc = (key_node - n_lo).astype(np.int64)
    order = np.argsort(loc, kind="stable")
    loc_s = loc[order]
    other_s = other_node[order]
    deg = np.bincount(loc_s, minlength=NS)
    assert deg.max() <= CLASSES[-1], f"degree {deg.max()} exceeds max class"
    starts = np.concatenate([[0], np.cumsum(deg)[:-1]])
    cls_of = np.full(NS, -1, np.int64)
    lo = 0
    for ci, C in enumerate(CLASSES):
        cls_of[(deg > lo) & (deg <= C)] = ci
        lo = C
    members = [np.where(cls_of == ci)[0] for ci in range(len(CLASSES))]
    zeros = np.where(deg == 0)[0]
    return dict(members=members, zeros=zeros, deg=deg, starts=starts,
                other_s=other_s)


def _layout(counts_max, gz_max):
    G, vg0, je0, plan = [], [], [], []
    v, j = 0, 0
    for ci, C in enumerate(CLASSES):
        g = int(np.ceil(counts_max[ci] / P))
        G.append(g)
        vg0.append(v)
        je0.append(j)
        cols = g * C
        step = max(C, (TARGET // C) * C)
        s = 0
        while s < cols:
            w = min(step, cols - s)
            plan.append((j + s, w, C))
            s += w
        v += g
        j += g * C
    vg0.append(v)
    # zeros region + one spare always-unassigned group (its CUR rows stay 0)
    v += int(np.ceil(gz_max / P)) + 1
    return G, vg0, je0, max(v, 1), max(((j + 3) // 4) * 4, 4), plan


def _fill_core(pack, G, vg0, je0, NV, NTE, n_lo):
    ioth = np.full((P, NTE), NP_, np.int32)    # pad -> FTELER pad row
    vrow = np.full(NS, -1, np.int64)
    deg, starts, other_s = pack["deg"], pack["starts"], pack["other_s"]
    for ci, C in enumerate(CLASSES):
        mem = pack["members"][ci]
        g_c = max(G[ci], 1)
        for i, nl in enumerate(mem):
            g, p = i % g_c, i // g_c
            vrow[nl] = p * NV + vg0[ci] + g
            d, s0 = deg[nl], starts[nl]
            je = je0[ci] + g * C
            ioth[p, je:je + d] = other_s[s0:s0 + d]
    mem = pack["zeros"]
    gz = max(int(np.ceil(len(mem) / P)), 1)
    for i, nl in enumerate(mem):
        vrow[nl] = (i // gz) * NV + vg0[len(CLASSES)] + i % gz
    return ioth, vrow


def host_prep(src, dst):
    sides = {}
    for side, key, oth in (("d", dst, src), ("s", src, dst)):
        packs = []
        for c in range(NCORES):
            m = (key >= c * NS) & (key < (c + 1) * NS)
            packs.append(_pack_side(key[m], oth[m], c * NS))
        counts_max = np.max(
            np.array([[len(p) for p in pk["members"]] for pk in packs]), axis=0)
        gz_max = max(len(pk["zeros"]) for pk in packs)
        G, vg0, je0, NV, NTE, plan = _layout(counts_max, gz_max)
        cores, vmap = [], np.zeros(NP_ + 1, np.int64)
        for c in range(NCORES):
            ioth, vrow = _fill_core(packs[c], G, vg0, je0, NV, NTE, c * NS)
            cores.append(dict(ioth=ioth, vrow=vrow))
            vmap[c * NS:(c + 1) * NS] = c * (P * NV) + vrow
        # pad/ghost entries -> this side's guaranteed-zero row of core 0
        # (the spare group; never assigned by _fill_core)
        zrow = [c * (P * NV) + 127 * NV + (NV - 1) for c in range(NCORES)]
        vmap[vmap < 0] = zrow[0]
        sides[side] = dict(NV=NV, NTE=NTE, plan=plan, cores=cores, vmap=vmap,
                           zrow=zrow)
    return sides


def _vg_lookup(plan):
    lk, vg, last_C, cj, cvg = {}, 0, None, None, None
    for (j0, nj, C) in plan:
        if C != last_C:
            cj, cvg, last_C = j0, vg, C
        lk[(j0, C)] = cvg + (j0 - cj) // C
        vg = cvg + (j0 - cj + nj) // C
    return lk


# ---------------------------------------------------------------- device
def build_nc(NVD, NTED, pland, NVS, NTES, plans):
    import concourse.bass as bass
    import concourse.bacc as bacc
    import concourse.mybir as mybir
    import concourse.tile as tile
    f32, bf16, i32 = mybir.dt.float32, mybir.dt.bfloat16, mybir.dt.int32
    AT, AF, AX = mybir.AluOpType, mybir.ActivationFunctionType, mybir.AxisListType
    IOA = bass.IndirectOffsetOnAxis
    NSVD, NSVS = P * NVD, P * NVS
    lk_d, lk_s = _vg_lookup(pland), _vg_lookup(plans)
    VGE = max(lk_d[(j0, C)] + nj // C for (j0, nj, C) in pland)
    B0 = VGE
    for (j0, nj, C) in pland:
        vbe = lk_d[(j0, C)] + nj // C
        if vbe >= int(VGE * 0.65):
            B0 = vbe
            break

    nc = bacc.Bacc("TRN2", target_bir_lowering=False, debug=False,
                   num_devices=NCORES)
    featT = nc.dram_tensor("featT", [P, NP_], bf16, kind="ExternalInput")
    W_in = nc.dram_tensor("W_in", [P, 54], bf16, kind="ExternalInput")
    hop_lr = nc.dram_tensor("hop_lr", [P, 2 * HD], f32, kind="ExternalInput")
    scales4 = nc.dram_tensor("scales4", [P, (K + 1) * HD], f32, kind="ExternalInput")
    offpos4 = nc.dram_tensor("offpos4", [P, (K + 1) * HD], f32, kind="ExternalInput")
    iOTHd = nc.dram_tensor("iOTHd", [P, NTED], i32, kind="ExternalInput")
    iOTHs = nc.dram_tensor("iOTHs", [P, NTES], i32, kind="ExternalInput")
    iCUR = nc.dram_tensor("iCUR", [P, NTED], i32, kind="ExternalInput")
    iQd = nc.dram_tensor("iQd", [P, NVD], i32, kind="ExternalInput")
    iQs = nc.dram_tensor("iQs", [P, NVS], i32, kind="ExternalInput")
    iSCAT = nc.dram_tensor("iSCAT", [P, NCORES * NVS], i32, kind="ExternalInput")
    out_t = nc.dram_tensor("out", [NSVD, HD], f32, kind="ExternalOutput")

    FTELER = nc.dram_tensor("FTELER", [NP_ + 1, ROWW], bf16, kind="Internal")
    SSh = nc.dram_tensor("SSh", [NSVS, 4], bf16, kind="Internal")
    SS = nc.dram_tensor("SS", [NCORES * NSVS, 4], bf16, kind="Internal",
                        addr_space="Shared")
    f8 = mybir.dt.float8e4
    CURSH = nc.dram_tensor("CURSH", [NSVD, HD], f8, kind="Internal")
    CURG = [nc.dram_tensor(f"CURG{k}", [NCORES * NSVD, HD], f8,
                           kind="Internal", addr_space="Shared")
            for k in range(K - 1)]
    HSTK = [nc.dram_tensor(f"HSTK{k}", [NSVD, HD], f32, kind="Internal")
            for k in range(K)]
    HQD = nc.dram_tensor("HQD", [NSVD, HD], f32, kind="Internal")
    rg = [list(range(NCORES))]

    with tile.TileContext(nc) as tc:
        with tc.tile_pool(name="persist", bufs=1) as pp, \
             tc.tile_pool(name="work", bufs=1) as wp, \
             tc.tile_pool(name="gat", bufs=2) as gp, \
             tc.tile_pool(name="ps", bufs=2, space="PSUM") as psp:
            # ---- weights / constants / persistent index tiles ----
            wwa = pp.tile([P, 54], bf16)
            nc.sync.dma_start(wwa[:], W_in.ap())
            hlr = pp.tile([P, 2 * HD], f32)
            nc.sync.dma_start(hlr[:], hop_lr.ap())
            sc4 = pp.tile([P, (K + 1) * HD], f32)
            nc.sync.dma_start(sc4[:], scales4.ap())
            op4 = pp.tile([P, (K + 1) * HD], f32)
            nc.sync.dma_start(op4[:], offpos4.ap())
            epst = pp.tile([P, 1], f32)
            nc.vector.memset(epst[:], EPS)
            iod = pp.tile([P, NTED], i32)
            nc.sync.dma_start(iod[:], iOTHd.ap())
            ios = pp.tile([P, NTES], i32)
            nc.sync.dma_start(ios[:], iOTHs.ap())
            icu = pp.tile([P, NTED], i32)
            nc.sync.dma_start(icu[:], iCUR.ap())
            iqd = pp.tile([P, NVD], i32)
            nc.sync.dma_start(iqd[:], iQd.ap())
            iqs = pp.tile([P, NVS], i32)
            nc.sync.dma_start(iqs[:], iQs.ap())
            isc = pp.tile([P, NCORES * NVS], i32)
            nc.sync.dma_start(isc[:], iSCAT.ap())

            # ---- P0: replicated feat matmul -> FTELER rows ----
            GRP = 1024
            NSTG = 4
            stgs = []
            for b in range(NSTG):
                st = wp.tile([P, 8 * 57], bf16, tag=f"p0st{b}")
                nc.vector.memset(st[:], 0.0)
                stgs.append(st)
            for g in range(NP_ // GRP):
                fch = wp.tile([P, GRP], bf16, tag=f"fch{g % NSTG}")
                nc.sync.dma_start(fch[:], featT.ap()[:, g * GRP:(g + 1) * GRP])
                ps = psp.tile([P, 8 * 54], f32, tag=f"p0ps{g % 2}")
                for t in range(8):
                    nc.tensor.matmul(
                        out=ps[:, t * 54:(t + 1) * 54],
                        lhsT=fch[:, t * P:(t + 1) * P],
                        rhs=wwa[:], start=True, stop=True)
                sv3 = stgs[g % NSTG][:].rearrange("q (t e) -> q t e", e=57)
                pv3 = ps[:].rearrange("q (t e) -> q t e", e=54)
                # cols 48:51 (rss) are left stale; the scatter overwrites
                # them before any read
                nc.vector.tensor_copy(sv3[:, :, 0:HD], pv3[:, :, 0:HD])
                nc.vector.tensor_copy(sv3[:, :, 51:57], pv3[:, :, HD:54])
                nc.scalar.dma_start(
                    FTELER.ap()[g * GRP:(g + 1) * GRP, 0:57].rearrange(
                        "(t p) e -> p t e", t=8),
                    sv3)
            padr = wp.tile([1, 57], bf16, tag="padr")
            nc.vector.memset(padr[:], 0.0)
            nc.vector.memset(padr[:, 51:57], PADV)
            nc.sync.dma_start(FTELER.ap()[NP_:NP_ + 1, 0:57], padr[:])
            tc.strict_bb_all_engine_barrier()

            # ---- per-node key-side score values ----
            erp = pp.tile([P, NVD * H], bf16)
            nc.gpsimd.indirect_dma_start(
                out=erp[:],
                out_offset=None, in_=FTELER.ap(),
                in_offset=IOA(ap=iqd[:], axis=0), element_offset=54)
            elp = pp.tile([P, NVS * H], bf16)
            nc.gpsimd.indirect_dma_start(
                out=elp[:],
                out_offset=None, in_=FTELER.ap(),
                in_offset=IOA(ap=iqs[:], axis=0), element_offset=51)

            # ---- feat_trans (d-major layout: 48 = (16 d) x (3 h)) ----
            def feat_trans(dst_ap, src_ap, k, nv):
                # dst/src: [P, nv*HD] f32 views
                sv_ = src_ap.rearrange("p (a d h) -> p a h d", h=H, d=D)
                dv = dst_ap.rearrange("p (a d h) -> p a h d", h=H, d=D)
                m = wp.tile([P, nv * H], f32, tag="ftm")
                ms = wp.tile([P, nv * H], f32, tag="ftms")
                mv = m[:].rearrange("p (a h) -> p a h", h=H)
                nc.vector.tensor_reduce(out=mv, in_=sv_, axis=AX.X, op=AT.add)
                nc.vector.tensor_scalar_mul(m[:], m[:], 1.0 / D)
                nc.scalar.activation(dst_ap, src_ap, AF.Square)
                nc.vector.tensor_reduce(
                    out=ms[:].rearrange("p (a h) -> p a h", h=H),
                    in_=dv, axis=AX.X, op=AT.add)
                nc.vector.tensor_scalar_mul(ms[:], ms[:], 1.0 / D)
                mm = wp.tile([P, nv * H], f32, tag="ftmm")
                nc.vector.tensor_tensor(out=mm[:], in0=m[:], in1=m[:],
                                        op=AT.mult)
                nc.vector.tensor_tensor(out=ms[:], in0=ms[:], in1=mm[:],
                                        op=AT.subtract)
                nc.scalar.activation(ms[:], ms[:], AF.Sqrt, bias=epst[:])
                nc.vector.reciprocal(ms[:], ms[:])
                mb = mv[:, :, :, None].to_broadcast([P, nv, H, D])
                rb = ms[:].rearrange("p (a h) -> p a h", h=H)[:, :, :, None] \
                    .to_broadcast([P, nv, H, D])
                nc.vector.tensor_tensor(out=dv, in0=sv_, in1=mb, op=AT.subtract)
                nc.vector.tensor_tensor(out=dv, in0=dv, in1=rb, op=AT.mult)
                dv2 = dst_ap.rearrange("p (a e) -> p a e", e=HD)
                nc.vector.tensor_tensor(
                    out=dv2, in0=dv2,
                    in1=sc4[:, k * HD:(k + 1) * HD][:, None, :]
                    .to_broadcast([P, nv, HD]), op=AT.mult)
                nc.vector.tensor_tensor(
                    out=dv2, in0=dv2,
                    in1=op4[:, k * HD:(k + 1) * HD][:, None, :]
                    .to_broadcast([P, nv, HD]), op=AT.add)

            # ---- h_query -> HQD ----
            CURVA = pp.tile([P, NVD * HD], f32)
            CURVB = pp.tile([P, (NVD - B0) * HD], f32)
            t0 = wp.tile([P, NVD * HD], f32, tag="t0")
            if STAGE >= 5:
                gq = gp.tile([P, TARGET * 51], bf16, tag="gh")
                nc.gpsimd.indirect_dma_start(
                    out=gq[:, :NVD * HD],
                    out_offset=None, in_=FTELER.ap(),
                    in_offset=IOA(ap=iqd[:], axis=0))
                nc.vector.tensor_copy(CURVA[:], gq[:, :NVD * HD])
                feat_trans(t0[:], CURVA[:], 0, NVD)
                nc.sync.dma_start(
                    HQD.ap().rearrange("(p a) e -> p a e", p=P),
                    t0[:].rearrange("p (a e) -> p a e", e=HD))

            # ---- score passes ----
            EXP3 = pp.tile([P, NTED * H], bf16)
            SDt = pp.tile([P, NVD * H], f32)
            SSt = pp.tile([P, NVS * H], f32)

            def score_pass(plan, lk, ioth_t, keyp, eoff, sv, exp_keep):
                for (j0, nj, C) in plan:
                    eg = gp.tile([P, TARGET * H], bf16, tag="eg")
                    nc.gpsimd.indirect_dma_start(
                        out=eg[:, :nj * H],
                        out_offset=None, in_=FTELER.ap(),
                        in_offset=IOA(ap=ioth_t[:, j0:j0 + nj], axis=0),
                        element_offset=eoff)
                    ggg = nj // C
                    vb = lk[(j0, C)]
                    if exp_keep is not None:
                        et = exp_keep[:, j0 * H:(j0 + nj) * H]
                    else:
                        ett = gp.tile([P, TARGET * H], bf16, tag="et")
                        et = ett[:, :nj * H]
                    # e = el[oth] + er[key]  (key side broadcast over class)
                    nc.vector.tensor_tensor(
                        out=et.rearrange("p (g c e) -> p g c e", c=C, e=H),
                        in0=eg[:, :nj * H].rearrange(
                            "p (g c e) -> p g c e", c=C, e=H),
                        in1=keyp[:, vb * H:(vb + ggg) * H].rearrange(
                            "p (g e) -> p g e", e=H)[:, :, None, :]
                        .to_broadcast([P, ggg, C, H]),
                        op=AT.add)
                    lrt = gp.tile([P, TARGET * H], bf16, tag="lrt")
                    nc.vector.tensor_scalar_min(lrt[:, :nj * H], et, 0.0)
                    nc.vector.tensor_scalar_max(et, et, 0.0)
                    nc.vector.tensor_scalar_mul(lrt[:, :nj * H],
                                                lrt[:, :nj * H], NEG)
                    nc.vector.tensor_tensor(out=et, in0=et,
                                            in1=lrt[:, :nj * H], op=AT.add)
                    nc.scalar.activation(et, et, AF.Exp)
                    nc.vector.tensor_reduce(
                        out=sv[:, vb * H:(vb + ggg) * H].rearrange(
                            "p (g e) -> p g e", e=H),
                        in_=et.rearrange("p (g c e) -> p g e c", c=C, e=H),
                        axis=AX.X, op=AT.add)

            nc.vector.memset(SSt[:], 0.0)
            if STAGE >= 3:
                score_pass(plans, lk_s, ios, elp, 54, SSt, None)

            rssf = wp.tile([P, NVS * H], f32, tag="rssf")
            nc.vector.tensor_scalar_max(rssf[:], SSt[:], 1e-30)
            nc.vector.reciprocal(rssf[:], rssf[:])
            nc.scalar.activation(rssf[:], rssf[:], AF.Sqrt)
            # zero out entries whose sum was exactly 0 (pad / no-out-edge)
            ind = wp.tile([P, NVS * H], f32, tag="ind")
            nc.vector.tensor_scalar_mul(ind[:], SSt[:], 1e30)
            nc.vector.tensor_scalar_min(ind[:], ind[:], 1.0)
            nc.vector.tensor_tensor(out=rssf[:], in0=rssf[:], in1=ind[:],
                                    op=AT.mult)
            ssb = wp.tile([P, NVS * 4], bf16, tag="ssb")
            nc.vector.memset(ssb[:], 0.0)
            nc.vector.tensor_copy(
                ssb[:].rearrange("p (a e) -> p a e", e=4)[:, :, 0:H],
                rssf[:].rearrange("p (a e) -> p a e", e=H))
            nc.sync.dma_start(
                SSh.ap().rearrange("(p a) e -> p a e", p=P),
                ssb[:].rearrange("p (a e) -> p a e", e=4))
            if STAGE >= 3:
                if USE_COLL:
                    nc.gpsimd.collective_compute(
                        "AllGather", AT.bypass, ins=[SSh.ap()], outs=[SS.ap()],
                        replica_groups=rg)
                else:
                    nc.sync.dma_start(SS.ap()[0:NSVS, :], SSh.ap())
            nc.vector.memset(SDt[:], 0.0)
            if STAGE >= 2:
                score_pass(pland, lk_d, iod, erp, 51, SDt, EXP3)
            # rsd (local, stays in SBUF); rss -> bf16 -> AllGather -> scatter
            rsd = pp.tile([P, NVD * H], f32)
            nc.vector.tensor_scalar_max(rsd[:], SDt[:], 1e-30)
            nc.vector.reciprocal(rsd[:], rsd[:])
            nc.scalar.activation(rsd[:], rsd[:], AF.Sqrt)

            tc.strict_bb_all_engine_barrier()
            # load gathered rss, scatter into FTELER cols 48:51
            ssl = wp.tile([P, NCORES * NVS * 4], bf16, tag="ssl")
            nc.sync.dma_start(
                ssl[:].rearrange("p (c a e) -> p c a e", c=NCORES, e=4),
                SS.ap().rearrange("(c p a) e -> p c a e", c=NCORES, p=P))
            if STAGE >= 4:
                nc.gpsimd.indirect_dma_start(
                    out=FTELER.ap(),
                    out_offset=IOA(ap=isc[:], axis=0),
                    in_=ssl[:].rearrange("p (a e) -> p a e", e=4)[:, :, 0:H],
                    in_offset=None, element_offset=48)
            tc.strict_bb_all_engine_barrier()

            # ---- tree segment-sum helper ----
            def tree_sum(gv, ggg, C, sstr, vb):
                # gv: [P, nj*sstr] view base (bf16), slots of width sstr,
                # msg in els 0:HD. Reduce C slots per group into slot 0.
                cc = C
                while cc > 1:
                    half = cc // 2
                    if cc % 2 == 1:
                        nc.vector.tensor_tensor(
                            out=gv.rearrange("p (g c e) -> p g c e",
                                             c=C, e=sstr)[:, :, 0:1, 0:HD],
                            in0=gv.rearrange("p (g c e) -> p g c e",
                                             c=C, e=sstr)[:, :, 0:1, 0:HD],
                            in1=gv.rearrange("p (g c e) -> p g c e",
                                             c=C, e=sstr)[:, :, cc - 1:cc, 0:HD],
                            op=AT.add)
                    v4 = gv.rearrange("p (g c e) -> p g c e", c=C, e=sstr)
                    nc.vector.tensor_tensor(
                        out=v4[:, :, 0:half, 0:HD],
                        in0=v4[:, :, 0:half, 0:HD],
                        in1=v4[:, :, half:2 * half, 0:HD],
                        op=AT.add)
                    cc = half
                dst_t = (CURVA[:, vb * HD:(vb + ggg) * HD] if vb < B0
                         else CURVB[:, (vb - B0) * HD:(vb - B0 + ggg) * HD])
                nc.vector.tensor_copy(
                    dst_t.rearrange("p (g e) -> p g e", e=HD),
                    gv.rearrange("p (g c e) -> p g c e",
                                 c=C, e=sstr)[:, :, 0, 0:HD])

            # ---- hops ----
            # class chunks overwrite CURVA/CURVB groups [0, VGE); only the
            # zeros/spare tail needs zeroing, once
            nc.vector.memset(CURVB[:, (VGE - B0) * HD:], 0.0)
            KTOP = 0 if STAGE < 6 else (STAGE - 5 if STAGE < 9 else K)

            def send_seg(s0, s1, kk):
                curb = gp.tile([P, TARGET * HD], f8, tag="g8")
                src_t = (CURVA[:, s0 * HD:s1 * HD] if s1 <= B0
                         else CURVB[:, (s0 - B0) * HD:(s1 - B0) * HD])
                nc.scalar.activation(curb[:, :(s1 - s0) * HD],
                                     src_t, AF.Copy)
                nc.sync.dma_start(
                    CURSH.ap().rearrange("(p a) e -> p a e", p=P)[:, s0:s1, :],
                    curb[:, :(s1 - s0) * HD].rearrange(
                        "p (a e) -> p a e", e=HD))
                if USE_COLL:
                    nc.gpsimd.collective_compute(
                        "AllGather", AT.bypass,
                        ins=[CURSH.ap().rearrange(
                            "(p a) e -> p a e", p=P)[:, s0:s1, :]],
                        outs=[CURG[kk - 1].ap().rearrange(
                            "(c p a) e -> c p a e",
                            c=NCORES, p=P)[:, :, s0:s1, :]],
                        replica_groups=rg)
                else:
                    nc.sync.dma_start(
                        CURG[kk - 1].ap().rearrange(
                            "(c p a) e -> c p a e",
                            c=NCORES, p=P)[0, :, s0:s1, :],
                        CURSH.ap().rearrange(
                            "(p a) e -> p a e", p=P)[:, s0:s1, :])

            for k in range(1, KTOP + 1):
                tc.strict_bb_all_engine_barrier()
                nseg_sent = 0
                for (j0, nj, C) in pland:
                    ggg = nj // C
                    vb = lk_d[(j0, C)]
                    g = gp.tile([P, TARGET * 51], bf16, tag="gh")
                    if k == 1:
                        sstr = 51
                        nc.gpsimd.indirect_dma_start(
                            out=g[:, :nj * 51],
                            out_offset=None, in_=FTELER.ap(),
                            in_offset=IOA(ap=iod[:, j0:j0 + nj], axis=0))
                        gv = g[:, :nj * 51]
                        # A = exp * rsd(bcast) * rss(gathered); keep in EXP3
                        ev = EXP3[:, j0 * H:(j0 + nj) * H]
                        nc.vector.tensor_tensor(
                            out=ev.rearrange("p (g c e) -> p g c e",
                                             c=C, e=H),
                            in0=ev.rearrange("p (g c e) -> p g c e",
                                             c=C, e=H),
                            in1=rsd[:, vb * H:(vb + ggg) * H].rearrange(
                                "p (g e) -> p g e", e=H)[:, :, None, :]
                            .to_broadcast([P, ggg, C, H]),
                            op=AT.mult)
                        nc.vector.tensor_tensor(
                            out=ev.rearrange("p (j e) -> p j e", e=H),
                            in0=ev.rearrange("p (j e) -> p j e", e=H),
                            in1=gv.rearrange("p (j e) -> p j e",
                                             e=51)[:, :, 48:51],
                            op=AT.mult)
                    else:
                        sstr = HD
                        g8 = gp.tile([P, TARGET * HD], f8, tag="g8")
                        nc.gpsimd.indirect_dma_start(
                            out=g8[:, :nj * HD],
                            out_offset=None,
                            in_=CURG[k - 2].ap().rearrange(
                                "r e -> (r e)")[None, :],
                            in_offset=IOA(ap=icu[:, j0:j0 + nj], axis=1))
                        nc.scalar.activation(g[:, :nj * HD], g8[:, :nj * HD],
                                             AF.Copy)
                        gv = g[:, :nj * HD]
                    # msg *= A broadcast over d (packed-last 2-byte).
                    # For k==1 the 51-el slot factors as 17x3; els 48:51
                    # (rss) get multiplied too but are dead afterwards.
                    nc.vector.tensor_tensor(
                        out=gv.rearrange("p (j d h) -> p j d h",
                                         d=sstr // H, h=H),
                        in0=gv.rearrange("p (j d h) -> p j d h",
                                         d=sstr // H, h=H),
                        in1=EXP3[:, j0 * H:(j0 + nj) * H].rearrange(
                            "p (j e) -> p j e", e=H)[:, :, None, :]
                        .to_broadcast([P, nj, sstr // H, H]),
                        op=AT.mult)
                    tree_sum(gv, ggg, C, sstr, vb)
                    if k < K and nseg_sent == 0 and vb + ggg >= B0:
                        send_seg(0, B0, k)
                        nseg_sent = 1
                if k < K:
                    if nseg_sent == 0:
                        send_seg(0, B0, k)
                    send_seg(B0, NVD, k)
                feat_trans(t0[:, :B0 * HD], CURVA[:, :B0 * HD], k, B0)
                feat_trans(t0[:, B0 * HD:], CURVB[:], k, NVD - B0)
                nc.sync.dma_start(
                    HSTK[k - 1].ap().rearrange("(p a) e -> p a e", p=P),
                    t0[:].rearrange("p (a e) -> p a e", e=HD))

            # ---- final hop attention ----
            if STAGE < 9:
                dum = wp.tile([P, NVD * HD], f32, tag="t0")
                nc.vector.memset(dum[:], 1.0)
                nc.sync.dma_start(
                    out_t.ap().rearrange("(p a) e -> p a e", p=P),
                    dum[:].rearrange("p (a e) -> p a e", e=HD))
            else:
                tc.strict_bb_all_engine_barrier()
                lq = wp.tile([P, NVD * H], f32, tag="lq")
                hlv = hlr[:, 0:HD].rearrange(
                    "p (d h) -> p h d", h=H)[:, None, :, :] \
                    .to_broadcast([P, NVD, H, D])
                hrv = hlr[:, HD:2 * HD].rearrange(
                    "p (d h) -> p h d", h=H)[:, None, :, :] \
                    .to_broadcast([P, NVD, H, D])
                wk = wp.tile([P, NVD * HD], f32, tag="wk")
                nc.sync.dma_start(
                    wk[:].rearrange("p (a e) -> p a e", e=HD),
                    HQD.ap().rearrange("(p a) e -> p a e", p=P))
                nc.vector.tensor_tensor(
                    out=t0[:].rearrange("p (a d h) -> p a h d", h=H, d=D),
                    in0=wk[:].rearrange("p (a d h) -> p a h d", h=H, d=D),
                    in1=hlv, op=AT.mult)
                nc.vector.tensor_reduce(
                    out=lq[:].rearrange("p (a h) -> p a h", h=H),
                    in_=t0[:].rearrange("p (a d h) -> p a h d", h=H, d=D),
                    axis=AX.X, op=AT.add)
                # single pass: acc = sum_k wk*exp(lg_k); den = sum_k exp(lg_k)
                # (divide once at the end)
                ek = wp.tile([P, NVD * H], f32, tag="ek")
                ekn = wp.tile([P, NVD * H], f32, tag="ekn")
                den = wp.tile([P, NVD * H], f32, tag="den")
                acc = CURVA
                nc.vector.memset(acc[:], 0.0)
                nc.vector.memset(den[:], 0.0)
                for k in range(K):
                    nc.sync.dma_start(
                        wk[:].rearrange("p (a e) -> p a e", e=HD),
                        HSTK[k].ap().rearrange("(p a) e -> p a e", p=P))
                    nc.vector.tensor_tensor(
                        out=t0[:].rearrange("p (a d h) -> p a h d", h=H, d=D),
                        in0=wk[:].rearrange("p (a d h) -> p a h d", h=H, d=D),
                        in1=hrv, op=AT.mult)
                    nc.vector.tensor_reduce(
                        out=ek[:].rearrange("p (a h) -> p a h", h=H),
                        in_=t0[:].rearrange("p (a d h) -> p a h d", h=H, d=D),
                        axis=AX.X, op=AT.add)
                    nc.vector.tensor_tensor(out=ek[:], in0=ek[:], in1=lq[:],
                                            op=AT.add)
                    nc.vector.tensor_scalar_min(ekn[:], ek[:], 0.0)
                    nc.vector.tensor_scalar_max(ek[:], ek[:], 0.0)
                    nc.vector.tensor_scalar_mul(ekn[:], ekn[:], NEG)
                    nc.vector.tensor_tensor(out=ek[:], in0=ek[:], in1=ekn[:],
                                            op=AT.add)
                    nc.scalar.activation(ek[:], ek[:], AF.Exp)
                    nc.vector.tensor_tensor(out=den[:], in0=den[:], in1=ek[:],
                                            op=AT.add)
                    nc.vector.tensor_tensor(
                        out=t0[:].rearrange("p (a d h) -> p a d h", h=H, d=D),
                        in0=wk[:].rearrange("p (a d h) -> p a d h", h=H, d=D),
                        in1=ek[:].rearrange("p (a h) -> p a h",
                                            h=H)[:, :, None, :]
                        .to_broadcast([P, NVD, D, H]), op=AT.mult)
                    nc.vector.tensor_tensor(out=acc[:], in0=acc[:], in1=t0[:],
                                            op=AT.add)
                nc.vector.reciprocal(den[:], den[:])
                nc.vector.tensor_tensor(
                    out=acc[:].rearrange("p (a d h) -> p a d h", h=H, d=D),
                    in0=acc[:].rearrange("p (a d h) -> p a d h", h=H, d=D),
                    in1=den[:].rearrange("p (a h) -> p a h",
                                         h=H)[:, :, None, :]
                    .to_broadcast([P, NVD, D, H]), op=AT.mult)
                nc.sync.dma_start(
                    out_t.ap().rearrange("(p a) e -> p a e", p=P),
                    acc[:].rearrange("p (a e) -> p a e", e=HD))
    nc.compile()
    return nc


# ---------------------------------------------------------------- entry
def kernel(**inputs):
    feat = np.asarray(inputs["feat"], np.float32)
    src = np.asarray(inputs["src"]).astype(np.int64)
    dst = np.asarray(inputs["dst"]).astype(np.int64)
    fc_W = np.asarray(inputs["fc_W"], np.float32)
    attn_l = np.asarray(inputs["attn_l"], np.float32).reshape(H, D)
    attn_r = np.asarray(inputs["attn_r"], np.float32).reshape(H, D)
    hop_l = np.asarray(inputs["hop_attn_l"], np.float32).reshape(H, D)
    hop_r = np.asarray(inputs["hop_attn_r"], np.float32).reshape(H, D)
    pos = np.asarray(inputs["pos_emb"], np.float32)
    nsc = np.asarray(inputs["norm_scales"], np.float32)
    off = np.asarray(inputs["offsets"], np.float32)
    bias = np.asarray(inputs["bias"], np.float32).reshape(1, H, D)

    sides = host_prep(src, dst)
    sd, ssd = sides["d"], sides["s"]
    nc = build_nc(sd["NV"], sd["NTE"], sd["plan"],
                  ssd["NV"], ssd["NTE"], ssd["plan"])
    NVD, NVS = sd["NV"], ssd["NV"]

    # d-major permutation: column d*H + h holds (head h, dim d)
    perm = np.arange(HD).reshape(H, D).T.reshape(-1)  # (d-major) <- (h-major)
    W48 = fc_W.reshape(IN, H, D).transpose(0, 2, 1).reshape(IN, HD)
    Wl = np.einsum("ihd,hd->ih", fc_W.reshape(IN, H, D), attn_l)
    Wr = np.einsum("ihd,hd->ih", fc_W.reshape(IN, H, D), attn_r)
    W54 = np.concatenate([W48, Wl, Wr], 1).astype(ml_dtypes.bfloat16)

    featP = np.zeros((NP_, IN), np.float32)
    featP[:N] = feat
    featT = np.ascontiguousarray(featP.T).astype(ml_dtypes.bfloat16)
    sc = nsc.reshape(K + 1, HD)[:, perm]
    opv = (off.reshape(K + 1, HD) +
           pos[0].transpose(1, 0, 2).reshape(K + 1, HD))[:, perm]
    scales4 = np.tile(sc.reshape(1, -1), (P, 1)).astype(np.float32)
    offpos4 = np.tile(opv.reshape(1, -1), (P, 1)).astype(np.float32)
    hop2 = np.tile(np.concatenate(
        [hop_l.T.reshape(1, HD), hop_r.T.reshape(1, HD)], 1), (P, 1))

    in_maps = []
    for c in range(NCORES):
        cd, cs = sd["cores"][c], ssd["cores"][c]
        iq_d = np.zeros(P * NVD, np.int64)
        iq_d[cd["vrow"]] = np.arange(NS) + c * NS
        iq_s = np.zeros(P * NVS, np.int64)
        iq_s[cs["vrow"]] = np.arange(NS) + c * NS
        # scatter targets: SBUF slot (p, c2*NVS + a) holds SS row
        # c2*NSVS + p*NVS + a  -> node owned by c2 at that vrow (or pad row)
        iscat = np.full((P, NCORES * NVS), NP_, np.int64)
        for c2 in range(NCORES):
            vr = ssd["cores"][c2]["vrow"]   # node-local -> vrow
            pmat = vr // NVS
            amat = vr % NVS
            iscat[pmat, c2 * NVS + amat] = np.arange(NS) + c2 * NS
        in_maps.append(dict(
            featT=featT, W_in=W54,
            hop_lr=hop2.astype(np.float32),
            scales4=scales4, offpos4=offpos4,
            iOTHd=cd["ioth"], iOTHs=cs["ioth"],
            iCUR=(sd["vmap"][cd["ioth"]] * HD).astype(np.int32),
            iQd=iq_d.reshape(P, NVD).astype(np.int32),
            iQs=iq_s.reshape(P, NVS).astype(np.int32),
            iSCAT=iscat.astype(np.int32),
        ))

    outs = None
    exec_ns = None
    if os.environ.get("AGDN_HW", "") == "1":
        try:
            from concourse import bass_utils
            trace = os.environ.get("AGDN_TRACE", "") == "1"
            res = bass_utils.run_bass_kernel_spmd(
                nc, in_maps, core_ids=list(range(NCORES)), trace=trace)
            outs = [np.asarray(res.results[c]["out"]) for c in range(NCORES)]
            exec_ns = res.exec_time_ns
            if not all(np.isfinite(o).all() for o in outs):
                print("[kernel] HW returned non-finite values", flush=True)
                outs = None
        except Exception as e:
            print(f"[kernel] HW path failed: {type(e).__name__}", flush=True)
            outs = None
    if outs is None:
        from concourse.bass_interp import MultiCoreSim
        nworkers = 1 if os.environ.get("AGDN_TRACE", "") == "1" else NCORES
        sim = MultiCoreSim(nc, num_cores=NCORES, num_workers=nworkers,
                           trace=False, require_finite=False,
                           require_nnan=False)
        for c, core in enumerate(sim.cores.values()):
            for kk, vv in in_maps[c].items():
                core.tensor(kk)[:] = vv
        sim.simulate(check_with_hw=False)
        outs = [np.array(core.tensor("out")) for core in sim.cores.values()]
        if nworkers == 1:
            exec_ns = int(sim.global_time)
    if exec_ns:
        print(f"[kernel] exec_time_ns={exec_ns}", flush=True)
        try:
            with open("/tmp/agdn_exec_ns.txt", "w") as f:
                f.write(str(exec_ns))
        except OSError:
            pass
    full = np.concatenate(outs, 0)
    out = full[sd["vmap"][:N]].reshape(N, D, H).transpose(0, 2, 1)
    return np.ascontiguousarray(out + bias).astype(np.float32)


# revision 33
# speedup vs baseline: 1.1306x; 1.0158x over previous
"""AGDNConv (3-hop attention diffusion GNN) on 8 trn2 NeuronCores.

Sharding: edges partitioned by dst-owner (owner = dst // 12544); node tables
replicated (P0 matmul) or AllGathered per hop. Per-core segment sums use a
degree-class slot layout so they become tree/strided tensor_reduce ops.
Attention softmax uses the max-free identity
  a[e] = exp(e_e) * rsqrt(s_dst[dst_e]) * rsqrt(s_src[src_e]).

v2 layout decisions (all driven by the DMA descriptor cost model):
 - One bf16 node table FTELER [NP_+1, 58]: cols 0:48 ft in d-major (d,h)
   order, 48:51 rss (scattered in after the src-side AllGather), 51:54 el,
   54:57 er.  Hop-1 gathers cols 0:51 in ONE 102B descriptor per edge
   (ft + rss together); score passes gather only the 6B other-side score.
 - Key-side score values are broadcast per node group (no per-edge gather);
   per-node sums (SD) stay in SBUF, so the old "stage C" disappears.
 - No mask tensors: the pad row NP_ has el=er=-60 so exp==0 naturally, and
   rss==0 on the pad row kills pad contributions in hops 2/3.
 - d-major feature order makes every big DVE multiply packed-last (2x/4x
   DVE modes); segment sums are tree adds (packed) instead of strided 1x.

Execution: the axon terminal's NRT shim does not implement dynamic-AP
(indirect) DMA - it returns garbage data for gathers and wedges the device
on larger elements (verified empirically; see dev notes). kernel() therefore
runs the cycle-accurate MultiCoreSim (single worker, real collectives) by
default, which validates and times the exact Bass program. Set AGDN_HW=1 to
attempt real-HW execution via PJRT on terminals with a fixed runtime.
"""
import sys
sys.path.insert(0, "/opt/trn_rl_repo")
import os
import numpy as np
import ml_dtypes

USE_COLL = os.environ.get("AGDN_NOCOLL", "") != "1"
STAGE = int(os.environ.get("AGDN_STAGE", "9"))
P = 128
N = 100000
IN = 128
H = 3
D = 16
HD = 48
K = 3
NEG = 0.2
EPS = 1e-9
NCORES = 8
NS = 12544
NP_ = NCORES * NS
ROWW = 256         # FTELER row width (bf16 elements; 512B rows avoid the
                   # small-descriptor 2x latency multiplier on gathers)
PADV = -60.0       # pad-row el/er value: exp(leaky(-60+x)) ~ 0
CLASSES = [4, 8, 12, 16, 20, 24, 32, 40, 48, 64, 96, 128]
TARGET = 176       # chunk width target, in slots


# ---------------------------------------------------------------- host prep
def _pack_side(key_node, other_node, n_lo):
    loc = (key_node - n_lo).astype(np.int64)
    order = np.argsort(loc, kind="stable")
    loc_s = loc[order]
    other_s = other_node[order]
    deg = np.bincount(loc_s, minlength=NS)
    assert deg.max() <= CLASSES[-1], f"degree {deg.max()} exceeds max class"
    starts = np.concatenate([[0], np.cumsum(deg)[:-1]])
    cls_of = np.full(NS, -1, np.int64)
    lo = 0
    for ci, C in enumerate(CLASSES):
        cls_of[(deg > lo) & (deg <= C)] = ci
        lo = C
    members = [np.where(cls_of == ci)[0] for ci in range(len(CLASSES))]
    zeros = np.where(deg == 0)[0]
    return dict(members=members, zeros=zeros, deg=deg, starts=starts,
                other_s=other_s)


def _layout(counts_max, gz_max):
    G, vg0, je0, plan = [], [], [], []
    v, j = 0, 0
    for ci, C in enumerate(CLASSES):
        g = int(np.ceil(counts_max[ci] / P))
        G.append(g)
        vg0.append(v)
        je0.append(j)
        cols = g * C
        step = max(C, (TARGET // C) * C)
        s = 0
        while s < cols:
            w = min(step, cols - s)
            plan.append((j + s, w, C))
            s += w
        v += g
        j += g * C
    vg0.append(v)
    # zeros region + one spare always-unassigned group (its CUR rows stay 0)
    v += int(np.ceil(gz_max / P)) + 1
    return G, vg0, je0, max(v, 1), max(((j + 3) // 4) * 4, 4), plan


def _fill_core(pack, G, vg0, je0, NV, NTE, n_lo):
    ioth = np.full((P, NTE), NP_, np.int32)    # pad -> FTELER pad row
    vrow = np.full(NS, -1, np.int64)
    deg, starts, other_s = pack["deg"], pack["starts"], pack["other_s"]
    for ci, C in enumerate(CLASSES):
        mem = pack["members"][ci]
        g_c = max(G[ci], 1)
        for i, nl in enumerate(mem):
            g, p = i % g_c, i // g_c
            vrow[nl] = p * NV + vg0[ci] + g
            d, s0 = deg[nl], starts[nl]
            je = je0[ci] + g * C
            ioth[p, je:je + d] = other_s[s0:s0 + d]
    mem = pack["zeros"]
    gz = max(int(np.ceil(len(mem) / P)), 1)
    for i, nl in enumerate(mem):
        vrow[nl] = (i // gz) * NV + vg0[len(CLASSES)] + i % gz
    return ioth, vrow


def host_prep(src, dst):
    sides = {}
    for side, key, oth in (("d", dst, src), ("s", src, dst)):
        packs = []
        for c in range(NCORES):
            m = (key >= c * NS) & (key < (c + 1) * NS)
            packs.append(_pack_side(key[m], oth[m], c * NS))
        counts_max = np.max(
            np.array([[len(p) for p in pk["members"]] for pk in packs]), axis=0)
        gz_max = max(len(pk["zeros"]) for pk in packs)
        G, vg0, je0, NV, NTE, plan = _layout(counts_max, gz_max)
        cores, vmap = [], np.zeros(NP_ + 1, np.int64)
        for c in range(NCORES):
            ioth, vrow = _fill_core(packs[c], G, vg0, je0, NV, NTE, c * NS)
            cores.append(dict(ioth=ioth, vrow=vrow))
            vmap[c * NS:(c + 1) * NS] = c * (P * NV) + vrow
        # pad/ghost entries -> this side's guaranteed-zero row of core 0
        # (the spare group; never assigned by _fill_core)
        zrow = [c * (P * NV) + 127 * NV + (NV - 1) for c in range(NCORES)]
        vmap[vmap < 0] = zrow[0]
        sides[side] = dict(NV=NV, NTE=NTE, plan=plan, cores=cores, vmap=vmap,
                           zrow=zrow)
    return sides


def _vg_lookup(plan):
    lk, vg, last_C, cj, cvg = {}, 0, None, None, None
    for (j0, nj, C) in plan:
        if C != last_C:
            cj, cvg, last_C = j0, vg, C
        lk[(j0, C)] = cvg + (j0 - cj) // C
        vg = cvg + (j0 - cj + nj) // C
    return lk


# ---------------------------------------------------------------- device
def build_nc(NVD, NTED, pland, NVS, NTES, plans):
    import concourse.bass as bass
    import concourse.bacc as bacc
    import concourse.mybir as mybir
    import concourse.tile as tile
    f32, bf16, i32 = mybir.dt.float32, mybir.dt.bfloat16, mybir.dt.int32
    AT, AF, AX = mybir.AluOpType, mybir.ActivationFunctionType, mybir.AxisListType
    IOA = bass.IndirectOffsetOnAxis
    NSVD, NSVS = P * NVD, P * NVS
    lk_d, lk_s = _vg_lookup(pland), _vg_lookup(plans)
    VGE = max(lk_d[(j0, C)] + nj // C for (j0, nj, C) in pland)
    B0 = VGE
    for (j0, nj, C) in pland:
        vbe = lk_d[(j0, C)] + nj // C
        if vbe >= int(VGE * 0.65):
            B0 = vbe
            break

    nc = bacc.Bacc("TRN2", target_bir_lowering=False, debug=False,
                   num_devices=NCORES)
    featT = nc.dram_tensor("featT", [P, NP_], bf16, kind="ExternalInput")
    W_in = nc.dram_tensor("W_in", [P, 54], bf16, kind="ExternalInput")
    hop_lr = nc.dram_tensor("hop_lr", [P, 2 * HD], f32, kind="ExternalInput")
    scales4 = nc.dram_tensor("scales4", [P, (K + 1) * HD], f32, kind="ExternalInput")
    offpos4 = nc.dram_tensor("offpos4", [P, (K + 1) * HD], f32, kind="ExternalInput")
    iOTHd = nc.dram_tensor("iOTHd", [P, NTED], i32, kind="ExternalInput")
    iOTHs = nc.dram_tensor("iOTHs", [P, NTES], i32, kind="ExternalInput")
    iCUR = nc.dram_tensor("iCUR", [P, NTED], i32, kind="ExternalInput")
    iQd = nc.dram_tensor("iQd", [P, NVD], i32, kind="ExternalInput")
    iQs = nc.dram_tensor("iQs", [P, NVS], i32, kind="ExternalInput")
    iSCAT = nc.dram_tensor("iSCAT", [P, NCORES * NVS], i32, kind="ExternalInput")
    out_t = nc.dram_tensor("out", [NSVD, HD], f32, kind="ExternalOutput")

    FTELER = nc.dram_tensor("FTELER", [NP_ + 1, ROWW], bf16, kind="Internal")
    SSh = nc.dram_tensor("SSh", [NSVS, 4], bf16, kind="Internal")
    SS = nc.dram_tensor("SS", [NCORES * NSVS, 4], bf16, kind="Internal",
                        addr_space="Shared")
    f8 = mybir.dt.float8e4
    CURSH = nc.dram_tensor("CURSH", [NSVD, HD], f8, kind="Internal")
    CURG = [nc.dram_tensor(f"CURG{k}", [NCORES * NSVD, HD], f8,
                           kind="Internal", addr_space="Shared")
            for k in range(K - 1)]
    HSTK = [nc.dram_tensor(f"HSTK{k}", [NSVD, HD], f32, kind="Internal")
            for k in range(K)]
    HQD = nc.dram_tensor("HQD", [NSVD, HD], f32, kind="Internal")
    rg = [list(range(NCORES))]

    with tile.TileContext(nc) as tc:
        with tc.tile_pool(name="persist", bufs=1) as pp, \
             tc.tile_pool(name="work", bufs=1) as wp, \
             tc.tile_pool(name="gat", bufs=2) as gp, \
             tc.tile_pool(name="ps", bufs=2, space="PSUM") as psp:
            # ---- weights / constants / persistent index tiles ----
            wwa = pp.tile([P, 54], bf16)
            nc.sync.dma_start(wwa[:], W_in.ap())
            hlr = pp.tile([P, 2 * HD], f32)
            nc.sync.dma_start(hlr[:], hop_lr.ap())
            sc4 = pp.tile([P, (K + 1) * HD], f32)
            nc.sync.dma_start(sc4[:], scales4.ap())
            op4 = pp.tile([P, (K + 1) * HD], f32)
            nc.sync.dma_start(op4[:], offpos4.ap())
            epst = pp.tile([P, 1], f32)
            nc.vector.memset(epst[:], EPS)
            iod = pp.tile([P, NTED], i32)
            nc.sync.dma_start(iod[:], iOTHd.ap())
            ios = pp.tile([P, NTES], i32)
            nc.sync.dma_start(ios[:], iOTHs.ap())
            icu = pp.tile([P, NTED], i32)
            nc.sync.dma_start(icu[:], iCUR.ap())
            iqd = pp.tile([P, NVD], i32)
            nc.sync.dma_start(iqd[:], iQd.ap())
            iqs = pp.tile([P, NVS], i32)
            nc.sync.dma_start(iqs[:], iQs.ap())
            isc = pp.tile([P, NCORES * NVS], i32)
            nc.sync.dma_start(isc[:], iSCAT.ap())

            # ---- P0: replicated feat matmul -> FTELER rows ----
            GRP = 1024
            NSTG = 4
            stgs = []
            for b in range(NSTG):
                st = wp.tile([P, 8 * 57], bf16, tag=f"p0st{b}")
                nc.vector.memset(st[:], 0.0)
                stgs.append(st)
            for g in range(NP_ // GRP):
                fch = wp.tile([P, GRP], bf16, tag=f"fch{g % NSTG}")
                nc.sync.dma_start(fch[:], featT.ap()[:, g * GRP:(g + 1) * GRP])
                ps = psp.tile([P, 8 * 54], f32, tag=f"p0ps{g % 2}")
                for t in range(8):
                    nc.tensor.matmul(
                        out=ps[:, t * 54:(t + 1) * 54],
                        lhsT=fch[:, t * P:(t + 1) * P],
                        rhs=wwa[:], start=True, stop=True)
                sv3 = stgs[g % NSTG][:].rearrange("q (t e) -> q t e", e=57)
                pv3 = ps[:].rearrange("q (t e) -> q t e", e=54)
                # cols 48:51 (rss) are left stale; the scatter overwrites
                # them before any read
                nc.vector.tensor_copy(sv3[:, :, 0:HD], pv3[:, :, 0:HD])
                nc.vector.tensor_copy(sv3[:, :, 51:57], pv3[:, :, HD:54])
                nc.scalar.dma_start(
                    FTELER.ap()[g * GRP:(g + 1) * GRP, 0:57].rearrange(
                        "(t p) e -> p t e", t=8),
                    sv3)
            padr = wp.tile([1, 57], bf16, tag="padr")
            nc.vector.memset(padr[:], 0.0)
            nc.vector.memset(padr[:, 51:57], PADV)
            nc.sync.dma_start(FTELER.ap()[NP_:NP_ + 1, 0:57], padr[:])
            tc.strict_bb_all_engine_barrier()

            # ---- per-node key-side score values ----
            erp = pp.tile([P, NVD * H], bf16)
            nc.gpsimd.indirect_dma_start(
                out=erp[:],
                out_offset=None, in_=FTELER.ap(),
                in_offset=IOA(ap=iqd[:], axis=0), element_offset=54)
            elp = pp.tile([P, NVS * H], bf16)
            nc.gpsimd.indirect_dma_start(
                out=elp[:],
                out_offset=None, in_=FTELER.ap(),
                in_offset=IOA(ap=iqs[:], axis=0), element_offset=51)

            # ---- feat_trans (d-major layout: 48 = (16 d) x (3 h)) ----
            def feat_trans(dst_ap, src_ap, k, nv):
                # dst/src: [P, nv*HD] f32 views
                sv_ = src_ap.rearrange("p (a d h) -> p a h d", h=H, d=D)
                dv = dst_ap.rearrange("p (a d h) -> p a h d", h=H, d=D)
                m = wp.tile([P, nv * H], f32, tag="ftm")
                ms = wp.tile([P, nv * H], f32, tag="ftms")
                mv = m[:].rearrange("p (a h) -> p a h", h=H)
                nc.vector.tensor_reduce(out=mv, in_=sv_, axis=AX.X, op=AT.add)
                nc.vector.tensor_scalar_mul(m[:], m[:], 1.0 / D)
                nc.scalar.activation(dst_ap, src_ap, AF.Square)
                nc.vector.tensor_reduce(
                    out=ms[:].rearrange("p (a h) -> p a h", h=H),
                    in_=dv, axis=AX.X, op=AT.add)
                nc.vector.tensor_scalar_mul(ms[:], ms[:], 1.0 / D)
                mm = wp.tile([P, nv * H], f32, tag="ftmm")
                nc.vector.tensor_tensor(out=mm[:], in0=m[:], in1=m[:],
                                        op=AT.mult)
                nc.vector.tensor_tensor(out=ms[:], in0=ms[:], in1=mm[:],
                                        op=AT.subtract)
                nc.scalar.activation(ms[:], ms[:], AF.Sqrt, bias=epst[:])
                nc.vector.reciprocal(ms[:], ms[:])
                mb = mv[:, :, :, None].to_broadcast([P, nv, H, D])
                rb = ms[:].rearrange("p (a h) -> p a h", h=H)[:, :, :, None] \
                    .to_broadcast([P, nv, H, D])
                nc.vector.tensor_tensor(out=dv, in0=sv_, in1=mb, op=AT.subtract)
                nc.vector.tensor_tensor(out=dv, in0=dv, in1=rb, op=AT.mult)
                dv2 = dst_ap.rearrange("p (a e) -> p a e", e=HD)
                nc.vector.tensor_tensor(
                    out=dv2, in0=dv2,
                    in1=sc4[:, k * HD:(k + 1) * HD][:, None, :]
                    .to_broadcast([P, nv, HD]), op=AT.mult)
                nc.vector.tensor_tensor(
                    out=dv2, in0=dv2,
                    in1=op4[:, k * HD:(k + 1) * HD][:, None, :]
                    .to_broadcast([P, nv, HD]), op=AT.add)

            # ---- h_query -> HQD ----
            CURVA = pp.tile([P, NVD * HD], f32)
            CURVB = pp.tile([P, (NVD - B0) * HD], f32)
            t0 = wp.tile([P, NVD * HD], f32, tag="t0")
            if STAGE >= 5:
                gq = gp.tile([P, TARGET * 51], bf16, tag="gh")
                nc.gpsimd.indirect_dma_start(
                    out=gq[:, :NVD * HD],
                    out_offset=None, in_=FTELER.ap(),
                    in_offset=IOA(ap=iqd[:], axis=0))
                feat_trans(t0[:], gq[:, :NVD * HD], 0, NVD)
                nc.sync.dma_start(
                    HQD.ap().rearrange("(p a) e -> p a e", p=P),
                    t0[:].rearrange("p (a e) -> p a e", e=HD))

            # ---- score passes ----
            EXP3 = pp.tile([P, NTED * H], bf16)
            SDt = pp.tile([P, NVD * H], f32)
            SSt = pp.tile([P, NVS * H], f32)

            def score_pass(plan, lk, ioth_t, keyp, eoff, sv, exp_keep):
                for (j0, nj, C) in plan:
                    eg = gp.tile([P, TARGET * H], bf16, tag="eg")
                    nc.gpsimd.indirect_dma_start(
                        out=eg[:, :nj * H],
                        out_offset=None, in_=FTELER.ap(),
                        in_offset=IOA(ap=ioth_t[:, j0:j0 + nj], axis=0),
                        element_offset=eoff)
                    ggg = nj // C
                    vb = lk[(j0, C)]
                    if exp_keep is not None:
                        et = exp_keep[:, j0 * H:(j0 + nj) * H]
                    else:
                        ett = gp.tile([P, TARGET * H], bf16, tag="et")
                        et = ett[:, :nj * H]
                    # e = el[oth] + er[key]  (key side broadcast over class)
                    nc.vector.tensor_tensor(
                        out=et.rearrange("p (g c e) -> p g c e", c=C, e=H),
                        in0=eg[:, :nj * H].rearrange(
                            "p (g c e) -> p g c e", c=C, e=H),
                        in1=keyp[:, vb * H:(vb + ggg) * H].rearrange(
                            "p (g e) -> p g e", e=H)[:, :, None, :]
                        .to_broadcast([P, ggg, C, H]),
                        op=AT.add)
                    lrt = gp.tile([P, TARGET * H], bf16, tag="lrt")
                    nc.vector.tensor_scalar_min(lrt[:, :nj * H], et, 0.0)
                    nc.vector.tensor_scalar_max(et, et, 0.0)
                    nc.vector.tensor_scalar_mul(lrt[:, :nj * H],
                                                lrt[:, :nj * H], NEG)
                    nc.vector.tensor_tensor(out=et, in0=et,
                                            in1=lrt[:, :nj * H], op=AT.add)
                    nc.scalar.activation(et, et, AF.Exp)
                    nc.vector.tensor_reduce(
                        out=sv[:, vb * H:(vb + ggg) * H].rearrange(
                            "p (g e) -> p g e", e=H),
                        in_=et.rearrange("p (g c e) -> p g e c", c=C, e=H),
                        axis=AX.X, op=AT.add)

            nc.vector.memset(SSt[:], 0.0)
            if STAGE >= 3:
                score_pass(plans, lk_s, ios, elp, 54, SSt, None)

            rssf = wp.tile([P, NVS * H], f32, tag="rssf")
            nc.vector.tensor_scalar_max(rssf[:], SSt[:], 1e-30)
            nc.vector.reciprocal(rssf[:], rssf[:])
            nc.scalar.activation(rssf[:], rssf[:], AF.Sqrt)
            # zero out entries whose sum was exactly 0 (pad / no-out-edge)
            ind = wp.tile([P, NVS * H], f32, tag="ind")
            nc.vector.tensor_scalar_mul(ind[:], SSt[:], 1e30)
            nc.vector.tensor_scalar_min(ind[:], ind[:], 1.0)
            nc.vector.tensor_tensor(out=rssf[:], in0=rssf[:], in1=ind[:],
                                    op=AT.mult)
            ssb = wp.tile([P, NVS * 4], bf16, tag="ssb")
            nc.vector.memset(ssb[:], 0.0)
            nc.vector.tensor_copy(
                ssb[:].rearrange("p (a e) -> p a e", e=4)[:, :, 0:H],
                rssf[:].rearrange("p (a e) -> p a e", e=H))
            nc.sync.dma_start(
                SSh.ap().rearrange("(p a) e -> p a e", p=P),
                ssb[:].rearrange("p (a e) -> p a e", e=4))
            if STAGE >= 3:
                if USE_COLL:
                    nc.gpsimd.collective_compute(
                        "AllGather", AT.bypass, ins=[SSh.ap()], outs=[SS.ap()],
                        replica_groups=rg)
                else:
                    nc.sync.dma_start(SS.ap()[0:NSVS, :], SSh.ap())
            nc.vector.memset(SDt[:], 0.0)
            if STAGE >= 2:
                score_pass(pland, lk_d, iod, erp, 51, SDt, EXP3)
            # rsd (local, stays in SBUF); rss -> bf16 -> AllGather -> scatter
            rsd = pp.tile([P, NVD * H], f32)
            nc.vector.tensor_scalar_max(rsd[:], SDt[:], 1e-30)
            nc.vector.reciprocal(rsd[:], rsd[:])
            nc.scalar.activation(rsd[:], rsd[:], AF.Sqrt)

            tc.strict_bb_all_engine_barrier()
            # load gathered rss, scatter into FTELER cols 48:51
            ssl = wp.tile([P, NCORES * NVS * 4], bf16, tag="ssl")
            nc.sync.dma_start(
                ssl[:].rearrange("p (c a e) -> p c a e", c=NCORES, e=4),
                SS.ap().rearrange("(c p a) e -> p c a e", c=NCORES, p=P))
            if STAGE >= 4:
                nc.gpsimd.indirect_dma_start(
                    out=FTELER.ap(),
                    out_offset=IOA(ap=isc[:], axis=0),
                    in_=ssl[:].rearrange("p (a e) -> p a e", e=4)[:, :, 0:H],
                    in_offset=None, element_offset=48)
            tc.strict_bb_all_engine_barrier()

            # ---- tree segment-sum helper ----
            def tree_sum(gv, ggg, C, sstr, vb):
                # gv: [P, nj*sstr] view base (bf16), slots of width sstr,
                # msg in els 0:HD. Reduce C slots per group into slot 0;
                # the last add writes straight into the f32 accumulator.
                dst_t = (CURVA[:, vb * HD:(vb + ggg) * HD] if vb < B0
                         else CURVB[:, (vb - B0) * HD:(vb - B0 + ggg) * HD])
                cc = C
                while cc > 1:
                    half = cc // 2
                    v4 = gv.rearrange("p (g c e) -> p g c e", c=C, e=sstr)
                    if cc % 2 == 1:
                        nc.vector.tensor_tensor(
                            out=v4[:, :, 0:1, 0:HD],
                            in0=v4[:, :, 0:1, 0:HD],
                            in1=v4[:, :, cc - 1:cc, 0:HD],
                            op=AT.add)
                    if half == 1:
                        nc.vector.tensor_tensor(
                            out=dst_t.rearrange("p (g e) -> p g e", e=HD),
                            in0=v4[:, :, 0, 0:HD],
                            in1=v4[:, :, 1, 0:HD],
                            op=AT.add)
                    else:
                        nc.vector.tensor_tensor(
                            out=v4[:, :, 0:half, 0:HD],
                            in0=v4[:, :, 0:half, 0:HD],
                            in1=v4[:, :, half:2 * half, 0:HD],
                            op=AT.add)
                    cc = half

            # ---- hops ----
            # class chunks overwrite CURVA/CURVB groups [0, VGE); only the
            # zeros/spare tail needs zeroing, once
            nc.vector.memset(CURVB[:, (VGE - B0) * HD:], 0.0)
            KTOP = 0 if STAGE < 6 else (STAGE - 5 if STAGE < 9 else K)

            def send_seg(s0, s1, kk):
                curb = gp.tile([P, TARGET * HD], f8, tag="g8")
                src_t = (CURVA[:, s0 * HD:s1 * HD] if s1 <= B0
                         else CURVB[:, (s0 - B0) * HD:(s1 - B0) * HD])
                nc.scalar.activation(curb[:, :(s1 - s0) * HD],
                                     src_t, AF.Copy)
                nc.sync.dma_start(
                    CURSH.ap().rearrange("(p a) e -> p a e", p=P)[:, s0:s1, :],
                    curb[:, :(s1 - s0) * HD].rearrange(
                        "p (a e) -> p a e", e=HD))
                if USE_COLL:
                    nc.gpsimd.collective_compute(
                        "AllGather", AT.bypass,
                        ins=[CURSH.ap().rearrange(
                            "(p a) e -> p a e", p=P)[:, s0:s1, :]],
                        outs=[CURG[kk - 1].ap().rearrange(
                            "(c p a) e -> c p a e",
                            c=NCORES, p=P)[:, :, s0:s1, :]],
                        replica_groups=rg)
                else:
                    nc.sync.dma_start(
                        CURG[kk - 1].ap().rearrange(
                            "(c p a) e -> c p a e",
                            c=NCORES, p=P)[0, :, s0:s1, :],
                        CURSH.ap().rearrange(
                            "(p a) e -> p a e", p=P)[:, s0:s1, :])

            for k in range(1, KTOP + 1):
                tc.strict_bb_all_engine_barrier()
                nseg_sent = 0
                for (j0, nj, C) in pland:
                    ggg = nj // C
                    vb = lk_d[(j0, C)]
                    g = gp.tile([P, TARGET * 51], bf16, tag="gh")
                    if k == 1:
                        sstr = 51
                        nc.gpsimd.indirect_dma_start(
                            out=g[:, :nj * 51],
                            out_offset=None, in_=FTELER.ap(),
                            in_offset=IOA(ap=iod[:, j0:j0 + nj], axis=0))
                        gv = g[:, :nj * 51]
                        # A = exp * rsd(bcast) * rss(gathered); keep in EXP3
                        ev = EXP3[:, j0 * H:(j0 + nj) * H]
                        nc.vector.tensor_tensor(
                            out=ev.rearrange("p (g c e) -> p g c e",
                                             c=C, e=H),
                            in0=ev.rearrange("p (g c e) -> p g c e",
                                             c=C, e=H),
                            in1=rsd[:, vb * H:(vb + ggg) * H].rearrange(
                                "p (g e) -> p g e", e=H)[:, :, None, :]
                            .to_broadcast([P, ggg, C, H]),
                            op=AT.mult)
                        nc.vector.tensor_tensor(
                            out=ev.rearrange("p (j e) -> p j e", e=H),
                            in0=ev.rearrange("p (j e) -> p j e", e=H),
                            in1=gv.rearrange("p (j e) -> p j e",
                                             e=51)[:, :, 48:51],
                            op=AT.mult)
                    else:
                        sstr = HD
                        g8 = gp.tile([P, TARGET * HD], f8, tag="g8")
                        nc.gpsimd.indirect_dma_start(
                            out=g8[:, :nj * HD],
                            out_offset=None,
                            in_=CURG[k - 2].ap().rearrange(
                                "r e -> (r e)")[None, :],
                            in_offset=IOA(ap=icu[:, j0:j0 + nj], axis=1))
                        nc.scalar.activation(g[:, :nj * HD], g8[:, :nj * HD],
                                             AF.Copy)
                        gv = g[:, :nj * HD]
                    # msg *= A broadcast over d (packed-last 2-byte).
                    # For k==1 the 51-el slot factors as 17x3; els 48:51
                    # (rss) get multiplied too but are dead afterwards.
                    nc.vector.tensor_tensor(
                        out=gv.rearrange("p (j d h) -> p j d h",
                                         d=sstr // H, h=H)[:, :, 0:D, :],
                        in0=gv.rearrange("p (j d h) -> p j d h",
                                         d=sstr // H, h=H)[:, :, 0:D, :],
                        in1=EXP3[:, j0 * H:(j0 + nj) * H].rearrange(
                            "p (j e) -> p j e", e=H)[:, :, None, :]
                        .to_broadcast([P, nj, D, H]),
                        op=AT.mult)
                    tree_sum(gv, ggg, C, sstr, vb)
                    if k < K and nseg_sent == 0 and vb + ggg >= B0:
                        send_seg(0, B0, k)
                        nseg_sent = 1
                if k < K:
                    if nseg_sent == 0:
                        send_seg(0, B0, k)
                    send_seg(B0, NVD, k)
                feat_trans(t0[:, :B0 * HD], CURVA[:, :B0 * HD], k, B0)
                feat_trans(t0[:, B0 * HD:], CURVB[:], k, NVD - B0)
                nc.sync.dma_start(
                    HSTK[k - 1].ap().rearrange("(p a) e -> p a e", p=P),
                    t0[:].rearrange("p (a e) -> p a e", e=HD))

            # ---- final hop attention ----
            if STAGE < 9:
                dum = wp.tile([P, NVD * HD], f32, tag="t0")
                nc.vector.memset(dum[:], 1.0)
                nc.sync.dma_start(
                    out_t.ap().rearrange("(p a) e -> p a e", p=P),
                    dum[:].rearrange("p (a e) -> p a e", e=HD))
            else:
                tc.strict_bb_all_engine_barrier()
                lq = wp.tile([P, NVD * H], f32, tag="lq")
                hlv = hlr[:, 0:HD].rearrange(
                    "p (d h) -> p h d", h=H)[:, None, :, :] \
                    .to_broadcast([P, NVD, H, D])
                hrv = hlr[:, HD:2 * HD].rearrange(
                    "p (d h) -> p h d", h=H)[:, None, :, :] \
                    .to_broadcast([P, NVD, H, D])
                wk = wp.tile([P, NVD * HD], f32, tag="wk")
                nc.sync.dma_start(
                    wk[:].rearrange("p (a e) -> p a e", e=HD),
                    HQD.ap().rearrange("(p a) e -> p a e", p=P))
                nc.vector.tensor_tensor(
                    out=t0[:].rearrange("p (a d h) -> p a h d", h=H, d=D),
                    in0=wk[:].rearrange("p (a d h) -> p a h d", h=H, d=D),
                    in1=hlv, op=AT.mult)
                nc.vector.tensor_reduce(
                    out=lq[:].rearrange("p (a h) -> p a h", h=H),
                    in_=t0[:].rearrange("p (a d h) -> p a h d", h=H, d=D),
                    axis=AX.X, op=AT.add)
                # single pass: acc = sum_k wk*exp(lg_k); den = sum_k exp(lg_k)
                # (divide once at the end)
                ek = wp.tile([P, NVD * H], f32, tag="ek")
                ekn = wp.tile([P, NVD * H], f32, tag="ekn")
                den = wp.tile([P, NVD * H], f32, tag="den")
                acc = CURVA
                nc.vector.memset(acc[:], 0.0)
                nc.vector.memset(den[:], 0.0)
                for k in range(K):
                    nc.sync.dma_start(
                        wk[:].rearrange("p (a e) -> p a e", e=HD),
                        HSTK[k].ap().rearrange("(p a) e -> p a e", p=P))
                    nc.vector.tensor_tensor(
                        out=t0[:].rearrange("p (a d h) -> p a h d", h=H, d=D),
                        in0=wk[:].rearrange("p (a d h) -> p a h d", h=H, d=D),
                        in1=hrv, op=AT.mult)
                    nc.vector.tensor_reduce(
                        out=ek[:].rearrange("p (a h) -> p a h", h=H),
                        in_=t0[:].rearrange("p (a d h) -> p a h d", h=H, d=D),
                        axis=AX.X, op=AT.add)
                    nc.vector.tensor_tensor(out=ek[:], in0=ek[:], in1=lq[:],
                                            op=AT.add)
                    nc.vector.tensor_scalar_min(ekn[:], ek[:], 0.0)
                    nc.vector.tensor_scalar_max(ek[:], ek[:], 0.0)
                    nc.vector.tensor_scalar_mul(ekn[:], ekn[:], NEG)
                    nc.vector.tensor_tensor(out=ek[:], in0=ek[:], in1=ekn[:],
                                            op=AT.add)
                    nc.scalar.activation(ek[:], ek[:], AF.Exp)
                    nc.vector.tensor_tensor(out=den[:], in0=den[:], in1=ek[:],
                                            op=AT.add)
                    nc.vector.tensor_tensor(
                        out=t0[:].rearrange("p (a d h) -> p a d h", h=H, d=D),
                        in0=wk[:].rearrange("p (a d h) -> p a d h", h=H, d=D),
                        in1=ek[:].rearrange("p (a h) -> p a h",
                                            h=H)[:, :, None, :]
                        .to_broadcast([P, NVD, D, H]), op=AT.mult)
                    nc.vector.tensor_tensor(out=acc[:], in0=acc[:], in1=t0[:],
                                            op=AT.add)
                nc.vector.reciprocal(den[:], den[:])
                nc.vector.tensor_tensor(
                    out=acc[:].rearrange("p (a d h) -> p a d h", h=H, d=D),
                    in0=acc[:].rearrange("p (a d h) -> p a d h", h=H, d=D),
                    in1=den[:].rearrange("p (a h) -> p a h",
                                         h=H)[:, :, None, :]
                    .to_broadcast([P, NVD, D, H]), op=AT.mult)
                nc.sync.dma_start(
                    out_t.ap().rearrange("(p a) e -> p a e", p=P),
                    acc[:].rearrange("p (a e) -> p a e", e=HD))
    nc.compile()
    return nc


# ---------------------------------------------------------------- entry
def kernel(**inputs):
    feat = np.asarray(inputs["feat"], np.float32)
    src = np.asarray(inputs["src"]).astype(np.int64)
    dst = np.asarray(inputs["dst"]).astype(np.int64)
    fc_W = np.asarray(inputs["fc_W"], np.float32)
    attn_l = np.asarray(inputs["attn_l"], np.float32).reshape(H, D)
    attn_r = np.asarray(inputs["attn_r"], np.float32).reshape(H, D)
    hop_l = np.asarray(inputs["hop_attn_l"], np.float32).reshape(H, D)
    hop_r = np.asarray(inputs["hop_attn_r"], np.float32).reshape(H, D)
    pos = np.asarray(inputs["pos_emb"], np.float32)
    nsc = np.asarray(inputs["norm_scales"], np.float32)
    off = np.asarray(inputs["offsets"], np.float32)
    bias = np.asarray(inputs["bias"], np.float32).reshape(1, H, D)

    sides = host_prep(src, dst)
    sd, ssd = sides["d"], sides["s"]
    nc = build_nc(sd["NV"], sd["NTE"], sd["plan"],
                  ssd["NV"], ssd["NTE"], ssd["plan"])
    NVD, NVS = sd["NV"], ssd["NV"]

    # d-major permutation: column d*H + h holds (head h, dim d)
    perm = np.arange(HD).reshape(H, D).T.reshape(-1)  # (d-major) <- (h-major)
    W48 = fc_W.reshape(IN, H, D).transpose(0, 2, 1).reshape(IN, HD)
    Wl = np.einsum("ihd,hd->ih", fc_W.reshape(IN, H, D), attn_l)
    Wr = np.einsum("ihd,hd->ih", fc_W.reshape(IN, H, D), attn_r)
    W54 = np.concatenate([W48, Wl, Wr], 1).astype(ml_dtypes.bfloat16)

    featP = np.zeros((NP_, IN), np.float32)
    featP[:N] = feat
    featT = np.ascontiguousarray(featP.T).astype(ml_dtypes.bfloat16)
    sc = nsc.reshape(K + 1, HD)[:, perm]
    opv = (off.reshape(K + 1, HD) +
           pos[0].transpose(1, 0, 2).reshape(K + 1, HD))[:, perm]
    scales4 = np.tile(sc.reshape(1, -1), (P, 1)).astype(np.float32)
    offpos4 = np.tile(opv.reshape(1, -1), (P, 1)).astype(np.float32)
    hop2 = np.tile(np.concatenate(
        [hop_l.T.reshape(1, HD), hop_r.T.reshape(1, HD)], 1), (P, 1))

    in_maps = []
    for c in range(NCORES):
        cd, cs = sd["cores"][c], ssd["cores"][c]
        iq_d = np.zeros(P * NVD, np.int64)
        iq_d[cd["vrow"]] = np.arange(NS) + c * NS
        iq_s = np.zeros(P * NVS, np.int64)
        iq_s[cs["vrow"]] = np.arange(NS) + c * NS
        # scatter targets: SBUF slot (p, c2*NVS + a) holds SS row
        # c2*NSVS + p*NVS + a  -> node owned by c2 at that vrow (or pad row)
        iscat = np.full((P, NCORES * NVS), NP_, np.int64)
        for c2 in range(NCORES):
            vr = ssd["cores"][c2]["vrow"]   # node-local -> vrow
            pmat = vr // NVS
            amat = vr % NVS
            iscat[pmat, c2 * NVS + amat] = np.arange(NS) + c2 * NS
        in_maps.append(dict(
            featT=featT, W_in=W54,
            hop_lr=hop2.astype(np.float32),
            scales4=scales4, offpos4=offpos4,
            iOTHd=cd["ioth"], iOTHs=cs["ioth"],
            iCUR=(sd["vmap"][cd["ioth"]] * HD).astype(np.int32),
            iQd=iq_d.reshape(P, NVD).astype(np.int32),
            iQs=iq_s.reshape(P, NVS).astype(np.int32),
            iSCAT=iscat.astype(np.int32),
        ))

    outs = None
    exec_ns = None
    if os.environ.get("AGDN_HW", "") == "1":
        try:
            from concourse import bass_utils
            trace = os.environ.get("AGDN_TRACE", "") == "1"
            res = bass_utils.run_bass_kernel_spmd(
                nc, in_maps, core_ids=list(range(NCORES)), trace=trace)
            outs = [np.asarray(res.results[c]["out"]) for c in range(NCORES)]
            exec_ns = res.exec_time_ns
            if not all(np.isfinite(o).all() for o in outs):
                print("[kernel] HW returned non-finite values", flush=True)
                outs = None
        except Exception as e:
            print(f"[kernel] HW path failed: {type(e).__name__}", flush=True)
            outs = None
    if outs is None:
        from concourse.bass_interp import MultiCoreSim
        nworkers = 1 if os.environ.get("AGDN_TRACE", "") == "1" else NCORES
        sim = MultiCoreSim(nc, num_cores=NCORES, num_workers=nworkers,
                           trace=False, require_finite=False,
                           require_nnan=False)
        for c, core in enumerate(sim.cores.values()):
            for kk, vv in in_maps[c].items():
                core.tensor(kk)[:] = vv
        sim.simulate(check_with_hw=False)
        outs = [np.array(core.tensor("out")) for core in sim.cores.values()]
        if nworkers == 1:
            exec_ns = int(sim.global_time)
    if exec_ns:
        print(f"[kernel] exec_time_ns={exec_ns}", flush=True)
        try:
            with open("/tmp/agdn_exec_ns.txt", "w") as f:
                f.write(str(exec_ns))
        except OSError:
            pass
    full = np.concatenate(outs, 0)
    out = full[sd["vmap"][:N]].reshape(N, D, H).transpose(0, 2, 1)
    return np.ascontiguousarray(out + bias).astype(np.float32)
